# revision 1
# baseline (speedup 1.0000x reference)
"""Trainium2 Bass kernel for nn_MACBlock (segmented attention + GEGLU FFN).

Sharding: 8 cores = 2 batches x 4 segments of 512 queries. The segment mask
makes attention block-diagonal (plus a 32-token always-visible prefix derived
from pooled memory + persistent memory), so each core is fully independent:
no collectives.

Layout: activations are kept feature-major (x^T [dim, tokens]) on-chip, so
every matmul contraction dim lands on partitions with zero transposes.
Scores are computed key-major ([keys, queries]); softmax is max-free (scores
are small by construction); the softmax denominator comes from an all-ones
stationary operand accumulated into the same PSUM tile as P@V.
Matmuls run as float32r (full fp32 data, full PE rate at free-dim>=256).
"""

import sys

if "/opt/trn_rl_repo" not in sys.path:
    sys.path.insert(0, "/opt/trn_rl_repo")

import numpy as np

B, N, DIM = 2, 2048, 1024
HEADS, DH = 16, 64
SEG = 512
NPM = NM = 16
PFX = NPM + NM          # 32 prefix keys
DFF = 2730
MFF = 22                # padded dff chunks
DFFP = MFF * 128        # 2816
KO = 8                  # 1024 / 128
P = 128
NCORES = 8
EPS = 1.1920929e-07
NEG = -1.0e9

_CACHE = {}


def _f32r(ap):
    import concourse.mybir as mybir
    return ap.bitcast(mybir.dt.float32r)


def build_nc(reps=1):
    import concourse.bass as bass
    from concourse import bacc
    import concourse.tile as tile
    import concourse.mybir as mybir

    f32 = mybir.dt.float32
    AF = mybir.ActivationFunctionType
    OP = mybir.AluOpType
    AX = mybir.AxisListType

    nc = bacc.Bacc("TRN2", target_bir_lowering=False, debug=False)

    dp = nc.declare_dram_parameter
    xT_d = dp("xT", [DIM, SEG], f32, isOutput=False)
    mo_d = dp("mo", [N, DIM], f32, isOutput=False)
    cq_d = dp("cq", [P, SEG], f32, isOutput=False)
    sq_d = dp("sq", [P, SEG], f32, isOutput=False)
    ck_d = dp("ck", [P, SEG], f32, isOutput=False)
    sk_d = dp("sk", [P, SEG], f32, isOutput=False)
    mask_d = dp("maskD", [P, P], f32, isOutput=False)
    rmat_d = dp("rmat", [P, P], f32, isOutput=False)
    ones_d = dp("ones", [P, P], f32, isOutput=False)
    qkw_d = dp("qkw", [16, P, KO, P], f32, isOutput=False)
    kvw_d = dp("kvw", [2, KO, P, DIM], f32, isOutput=False)
    outw_d = dp("outw", [KO, P, KO, P], f32, isOutput=False)
    w1a_d = dp("w1a", [MFF, P, KO, P], f32, isOutput=False)
    w1g_d = dp("w1g", [MFF, P, KO, P], f32, isOutput=False)
    w2_d = dp("w2", [KO, P, MFF, P], f32, isOutput=False)
    mtw_d = dp("mtw", [KO, P, DIM], f32, isOutput=False)
    pmv_d = dp("pmv", [HEADS, NPM, DH], f32, isOutput=False)
    pmk_d = dp("pmk", [HEADS, DH, NPM], f32, isOutput=False)
    b1a_d = dp("b1a", [P, MFF], f32, isOutput=False)
    b1g_d = dp("b1g", [P, MFF], f32, isOutput=False)
    b2_d = dp("b2", [P, KO], f32, isOutput=False)
    anw_d = dp("anw", [P, KO], f32, isOutput=False)
    fnw_d = dp("fnw", [P, KO], f32, isOutput=False)
    mpnw_d = dp("mpnw", [1, DIM], f32, isOutput=False)
    yT_d = dp("yT", [DIM, SEG], f32, isOutput=True)

    def _emit(nc):
      with tile.TileContext(nc) as tc, \
            nc.allow_low_precision(reason="float32r matmul rounding"):
        from contextlib import ExitStack
        ctx = ExitStack()
        with ctx:
            persist = ctx.enter_context(tc.tile_pool(name="persist", bufs=1))
            wpool = ctx.enter_context(tc.tile_pool(name="wpool", bufs=3))
            kvpool = ctx.enter_context(tc.tile_pool(name="kvpool", bufs=2))
            w2pool = ctx.enter_context(tc.tile_pool(name="w2pool", bufs=2))
            mopool = ctx.enter_context(tc.tile_pool(name="mopool", bufs=2))
            rot = ctx.enter_context(tc.tile_pool(name="rot", bufs=2))
            epool = ctx.enter_context(tc.tile_pool(name="epool", bufs=2))
            pa = ctx.enter_context(tc.tile_pool(name="pa", bufs=4, space="PSUM"))
            psc = ctx.enter_context(tc.tile_pool(name="psc", bufs=2, space="PSUM"))
            pso = ctx.enter_context(tc.tile_pool(name="pso", bufs=2, space="PSUM"))

            cnt = [0]

            def pa_t():
                cnt[0] += 1
                return pa.tile([P, SEG], f32, tag="ps", name=f"pa{cnt[0]}")

            def psc_t():
                cnt[0] += 1
                return psc.tile([P, SEG], f32, tag="sc", name=f"sc{cnt[0]}")

            def pso_t():
                cnt[0] += 1
                return pso.tile([P, SEG], f32, tag="o", name=f"o{cnt[0]}")

            # ---------------- persistent SBUF tensors ----------------
            xT = persist.tile([P, KO, SEG], f32, tag="xT")       # x^T, later x1^T
            xnT = persist.tile([P, KO, SEG], f32, tag="xnT")     # xn^T, later xn1^T
            kT = persist.tile([P, KO, SEG], f32, tag="kT")       # roped k^T
            vA = persist.tile([P, 4, HEADS, DH], f32, tag="vA")  # v key-major
            vP = persist.tile([PFX, HEADS, DH], f32, tag="vP")   # prefix v rows
            kP = persist.tile([P, HEADS, PFX], f32, tag="kP")    # prefix k^T @64*(h%2)
            oA = persist.tile([P, KO, SEG], f32, tag="oA")       # attn o^T, later outT
            cq = persist.tile([P, SEG], f32, tag="cq")
            sq_ = persist.tile([P, SEG], f32, tag="sq")
            ck = persist.tile([P, SEG], f32, tag="ck")
            sk = persist.tile([P, SEG], f32, tag="sk")
            maskD = persist.tile([P, P], f32, tag="maskD")
            rmat = persist.tile([P, P], f32, tag="rmat")
            b1a = persist.tile([P, MFF], f32, tag="b1a")
            b1g = persist.tile([P, MFF], f32, tag="b1g")
            b2 = persist.tile([P, KO], f32, tag="b2")
            anw = persist.tile([P, KO], f32, tag="anw")
            fnw = persist.tile([P, KO], f32, tag="fnw")
            mpnw = persist.tile([1, DIM], f32, tag="mpnw")
            ones128 = persist.tile([P, 1], f32, tag="o128")      # lhsT K=128,M=1
            ones1x128 = persist.tile([1, P], f32, tag="o1x128")  # lhsT K=1,M=128
            ones16 = persist.tile([1, 16], f32, tag="o16")
            ones11 = persist.tile([1, 1], f32, tag="o11")
            onesPV = persist.tile([P, DH], f32, tag="oPV")       # sums stationary
            pooledT = persist.tile([P, KO], f32, tag="pooledT")
            memtokT = persist.tile([P, KO], f32, tag="memtokT")
            mrow = persist.tile([1, 3 * DIM], f32, tag="mrow")
            rrow = persist.tile([1, DIM], f32, tag="rrow")
            epsc = persist.tile([P, 1], f32, tag="epsc")
            zeroc = persist.tile([P, 1], f32, tag="zeroc")

            dma = nc.sync.dma_start
            dma(out=cq, in_=cq_d[:])
            dma(out=sq_, in_=sq_d[:])
            dma(out=ck, in_=ck_d[:])
            dma(out=sk, in_=sk_d[:])
            dma(out=maskD, in_=mask_d[:])
            dma(out=_f32r(rmat), in_=_f32r(rmat_d[:]))
            dma(out=b1a, in_=b1a_d[:])
            dma(out=b1g, in_=b1g_d[:])
            dma(out=b2, in_=b2_d[:])
            dma(out=anw, in_=anw_d[:])
            dma(out=fnw, in_=fnw_d[:])
            dma(out=mpnw, in_=mpnw_d[:])
            dma(out=_f32r(ones128), in_=_f32r(ones_d[:, 0:1]))
            dma(out=_f32r(ones1x128), in_=_f32r(ones_d[0:1, :]))
            dma(out=_f32r(ones16), in_=_f32r(ones_d[0:1, 0:16]))
            dma(out=_f32r(ones11), in_=_f32r(ones_d[0:1, 0:1]))
            dma(out=_f32r(onesPV), in_=_f32r(ones_d[:, 0:DH]))
            nc.vector.memset(epsc, EPS)
            nc.vector.memset(zeroc, 0.0)
            for h in range(HEADS):
                hb = DH * (h % 2)
                dma(out=_f32r(kP[hb:hb + DH, h, NPM:PFX]), in_=_f32r(pmk_d[h]))
                dma(out=_f32r(vP[NPM:PFX, h, :]), in_=_f32r(pmv_d[h]))

            if True:
              dma(out=xT, in_=xT_d.rearrange("(ko p) n -> p ko n", p=P))

              mm = nc.tensor.matmul

              def rmsnorm_into(dst, src, w_sb, sq_tag):
                  """dst[:,ko,:] = src[:,ko,:] * w[:,ko] * rsqrt(mean_dim(src^2)+eps)"""
                  ss = psc_t()  # [1,512] slice used
                  sq8 = persist.tile([P, KO, SEG], f32, tag=sq_tag, name="sq8")
                  for ko in range(KO):
                      nc.vector.tensor_mul(_f32r(sq8[:, ko, :]), src[:, ko, :],
                                           src[:, ko, :])
                      mm(ss[0:1, :], _f32r(ones128), _f32r(sq8[:, ko, :]),
                         start=(ko == 0), stop=(ko == KO - 1))
                  rr = rrow
                  nc.scalar.activation(_f32r(rr[:, 0:SEG]), ss[0:1, :], AF.Sqrt,
                                       bias=epsc[0:1], scale=1.0 / DIM)
                  nc.vector.reciprocal(_f32r(rr[:, SEG:2 * SEG]), rr[:, 0:SEG])
                  bc = pso_t()  # broadcast rstd over 128 partitions
                  mm(bc, ones1x128, rr[:, SEG:2 * SEG],
                     start=True, stop=True)
                  for ko in range(KO):
                      nc.vector.scalar_tensor_tensor(
                          out=_f32r(dst[:, ko, :]), in0=src[:, ko, :],
                          scalar=w_sb[:, ko:ko + 1], in1=bc,
                          op0=OP.mult, op1=OP.mult)

              # ---------------- attn rmsnorm ----------------
              rmsnorm_into(xnT, xT, anw, "big16")
              qT = persist.tile([P, KO, SEG], f32, tag="qT")       # roped,scaled q^T

              # ---------------- q/k projections + rope, interleaved with
              # ---------------- mem_out mean accumulation ----------------
              mean_ps = [psc_t(), psc_t()]   # two [1,512] accumulators (slices)

              def mo_mean_step(t):
                  mot = mopool.tile([P, DIM], f32, tag="mo", name="mot")
                  dma(out=_f32r(mot), in_=_f32r(mo_d[t * P:(t + 1) * P, :]))
                  for half in range(2):
                      mm(mean_ps[half][0:1, :], _f32r(ones128),
                         _f32r(mot[:, half * SEG:(half + 1) * SEG]),
                         start=(t == 0), stop=(t == 15))

              for m in range(16):
                  wt = wpool.tile([P, KO, P], f32, tag="w8")
                  dma(out=_f32r(wt), in_=_f32r(qkw_d[m]))
                  ps = pa_t()
                  for ko in range(KO):
                      mm(ps, _f32r(wt[:, ko]), _f32r(xnT[:, ko, :]),
                         start=(ko == 0), stop=(ko == KO - 1))
                  is_q = m < 8
                  c_t, s_t = (cq, sq_) if is_q else (ck, sk)
                  dst = qT if is_q else kT
                  ko_out = m % 8
                  qraw = rot.tile([P, SEG], f32, tag="ropeA")
                  nc.scalar.copy(_f32r(qraw), ps)
                  rps = pa_t()
                  mm(rps, _f32r(rmat), _f32r(qraw), start=True, stop=True)
                  At = rot.tile([P, SEG], f32, tag="ropeB")
                  nc.vector.tensor_mul(At, ps, c_t)
                  Bt = rot.tile([P, SEG], f32, tag="ropeA")
                  nc.vector.tensor_mul(Bt, rps, s_t)
                  nc.vector.tensor_add(_f32r(dst[:, ko_out, :]), At, Bt)
                  mo_mean_step(m)

              # ---------------- v projection (token-major) ----------------
              for half in range(2):
                  kvv = persist.tile([P, KO, SEG], f32, tag="big16")
                  for ko in range(KO):
                      dma(out=_f32r(kvv[:, ko, :]),
                          in_=_f32r(kvw_d[1, ko, :, half * SEG:(half + 1) * SEG]))
                  for tc_ in range(4):
                      ps = pa_t()
                      for ko in range(KO):
                          mm(ps, _f32r(xnT[:, ko, tc_ * P:(tc_ + 1) * P]),
                             _f32r(kvv[:, ko, :]),
                             start=(ko == 0), stop=(ko == KO - 1))
                      nc.vector.tensor_copy(
                          out=_f32r(vA[:, tc_, half * 8:(half + 1) * 8, :]),
                          in_=ps.rearrange("p (h d) -> p h d", d=DH))

              # ---------------- memory-context chain ----------------
              pooled_raw = mrow[:, 0:DIM]
              for half in range(2):
                  nc.scalar.activation(_f32r(pooled_raw[:, half * SEG:(half + 1) * SEG]),
                                       mean_ps[half][0:1, :], AF.Copy,
                                       scale=1.0 / N)
              sqr = mrow[:, DIM:2 * DIM]
              nc.vector.tensor_mul(_f32r(sqr), pooled_raw, pooled_raw)
              nc.vector.reduce_sum(_f32r(sqr[:, 0:1]), sqr, axis=AX.X)
              nc.scalar.activation(_f32r(sqr[:, 1:2]), sqr[:, 0:1], AF.Sqrt,
                                   bias=epsc[0:1], scale=1.0 / DIM)
              nc.vector.reciprocal(_f32r(sqr[:, 2:3]), sqr[:, 1:2])
              pooled = mrow[:, 2 * DIM:3 * DIM]
              nc.vector.scalar_tensor_tensor(out=_f32r(pooled), in0=pooled_raw,
                                             scalar=sqr[:, 2:3], in1=mpnw,
                                             op0=OP.mult, op1=OP.mult)
              # pooled^T via K=1 transpose matmuls
              pT = pa_t()
              for ko in range(KO):
                  mm(pT[:, ko:ko + 1], pooled[0:1, ko * P:(ko + 1) * P],
                     ones11, start=True, stop=True, skip_group_check=True)
              nc.vector.tensor_copy(out=_f32r(pooledT), in_=pT[:, 0:KO])
              # mem_tok row = pooled @ to_mem_tokens_w
              mt_ps = [psc_t(), psc_t()]
              for ko in range(KO):
                  mtw_t = kvpool.tile([P, DIM], f32, tag="kv")
                  dma(out=_f32r(mtw_t), in_=_f32r(mtw_d[ko]))
                  for half in range(2):
                      mm(mt_ps[half][0:1, :], _f32r(pooledT[:, ko:ko + 1]),
                         _f32r(mtw_t[:, half * SEG:(half + 1) * SEG]),
                         start=(ko == 0), stop=(ko == KO - 1))
              memtok = mrow[:, 0:DIM]
              for half in range(2):
                  nc.scalar.activation(_f32r(memtok[:, half * SEG:(half + 1) * SEG]),
                                       mt_ps[half][0:1, :], AF.Copy)
              mT = pa_t()
              for ko in range(KO):
                  mm(mT[:, ko:ko + 1], memtok[0:1, ko * P:(ko + 1) * P],
                     ones11, start=True, stop=True, skip_group_check=True)
              nc.vector.tensor_copy(out=_f32r(memtokT), in_=mT[:, 0:KO])
              # k_c / v_c rows = mem_tok @ Wk / Wv
              kcvc = []
              for c in range(2):
                  r_ps = [psc_t(), psc_t()]
                  for ko in range(KO):
                      kv_t = kvpool.tile([P, DIM], f32, tag="kv")
                      dma(out=_f32r(kv_t), in_=_f32r(kvw_d[c, ko]))
                      for half in range(2):
                          mm(r_ps[half][0:1, :], _f32r(memtokT[:, ko:ko + 1]),
                             _f32r(kv_t[:, half * SEG:(half + 1) * SEG]),
                             start=(ko == 0), stop=(ko == KO - 1))
                  row = mrow[:, DIM:2 * DIM] if c == 0 else mrow[:, 2 * DIM:3 * DIM]
                  for half in range(2):
                      nc.scalar.activation(_f32r(row[:, half * SEG:(half + 1) * SEG]),
                                           r_ps[half][0:1, :], AF.Copy)
                  kcvc.append(row)
              kc_row, vc_row = kcvc
              # k_extra^T into kP (16 identical columns per head)
              for j in range(KO):  # 2 heads per chunk
                  kx = pa_t()
                  mm(kx[:, 0:16], kc_row[0:1, j * P:(j + 1) * P],
                     ones16, start=True, stop=True, skip_group_check=True)
                  nc.vector.tensor_copy(out=_f32r(kP[0:DH, 2 * j, 0:NPM]),
                                        in_=kx[0:DH, 0:16])
                  nc.vector.tensor_copy(out=_f32r(kP[DH:P, 2 * j + 1, 0:NPM]),
                                        in_=kx[DH:P, 0:16])
              # v_extra rows into vP (16 identical rows per head)
              for half in range(2):
                  vx = pa_t()
                  mm(vx[0:16, :], ones16,
                     vc_row[0:1, half * SEG:(half + 1) * SEG],
                     start=True, stop=True, skip_group_check=True)
                  nc.vector.tensor_copy(
                      out=_f32r(vP[0:NPM, half * 8:(half + 1) * 8, :]),
                      in_=vx[0:16, :].rearrange("p (h d) -> p h d", d=DH))

              # ---------------- attention heads ----------------
              for h in range(HEADS):
                  ko_h, hf = h // 2, h % 2
                  qr = DH * hf
                  q_h = qT[qr:qr + DH, ko_h, :]
                  k_h = kT[qr:qr + DH, ko_h, :]
                  # prefix scores [32, 512]
                  scp = psc_t()
                  mm(scp[0:PFX, :], _f32r(kP[qr:qr + DH, h, :]), _f32r(q_h),
                     start=True, stop=True, skip_group_check=True)
                  eP = epool.tile([PFX, SEG], f32, tag="eP")
                  nc.scalar.activation(_f32r(eP), scp[0:PFX, :], AF.Exp,
                                       bias=zeroc[0:PFX])
                  eS = []
                  for c in range(4):
                      w = SEG - P * c
                      sc = psc_t()
                      mm(sc[:, 0:w], _f32r(k_h[:, c * P:(c + 1) * P]),
                         _f32r(q_h[:, c * P:]),
                         start=True, stop=True, skip_group_check=True)
                      et = epool.tile([P, w], f32, tag=("e0" if c < 2 else "e2"))
                      nc.scalar.activation(_f32r(et), sc[:, 0:w], AF.Exp,
                                           bias=zeroc)
                      nc.vector.tensor_mul(_f32r(et[:, 0:P]), et[:, 0:P], maskD)
                      eS.append(et)
                  # P@V and softmax denominator in separate base-0 PSUM tiles
                  po = pso_t()
                  sm = pso_t()
                  mm(po[0:DH, :], _f32r(vP[:, h, :]), _f32r(eP),
                     start=True, stop=False, skip_group_check=True)
                  mm(sm[0:DH, :], _f32r(onesPV[0:PFX, 0:DH]), _f32r(eP),
                     start=True, stop=False, skip_group_check=True)
                  for c in range(4):
                      w = SEG - P * c
                      last = c == 3
                      mm(po[0:DH, c * P:], _f32r(vA[:, c, h, :]), _f32r(eS[c]),
                         start=False, stop=last, skip_group_check=True)
                      mm(sm[0:DH, c * P:], _f32r(onesPV[:, 0:DH]), _f32r(eS[c]),
                         start=False, stop=last, skip_group_check=True)
                  rv = rot.tile([P, SEG], f32, tag="ropeB")
                  nc.vector.reciprocal(rv[0:DH, :], sm[0:DH, :])
                  nc.vector.tensor_mul(_f32r(oA[qr:qr + DH, ko_h, :]),
                                       po[0:DH, :], rv[0:DH, :])

              # ---------------- output projection + residual ----------------
              for m in range(KO):
                  wt = wpool.tile([P, KO, P], f32, tag="w8")
                  dma(out=_f32r(wt), in_=_f32r(outw_d[m]))
                  ps = pa_t()
                  for k in range(KO):
                      mm(ps, _f32r(wt[:, k]), _f32r(oA[:, k, :]),
                         start=(k == 0), stop=(k == KO - 1))
                  nc.vector.tensor_add(xT[:, m, :], ps, xT[:, m, :])  # x1, in place

              # ---------------- FFN ----------------
              rmsnorm_into(xnT, xT, fnw, "big16")  # xn1^T
              u_parts = [qT, kT]  # reuse dead slots as u storage
              u_c = persist.tile([P, 6, SEG], f32, tag="big16")

              def u_slice(k):
                  if k < 8:
                      return u_parts[0][:, k, :]
                  if k < 16:
                      return u_parts[1][:, k - 8, :]
                  return u_c[:, k - 16, :]

              for m in range(MFF):
                  wa = wpool.tile([P, KO, P], f32, tag="w8")
                  dma(out=_f32r(wa), in_=_f32r(w1a_d[m]))
                  wg = wpool.tile([P, KO, P], f32, tag="w8")
                  dma(out=_f32r(wg), in_=_f32r(w1g_d[m]))
                  psa = pa_t()
                  psg = pa_t()
                  for ko in range(KO):
                      mm(psa, _f32r(wa[:, ko]), _f32r(xnT[:, ko, :]),
                         start=(ko == 0), stop=(ko == KO - 1))
                      mm(psg, _f32r(wg[:, ko]), _f32r(xnT[:, ko, :]),
                         start=(ko == 0), stop=(ko == KO - 1))
                  sig = rot.tile([P, SEG], f32, tag="ropeA")
                  nc.scalar.activation(sig, psg, AF.Sigmoid,
                                       bias=b1g[:, m:m + 1], scale=1.0)
                  silu = rot.tile([P, SEG], f32, tag="ropeB")
                  nc.vector.scalar_tensor_tensor(
                      out=silu, in0=psg, scalar=b1g[:, m:m + 1],
                      in1=sig, op0=OP.add, op1=OP.mult)
                  nc.vector.scalar_tensor_tensor(
                      out=_f32r(u_slice(m)), in0=psa, scalar=b1a[:, m:m + 1],
                      in1=silu, op0=OP.add, op1=OP.mult)

              for o in range(KO):
                  ps = pa_t()
                  for half in range(2):
                      w2t = w2pool.tile([P, 11, P], f32, tag="w2")
                      dma(out=_f32r(w2t), in_=_f32r(w2_d[o][:, half * 11:(half + 1) * 11, :]))
                      for k2 in range(11):
                          k = half * 11 + k2
                          mm(ps, _f32r(w2t[:, k2]), _f32r(u_slice(k)),
                             start=(k == 0), stop=(k == MFF - 1))
                  outT = persist.tile([P, KO, SEG], f32, tag="vA",
                                      name=f"outT{o}")
                  nc.vector.scalar_tensor_tensor(
                      out=outT[:, o, :], in0=ps, scalar=b2[:, o:o + 1],
                      in1=xT[:, o, :], op0=OP.add, op1=OP.add)
                  dma(out=yT_d[o * P:(o + 1) * P, :], in_=outT[:, o, :])

    for _rep in range(reps):
        _emit(nc)
    nc.compile()
    return nc


# ======================= host-side preparation =======================

def _prep_shared(inputs):
    f32 = np.float32
    qkv = np.asarray(inputs["to_qkv_w"], f32)
    shared = {}
    shared["qkw"] = np.ascontiguousarray(
        qkv[:, :2048].reshape(KO, P, 16, P).transpose(2, 1, 0, 3))
    shared["kvw"] = np.ascontiguousarray(
        np.stack([qkv[:, 1024:2048], qkv[:, 2048:3072]])
        .reshape(2, KO, P, DIM))
    shared["outw"] = np.ascontiguousarray(
        np.asarray(inputs["to_out_w"], f32)
        .reshape(KO, P, KO, P).transpose(2, 1, 0, 3))
    w1 = np.asarray(inputs["ff_w1"], f32)
    w1a = np.zeros((DIM, DFFP), f32)
    w1g = np.zeros((DIM, DFFP), f32)
    w1a[:, :DFF] = w1[:, :DFF]
    w1g[:, :DFF] = w1[:, DFF:]
    shared["w1a"] = np.ascontiguousarray(
        w1a.reshape(KO, P, MFF, P).transpose(2, 1, 0, 3))
    shared["w1g"] = np.ascontiguousarray(
        w1g.reshape(KO, P, MFF, P).transpose(2, 1, 0, 3))
    w2 = np.zeros((DFFP, DIM), f32)
    w2[:DFF] = np.asarray(inputs["ff_w2"], f32)
    shared["w2"] = np.ascontiguousarray(
        w2.reshape(MFF, P, KO, P).transpose(2, 1, 0, 3))
    shared["mtw"] = np.ascontiguousarray(
        np.asarray(inputs["to_mem_tokens_w"], f32).reshape(KO, P, DIM))
    pm = np.asarray(inputs["persist_mem"], f32)
    shared["pmv"] = np.ascontiguousarray(pm)
    shared["pmk"] = np.ascontiguousarray(pm.transpose(0, 2, 1))
    b1 = np.asarray(inputs["ff_b1"], f32)
    b1a = np.zeros(DFFP, f32)
    b1g = np.zeros(DFFP, f32)
    b1a[:DFF] = b1[:DFF]
    b1g[:DFF] = b1[DFF:]
    shared["b1a"] = np.ascontiguousarray(b1a.reshape(MFF, P).T)
    shared["b1g"] = np.ascontiguousarray(b1g.reshape(MFF, P).T)
    shared["b2"] = np.ascontiguousarray(
        np.asarray(inputs["ff_b2"], f32).reshape(KO, P).T)
    shared["anw"] = np.ascontiguousarray(
        np.asarray(inputs["attn_norm_w"], f32).reshape(KO, P).T)
    shared["fnw"] = np.ascontiguousarray(
        np.asarray(inputs["ff_norm_w"], f32).reshape(KO, P).T)
    shared["mpnw"] = np.ascontiguousarray(
        np.asarray(inputs["mem_pool_norm_w"], f32).reshape(1, DIM))
    rl = np.zeros((P, P), f32)
    ii = np.arange(0, P, 2)
    rl[ii + 1, ii] = f32(-1.0)
    rl[ii, ii + 1] = f32(1.0)
    shared["rmat"] = rl
    shared["ones"] = np.ones((P, P), f32)
    shared["maskD"] = np.where(
        np.arange(P)[None, :] >= np.arange(P)[:, None], f32(1.0), f32(0.0)
    ).astype(f32)

    # rope tables, float32 math to match the reference
    pos = np.arange(N, dtype=f32)
    expo = (np.arange(0, DH, 2).astype(f32) / f32(DH)).astype(f32)
    inv = (f32(1.0) / np.power(f32(10000.0), expo)).astype(f32)
    ang = np.repeat(pos[:, None] * inv[None, :], 2, axis=1).astype(f32)
    cosf, sinf = np.cos(ang).astype(f32), np.sin(ang).astype(f32)
    scale = f32(DH ** -0.5)
    shared["_cos"], shared["_sin"], shared["_scale"] = cosf, sinf, scale
    return shared


def _prep_core(inputs, shared, b, s):
    f32 = np.float32
    x = np.asarray(inputs["x"], f32)
    mo = np.asarray(inputs["mem_out"], f32)
    cosf, sinf, scale = shared["_cos"], shared["_sin"], shared["_scale"]
    seg = slice(s * SEG, (s + 1) * SEG)
    ct = np.ascontiguousarray(np.tile(cosf[seg].T, (2, 1)))
    st = np.ascontiguousarray(np.tile(sinf[seg].T, (2, 1)))
    m = {k: v for k, v in shared.items() if not k.startswith("_")}
    m["xT"] = np.ascontiguousarray(x[b, seg].T)
    m["mo"] = np.ascontiguousarray(mo[b])
    m["cq"] = (ct * scale).astype(f32)
    m["sq"] = (st * scale).astype(f32)
    m["ck"] = ct
    m["sk"] = st
    return m


def _get_nc():
    if "nc" not in _CACHE:
        _CACHE["nc"] = build_nc()
    return _CACHE["nc"]


def kernel(**inputs) -> np.ndarray:
    nc = _get_nc()
    shared = _prep_shared(inputs)
    cores = [(b, s) for b in range(B) for s in range(4)]
    in_maps = [_prep_core(inputs, shared, b, s) for b, s in cores]
    from concourse import bass_utils
    import os
    res = bass_utils.run_bass_kernel_spmd(
        nc, in_maps, core_ids=list(range(NCORES)),
        trace=bool(os.environ.get("MAC_TRACE")))
    _CACHE["last_results"] = res
    out = np.empty((B, N, DIM), np.float32)
    for i, (b, s) in enumerate(cores):
        out[b, s * SEG:(s + 1) * SEG, :] = res.results[i]["yT"].T
    return out



# revision 3
# speedup vs baseline: 1.2335x; 1.2335x over previous
"""Trainium2 Bass kernel for nn_MACBlock (segmented attention + GEGLU FFN).

Sharding: 8 cores = 2 batches x 4 segments of 512 queries. The segment mask
makes attention block-diagonal (plus a 32-token always-visible prefix derived
from pooled memory + persistent memory), so each core is fully independent:
no collectives.

Layout: activations are kept feature-major (x^T [dim, tokens]) on-chip, so
every matmul contraction dim lands on partitions with zero transposes.
All heavy GEMMs run in bf16 (weights pre-cast on host, activations cast
on-chip): bf16 enables Fast Weight Load and avoids the fp32-HIGH power
throttle that halves the PE clock. PSUM accumulation stays fp32.
Scores are computed key-major ([keys, queries]); softmax is max-free; the
softmax denominator comes from a ones-column folded into the P@V stationary
operand (row DH of the same PSUM tile).
"""

import sys

if "/opt/trn_rl_repo" not in sys.path:
    sys.path.insert(0, "/opt/trn_rl_repo")

import numpy as np

B, N, DIM = 2, 2048, 1024
HEADS, DH = 16, 64
SEG = 512
NPM = NM = 16
PFX = NPM + NM          # 32 prefix keys
DFF = 2730
MFF = 22                # padded dff chunks
DFFP = MFF * 128        # 2816
KO = 8                  # 1024 / 128
P = 128
NCORES = 8
EPS = 1.1920929e-07

_CACHE = {}


def _f32r(ap):
    import concourse.mybir as mybir
    return ap.bitcast(mybir.dt.float32r)


def build_nc(reps=1):
    import concourse.bass as bass
    from concourse import bacc
    import concourse.tile as tile
    import concourse.mybir as mybir

    f32 = mybir.dt.float32
    bf16 = mybir.dt.bfloat16
    AF = mybir.ActivationFunctionType
    OP = mybir.AluOpType
    AX = mybir.AxisListType

    nc = bacc.Bacc("TRN2", target_bir_lowering=False, debug=False)

    dp = nc.declare_dram_parameter
    xT_d = dp("xT", [DIM, SEG], f32, isOutput=False)
    mo_d = dp("mo", [N, DIM], bf16, isOutput=False)
    cq_d = dp("cq", [P, SEG], f32, isOutput=False)
    sq_d = dp("sq", [P, SEG], f32, isOutput=False)
    ck_d = dp("ck", [P, SEG], f32, isOutput=False)
    sk_d = dp("sk", [P, SEG], f32, isOutput=False)
    mask_d = dp("maskD", [P, P], bf16, isOutput=False)
    rmat_d = dp("rmat", [P, P], bf16, isOutput=False)
    qkw_d = dp("qkw", [16, P, KO, P], bf16, isOutput=False)
    kvw_d = dp("kvw", [2, KO, P, DIM], bf16, isOutput=False)
    outw_d = dp("outw", [KO, P, KO, P], bf16, isOutput=False)
    w1a_d = dp("w1a", [MFF, P, KO, P], bf16, isOutput=False)
    w1g_d = dp("w1g", [MFF, P, KO, P], bf16, isOutput=False)
    w2_d = dp("w2", [KO, P, MFF, P], bf16, isOutput=False)
    mtw_d = dp("mtw", [KO, P, DIM], bf16, isOutput=False)
    pmv_d = dp("pmv", [HEADS, NPM, DH], bf16, isOutput=False)
    pmk_d = dp("pmk", [HEADS, DH, NPM], bf16, isOutput=False)
    b1a_d = dp("b1a", [P, MFF], f32, isOutput=False)
    b1g_d = dp("b1g", [P, MFF], f32, isOutput=False)
    b2_d = dp("b2", [P, KO], f32, isOutput=False)
    anw_d = dp("anw", [P, KO], f32, isOutput=False)
    fnw_d = dp("fnw", [P, KO], f32, isOutput=False)
    mpnw_d = dp("mpnw", [1, DIM], f32, isOutput=False)
    yT_d = dp("yT", [DIM, SEG], f32, isOutput=True)

    def _emit(nc):
      with tile.TileContext(nc) as tc, \
            nc.allow_low_precision(reason="bf16 matmul rounding"):
        from contextlib import ExitStack
        ctx = ExitStack()
        with ctx:
            persist = ctx.enter_context(tc.tile_pool(name="persist", bufs=1))
            wpool = ctx.enter_context(tc.tile_pool(name="wpool", bufs=3))
            kvpool = ctx.enter_context(tc.tile_pool(name="kvpool", bufs=2))
            w2pool = ctx.enter_context(tc.tile_pool(name="w2pool", bufs=2))
            mopool = ctx.enter_context(tc.tile_pool(name="mopool", bufs=2))
            rot = ctx.enter_context(tc.tile_pool(name="rot", bufs=2))
            epool = ctx.enter_context(tc.tile_pool(name="epool", bufs=2))
            pa = ctx.enter_context(tc.tile_pool(name="pa", bufs=4, space="PSUM"))
            psc = ctx.enter_context(tc.tile_pool(name="psc", bufs=2, space="PSUM"))
            pso = ctx.enter_context(tc.tile_pool(name="pso", bufs=2, space="PSUM"))

            cnt = [0]

            def pa_t():
                cnt[0] += 1
                return pa.tile([P, SEG], f32, tag="ps", name=f"pa{cnt[0]}")

            def psc_t():
                cnt[0] += 1
                return psc.tile([P, SEG], f32, tag="sc", name=f"sc{cnt[0]}")

            def pso_t():
                cnt[0] += 1
                return pso.tile([P, SEG], f32, tag="o", name=f"o{cnt[0]}")

            # ---------------- persistent SBUF tensors ----------------
            xT = persist.tile([P, KO, SEG], f32, tag="xT")       # x^T, later x1^T
            xnT = persist.tile([P, KO, SEG], bf16, tag="xnT")    # xn^T, later xn1^T
            kT = persist.tile([P, KO, SEG], bf16, tag="kT")      # roped k^T
            vA = persist.tile([P, 4, HEADS, DH + 1], bf16, tag="vA")  # v key-major
            vP = persist.tile([PFX, HEADS, DH + 1], bf16, tag="vP")   # prefix v rows
            kP = persist.tile([P, HEADS, PFX], bf16, tag="kP")   # prefix k^T @64*(h%2)
            oA = persist.tile([P, KO, SEG], bf16, tag="oA")      # attn o^T
            cq = persist.tile([P, SEG], f32, tag="cq")
            sq_ = persist.tile([P, SEG], f32, tag="sq")
            ck = persist.tile([P, SEG], f32, tag="ck")
            sk = persist.tile([P, SEG], f32, tag="sk")
            maskD = persist.tile([P, P], bf16, tag="maskD")
            rmat = persist.tile([P, P], bf16, tag="rmat")
            b1a = persist.tile([P, MFF], f32, tag="b1a")
            b1g = persist.tile([P, MFF], f32, tag="b1g")
            b2 = persist.tile([P, KO], f32, tag="b2")
            anw = persist.tile([P, KO], f32, tag="anw")
            fnw = persist.tile([P, KO], f32, tag="fnw")
            mpnw = persist.tile([1, DIM], f32, tag="mpnw")
            ones16 = persist.tile([1, 16], f32, tag="o16")
            ones11 = persist.tile([1, 1], f32, tag="o11")
            ones128b = persist.tile([P, 1], bf16, tag="o128b")   # lhsT K=128,M=1
            ones1xPb = persist.tile([1, P], bf16, tag="o1xPb")   # lhsT K=1,M=128
            pooledT = persist.tile([P, KO], bf16, tag="pooledT")
            memtokT = persist.tile([P, KO], bf16, tag="memtokT")
            mrow = persist.tile([1, 3 * DIM], f32, tag="mrow")
            rrow = persist.tile([1, 2 * SEG], bf16, tag="rrow")
            epsc = persist.tile([P, 1], f32, tag="epsc")
            zeroc = persist.tile([P, 1], f32, tag="zeroc")

            dma = nc.sync.dma_start
            dma(out=cq, in_=cq_d[:])
            dma(out=sq_, in_=sq_d[:])
            dma(out=ck, in_=ck_d[:])
            dma(out=sk, in_=sk_d[:])
            dma(out=maskD, in_=mask_d[:])
            dma(out=rmat, in_=rmat_d[:])
            dma(out=b1a, in_=b1a_d[:])
            dma(out=b1g, in_=b1g_d[:])
            dma(out=b2, in_=b2_d[:])
            dma(out=anw, in_=anw_d[:])
            dma(out=fnw, in_=fnw_d[:])
            dma(out=mpnw, in_=mpnw_d[:])
            nc.vector.memset(ones16, 1.0)
            nc.vector.memset(ones11, 1.0)
            nc.vector.memset(ones128b, 1.0)
            nc.vector.memset(ones1xPb, 1.0)
            nc.vector.memset(epsc, EPS)
            nc.vector.memset(zeroc, 0.0)
            # denominator ones-columns in the P@V stationary operands
            nc.vector.memset(vA[:, :, :, DH:DH + 1], 1.0)
            nc.vector.memset(vP[:, :, DH:DH + 1], 1.0)
            for h in range(HEADS):
                hb = DH * (h % 2)
                dma(out=kP[hb:hb + DH, h, NPM:PFX], in_=pmk_d[h])
                dma(out=vP[NPM:PFX, h, 0:DH], in_=pmv_d[h])

            if True:
              dma(out=xT, in_=xT_d.rearrange("(ko p) n -> p ko n", p=P))

              mm = nc.tensor.matmul

              def rmsnorm_into(dst, src, w_sb, sq_tag):
                  """dst[:,ko,:] = src[:,ko,:] * w[:,ko] * rsqrt(mean_dim(src^2)+eps)"""
                  ss = psc_t()  # [1,512] slice used
                  sq8 = persist.tile([P, KO, SEG], bf16, tag=sq_tag, name="sq8")
                  for ko in range(KO):
                      nc.vector.tensor_mul(sq8[:, ko, :], src[:, ko, :],
                                           src[:, ko, :])
                      mm(ss[0:1, :], ones128b, sq8[:, ko, :],
                         start=(ko == 0), stop=(ko == KO - 1))
                  nc.scalar.activation(rrow[:, 0:SEG], ss[0:1, :], AF.Sqrt,
                                       bias=epsc[0:1], scale=1.0 / DIM)
                  nc.vector.reciprocal(rrow[:, SEG:2 * SEG], rrow[:, 0:SEG])
                  bc = pso_t()  # broadcast rstd over 128 partitions
                  mm(bc, ones1xPb, rrow[:, SEG:2 * SEG],
                     start=True, stop=True)
                  for ko in range(KO):
                      nc.vector.scalar_tensor_tensor(
                          out=dst[:, ko, :], in0=src[:, ko, :],
                          scalar=w_sb[:, ko:ko + 1], in1=bc,
                          op0=OP.mult, op1=OP.mult)

              # ---------------- attn rmsnorm ----------------
              rmsnorm_into(xnT, xT, anw, "big16")
              qT = persist.tile([P, KO, SEG], bf16, tag="qT")    # roped,scaled q^T

              # ---------------- q/k projections + rope, interleaved with
              # ---------------- mem_out mean accumulation ----------------
              mean_ps = [psc_t(), psc_t()]   # two [1,512] accumulators (slices)

              def mo_mean_step(t):
                  mot = mopool.tile([P, DIM], bf16, tag="mo", name="mot")
                  dma(out=mot, in_=mo_d[t * P:(t + 1) * P, :])
                  for half in range(2):
                      mm(mean_ps[half][0:1, :], ones128b,
                         mot[:, half * SEG:(half + 1) * SEG],
                         start=(t == 0), stop=(t == 15))

              for m in range(16):
                  wt = wpool.tile([P, KO, P], bf16, tag="w8")
                  dma(out=wt, in_=qkw_d[m])
                  ps = pa_t()
                  for ko in range(KO):
                      mm(ps, wt[:, ko], xnT[:, ko, :],
                         start=(ko == 0), stop=(ko == KO - 1))
                  is_q = m < 8
                  c_t, s_t = (cq, sq_) if is_q else (ck, sk)
                  dst = qT if is_q else kT
                  ko_out = m % 8
                  qraw = rot.tile([P, SEG], bf16, tag="ropeA")
                  nc.scalar.copy(qraw, ps)
                  rps = pa_t()
                  mm(rps, rmat, qraw, start=True, stop=True)
                  At = rot.tile([P, SEG], bf16, tag="ropeB")
                  nc.vector.tensor_mul(At, ps, c_t)
                  Bt = rot.tile([P, SEG], bf16, tag="ropeA")
                  nc.vector.tensor_mul(Bt, rps, s_t)
                  nc.vector.tensor_add(dst[:, ko_out, :], At, Bt)
                  mo_mean_step(m)

              # ---------------- v projection (token-major) ----------------
              for half in range(2):
                  kvv = persist.tile([P, KO, SEG], bf16, tag="big16")
                  for ko in range(KO):
                      dma(out=kvv[:, ko, :],
                          in_=kvw_d[1, ko, :, half * SEG:(half + 1) * SEG])
                  for tc_ in range(4):
                      ps = pa_t()
                      for ko in range(KO):
                          mm(ps, xnT[:, ko, tc_ * P:(tc_ + 1) * P],
                             kvv[:, ko, :],
                             start=(ko == 0), stop=(ko == KO - 1))
                      nc.vector.tensor_copy(
                          out=vA[:, tc_, half * 8:(half + 1) * 8, 0:DH],
                          in_=ps.rearrange("p (h d) -> p h d", d=DH))

              # ---------------- memory-context chain ----------------
              pooled_raw = mrow[:, 0:DIM]
              for half in range(2):
                  nc.scalar.activation(_f32r(pooled_raw[:, half * SEG:(half + 1) * SEG]),
                                       mean_ps[half][0:1, :], AF.Copy,
                                       scale=1.0 / N)
              sqr = mrow[:, DIM:2 * DIM]
              nc.vector.tensor_mul(_f32r(sqr), pooled_raw, pooled_raw)
              nc.vector.reduce_sum(_f32r(sqr[:, 0:1]), sqr, axis=AX.X)
              nc.scalar.activation(_f32r(sqr[:, 1:2]), sqr[:, 0:1], AF.Sqrt,
                                   bias=epsc[0:1], scale=1.0 / DIM)
              nc.vector.reciprocal(_f32r(sqr[:, 2:3]), sqr[:, 1:2])
              pooled = mrow[:, 2 * DIM:3 * DIM]
              nc.vector.scalar_tensor_tensor(out=_f32r(pooled), in0=pooled_raw,
                                             scalar=sqr[:, 2:3], in1=mpnw,
                                             op0=OP.mult, op1=OP.mult)
              # pooled^T via K=1 transpose matmuls
              pT = pa_t()
              for ko in range(KO):
                  mm(pT[:, ko:ko + 1], pooled[0:1, ko * P:(ko + 1) * P],
                     ones11, start=True, stop=True, skip_group_check=True)
              nc.vector.tensor_copy(out=pooledT, in_=pT[:, 0:KO])
              # mem_tok row = pooled @ to_mem_tokens_w
              mt_ps = [psc_t(), psc_t()]
              for ko in range(KO):
                  mtw_t = kvpool.tile([P, DIM], bf16, tag="kv")
                  dma(out=mtw_t, in_=mtw_d[ko])
                  for half in range(2):
                      mm(mt_ps[half][0:1, :], pooledT[:, ko:ko + 1],
                         mtw_t[:, half * SEG:(half + 1) * SEG],
                         start=(ko == 0), stop=(ko == KO - 1))
              memtok = mrow[:, 0:DIM]
              for half in range(2):
                  nc.scalar.activation(_f32r(memtok[:, half * SEG:(half + 1) * SEG]),
                                       mt_ps[half][0:1, :], AF.Copy)
              mT = pa_t()
              for ko in range(KO):
                  mm(mT[:, ko:ko + 1], memtok[0:1, ko * P:(ko + 1) * P],
                     ones11, start=True, stop=True, skip_group_check=True)
              nc.vector.tensor_copy(out=memtokT, in_=mT[:, 0:KO])
              # k_c / v_c rows = mem_tok @ Wk / Wv
              kcvc = []
              for c in range(2):
                  r_ps = [psc_t(), psc_t()]
                  for ko in range(KO):
                      kv_t = kvpool.tile([P, DIM], bf16, tag="kv")
                      dma(out=kv_t, in_=kvw_d[c, ko])
                      for half in range(2):
                          mm(r_ps[half][0:1, :], memtokT[:, ko:ko + 1],
                             kv_t[:, half * SEG:(half + 1) * SEG],
                             start=(ko == 0), stop=(ko == KO - 1))
                  row = mrow[:, DIM:2 * DIM] if c == 0 else mrow[:, 2 * DIM:3 * DIM]
                  for half in range(2):
                      nc.scalar.activation(_f32r(row[:, half * SEG:(half + 1) * SEG]),
                                           r_ps[half][0:1, :], AF.Copy)
                  kcvc.append(row)
              kc_row, vc_row = kcvc
              # k_extra^T into kP (16 identical columns per head)
              for j in range(KO):  # 2 heads per chunk
                  kx = pa_t()
                  mm(kx[:, 0:16], _f32r(kc_row[0:1, j * P:(j + 1) * P]),
                     _f32r(ones16), start=True, stop=True, skip_group_check=True)
                  nc.vector.tensor_copy(out=kP[0:DH, 2 * j, 0:NPM],
                                        in_=kx[0:DH, 0:16])
                  nc.vector.tensor_copy(out=kP[DH:P, 2 * j + 1, 0:NPM],
                                        in_=kx[DH:P, 0:16])
              # v_extra rows into vP (16 identical rows per head)
              for half in range(2):
                  vx = pa_t()
                  mm(vx[0:16, :], _f32r(ones16),
                     _f32r(vc_row[0:1, half * SEG:(half + 1) * SEG]),
                     start=True, stop=True, skip_group_check=True)
                  nc.vector.tensor_copy(
                      out=vP[0:NPM, half * 8:(half + 1) * 8, 0:DH],
                      in_=vx[0:16, :].rearrange("p (h d) -> p h d", d=DH))

              # ---------------- attention heads ----------------
              for h in range(HEADS):
                  ko_h, hf = h // 2, h % 2
                  qr = DH * hf
                  q_h = qT[qr:qr + DH, ko_h, :]
                  k_h = kT[qr:qr + DH, ko_h, :]
                  # prefix scores [32, 512]
                  scp = psc_t()
                  mm(scp[0:PFX, :], kP[qr:qr + DH, h, :], q_h,
                     start=True, stop=True, skip_group_check=True)
                  eP = epool.tile([PFX, SEG], bf16, tag="eP")
                  nc.scalar.activation(eP, scp[0:PFX, :], AF.Exp,
                                       bias=zeroc[0:PFX])
                  eS = []
                  for c in range(4):
                      w = SEG - P * c
                      sc = psc_t()
                      mm(sc[:, 0:w], k_h[:, c * P:(c + 1) * P],
                         q_h[:, c * P:],
                         start=True, stop=True, skip_group_check=True)
                      et = epool.tile([P, w], bf16, tag=("e0" if c < 2 else "e2"))
                      nc.scalar.activation(et, sc[:, 0:w], AF.Exp,
                                           bias=zeroc)
                      nc.vector.tensor_mul(et[:, 0:P], et[:, 0:P], maskD)
                      eS.append(et)
                  # P@V with ones-column: row DH = softmax denominator
                  po = pso_t()
                  mm(po[0:DH + 1, :], vP[:, h, :], eP,
                     start=True, stop=False, skip_group_check=True)
                  for c in range(4):
                      last = c == 3
                      mm(po[0:DH + 1, c * P:], vA[:, c, h, :], eS[c],
                         start=False, stop=last, skip_group_check=True)
                  rv = rot.tile([1, SEG], bf16, tag="rv")
                  nc.vector.reciprocal(rv, po[DH:DH + 1, :])
                  bcp = pso_t()  # broadcast 1/denom over DH partitions
                  mm(bcp[0:DH, :], ones1xPb[0:1, 0:DH], rv,
                     start=True, stop=True, skip_group_check=True)
                  bcs = epool.tile([DH, SEG], bf16, tag="bcs")
                  nc.scalar.copy(bcs, bcp[0:DH, :])
                  nc.vector.tensor_mul(oA[qr:qr + DH, ko_h, :],
                                       po[0:DH, :], bcs)

              # ---------------- output projection + residual ----------------
              for m in range(KO):
                  wt = wpool.tile([P, KO, P], bf16, tag="w8")
                  dma(out=wt, in_=outw_d[m])
                  ps = pa_t()
                  for k in range(KO):
                      mm(ps, wt[:, k], oA[:, k, :],
                         start=(k == 0), stop=(k == KO - 1))
                  nc.vector.tensor_add(xT[:, m, :], ps, xT[:, m, :])  # x1

              # ---------------- FFN ----------------
              rmsnorm_into(xnT, xT, fnw, "big16")  # xn1^T
              u_parts = [qT, kT]  # reuse dead slots as u storage
              u_c = persist.tile([P, 6, SEG], bf16, tag="big16")

              def u_slice(k):
                  if k < 8:
                      return u_parts[0][:, k, :]
                  if k < 16:
                      return u_parts[1][:, k - 8, :]
                  return u_c[:, k - 16, :]

              for m in range(MFF):
                  wa = wpool.tile([P, KO, P], bf16, tag="w8")
                  dma(out=wa, in_=w1a_d[m])
                  wg = wpool.tile([P, KO, P], bf16, tag="w8")
                  dma(out=wg, in_=w1g_d[m])
                  psa = pa_t()
                  psg = pa_t()
                  for ko in range(KO):
                      mm(psa, wa[:, ko], xnT[:, ko, :],
                         start=(ko == 0), stop=(ko == KO - 1))
                      mm(psg, wg[:, ko], xnT[:, ko, :],
                         start=(ko == 0), stop=(ko == KO - 1))
                  sig = rot.tile([P, SEG], f32, tag="ropeA")
                  nc.scalar.activation(sig, psg, AF.Sigmoid,
                                       bias=b1g[:, m:m + 1], scale=1.0)
                  silu = rot.tile([P, SEG], f32, tag="ropeB")
                  nc.vector.scalar_tensor_tensor(
                      out=silu, in0=psg, scalar=b1g[:, m:m + 1],
                      in1=sig, op0=OP.add, op1=OP.mult)
                  nc.vector.scalar_tensor_tensor(
                      out=u_slice(m), in0=psa, scalar=b1a[:, m:m + 1],
                      in1=silu, op0=OP.add, op1=OP.mult)

              for o in range(KO):
                  ps = pa_t()
                  for half in range(2):
                      w2t = w2pool.tile([P, 11, P], bf16, tag="w2")
                      dma(out=w2t, in_=w2_d[o][:, half * 11:(half + 1) * 11, :])
                      for k2 in range(11):
                          k = half * 11 + k2
                          mm(ps, w2t[:, k2], u_slice(k),
                             start=(k == 0), stop=(k == MFF - 1))
                  outT = persist.tile([P, KO, SEG], f32, tag="outT",
                                      name=f"outT{o}")
                  nc.vector.scalar_tensor_tensor(
                      out=outT[:, o, :], in0=ps, scalar=b2[:, o:o + 1],
                      in1=xT[:, o, :], op0=OP.add, op1=OP.add)
                  dma(out=yT_d[o * P:(o + 1) * P, :], in_=outT[:, o, :])

    for _rep in range(reps):
        _emit(nc)
    nc.compile()
    return nc


# ======================= host-side preparation =======================

def _prep_shared(inputs):
    import ml_dtypes
    f32 = np.float32
    bf16 = ml_dtypes.bfloat16
    qkv = np.asarray(inputs["to_qkv_w"], f32)
    shared = {}
    shared["qkw"] = np.ascontiguousarray(
        qkv[:, :2048].reshape(KO, P, 16, P).transpose(2, 1, 0, 3)).astype(bf16)
    shared["kvw"] = np.ascontiguousarray(
        np.stack([qkv[:, 1024:2048], qkv[:, 2048:3072]])
        .reshape(2, KO, P, DIM)).astype(bf16)
    shared["outw"] = np.ascontiguousarray(
        np.asarray(inputs["to_out_w"], f32)
        .reshape(KO, P, KO, P).transpose(2, 1, 0, 3)).astype(bf16)
    w1 = np.asarray(inputs["ff_w1"], f32)
    w1a = np.zeros((DIM, DFFP), f32)
    w1g = np.zeros((DIM, DFFP), f32)
    w1a[:, :DFF] = w1[:, :DFF]
    w1g[:, :DFF] = w1[:, DFF:]
    shared["w1a"] = np.ascontiguousarray(
        w1a.reshape(KO, P, MFF, P).transpose(2, 1, 0, 3)).astype(bf16)
    shared["w1g"] = np.ascontiguousarray(
        w1g.reshape(KO, P, MFF, P).transpose(2, 1, 0, 3)).astype(bf16)
    w2 = np.zeros((DFFP, DIM), f32)
    w2[:DFF] = np.asarray(inputs["ff_w2"], f32)
    shared["w2"] = np.ascontiguousarray(
        w2.reshape(MFF, P, KO, P).transpose(2, 1, 0, 3)).astype(bf16)
    shared["mtw"] = np.ascontiguousarray(
        np.asarray(inputs["to_mem_tokens_w"], f32).reshape(KO, P, DIM)).astype(bf16)
    pm = np.asarray(inputs["persist_mem"], f32)
    shared["pmv"] = np.ascontiguousarray(pm).astype(bf16)
    shared["pmk"] = np.ascontiguousarray(pm.transpose(0, 2, 1)).astype(bf16)
    b1 = np.asarray(inputs["ff_b1"], f32)
    b1a = np.zeros(DFFP, f32)
    b1g = np.zeros(DFFP, f32)
    b1a[:DFF] = b1[:DFF]
    b1g[:DFF] = b1[DFF:]
    shared["b1a"] = np.ascontiguousarray(b1a.reshape(MFF, P).T)
    shared["b1g"] = np.ascontiguousarray(b1g.reshape(MFF, P).T)
    shared["b2"] = np.ascontiguousarray(
        np.asarray(inputs["ff_b2"], f32).reshape(KO, P).T)
    shared["anw"] = np.ascontiguousarray(
        np.asarray(inputs["attn_norm_w"], f32).reshape(KO, P).T)
    shared["fnw"] = np.ascontiguousarray(
        np.asarray(inputs["ff_norm_w"], f32).reshape(KO, P).T)
    shared["mpnw"] = np.ascontiguousarray(
        np.asarray(inputs["mem_pool_norm_w"], f32).reshape(1, DIM))
    rl = np.zeros((P, P), f32)
    ii = np.arange(0, P, 2)
    rl[ii + 1, ii] = f32(-1.0)
    rl[ii, ii + 1] = f32(1.0)
    shared["rmat"] = rl.astype(bf16)
    shared["maskD"] = np.where(
        np.arange(P)[None, :] >= np.arange(P)[:, None], f32(1.0), f32(0.0)
    ).astype(bf16)

    # per-batch bf16 mem_out
    mo = np.asarray(inputs["mem_out"], f32)
    shared["_mo"] = [np.ascontiguousarray(mo[b]).astype(bf16) for b in range(B)]

    # rope tables, float32 math to match the reference
    pos = np.arange(N, dtype=f32)
    expo = (np.arange(0, DH, 2).astype(f32) / f32(DH)).astype(f32)
    inv = (f32(1.0) / np.power(f32(10000.0), expo)).astype(f32)
    ang = np.repeat(pos[:, None] * inv[None, :], 2, axis=1).astype(f32)
    cosf, sinf = np.cos(ang).astype(f32), np.sin(ang).astype(f32)
    scale = f32(DH ** -0.5)
    shared["_cos"], shared["_sin"], shared["_scale"] = cosf, sinf, scale
    return shared


def _prep_core(inputs, shared, b, s):
    f32 = np.float32
    x = np.asarray(inputs["x"], f32)
    cosf, sinf, scale = shared["_cos"], shared["_sin"], shared["_scale"]
    seg = slice(s * SEG, (s + 1) * SEG)
    ct = np.ascontiguousarray(np.tile(cosf[seg].T, (2, 1)))
    st = np.ascontiguousarray(np.tile(sinf[seg].T, (2, 1)))
    m = {k: v for k, v in shared.items() if not k.startswith("_")}
    m["xT"] = np.ascontiguousarray(x[b, seg].T)
    m["mo"] = shared["_mo"][b]
    m["cq"] = (ct * scale).astype(f32)
    m["sq"] = (st * scale).astype(f32)
    m["ck"] = ct
    m["sk"] = st
    return m


def _get_nc():
    if "nc" not in _CACHE:
        _CACHE["nc"] = build_nc()
    return _CACHE["nc"]


def kernel(**inputs) -> np.ndarray:
    nc = _get_nc()
    shared = _prep_shared(inputs)
    cores = [(b, s) for b in range(B) for s in range(4)]
    in_maps = [_prep_core(inputs, shared, b, s) for b, s in cores]
    from concourse import bass_utils
    import os
    res = bass_utils.run_bass_kernel_spmd(
        nc, in_maps, core_ids=list(range(NCORES)),
        trace=bool(os.environ.get("MAC_TRACE")))
    _CACHE["last_results"] = res
    out = np.empty((B, N, DIM), np.float32)
    for i, (b, s) in enumerate(cores):
        out[b, s * SEG:(s + 1) * SEG, :] = res.results[i]["yT"].T
    return out


# revision 10
# speedup vs baseline: 1.3098x; 1.0619x over previous
"""Trainium2 Bass kernel for nn_MACBlock (segmented attention + GEGLU FFN).

Sharding: 8 cores = 2 batches x 4 segments of 512 queries. The segment mask
makes attention block-diagonal (plus a 32-token always-visible prefix derived
from pooled memory + persistent memory), so each core is fully independent:
no collectives.

Layout: activations are kept feature-major (x^T [dim, tokens]) on-chip, so
every matmul contraction dim lands on partitions with zero transposes.
All heavy GEMMs run in bf16 (weights pre-cast on host, activations cast
on-chip): bf16 enables Fast Weight Load and avoids the fp32-HIGH power
throttle that halves the PE clock. PSUM accumulation stays fp32.
Scores are computed key-major ([keys, queries]); softmax is max-free; the
softmax denominator comes from a ones-column folded into the P@V stationary
operand (row DH of the same PSUM tile).
"""

import sys

if "/opt/trn_rl_repo" not in sys.path:
    sys.path.insert(0, "/opt/trn_rl_repo")

import numpy as np

B, N, DIM = 2, 2048, 1024
HEADS, DH = 16, 64
SEG = 512
NPM = NM = 16
PFX = NPM + NM          # 32 prefix keys
DFF = 2730
MFF = 22                # padded dff chunks
DFFP = MFF * 128        # 2816
KO = 8                  # 1024 / 128
P = 128
NCORES = 8
EPS = 1.1920929e-07

_CACHE = {}


def _f32r(ap):
    import concourse.mybir as mybir
    return ap.bitcast(mybir.dt.float32r)


def build_nc(reps=1):
    import concourse.bass as bass
    from concourse import bacc
    import concourse.tile as tile
    import concourse.mybir as mybir

    f32 = mybir.dt.float32
    bf16 = mybir.dt.bfloat16
    AF = mybir.ActivationFunctionType
    OP = mybir.AluOpType
    AX = mybir.AxisListType

    nc = bacc.Bacc("TRN2", target_bir_lowering=False, debug=False)

    dp = nc.declare_dram_parameter
    xT_d = dp("xT", [DIM, SEG], f32, isOutput=False)
    mo_d = dp("mo", [N, DIM], bf16, isOutput=False)
    cq_d = dp("cq", [P, SEG], f32, isOutput=False)
    sq_d = dp("sq", [P, SEG], f32, isOutput=False)
    ck_d = dp("ck", [P, SEG], f32, isOutput=False)
    sk_d = dp("sk", [P, SEG], f32, isOutput=False)
    mask_d = dp("maskD", [P, P], bf16, isOutput=False)
    rmat_d = dp("rmat", [P, P], bf16, isOutput=False)
    qkw_d = dp("qkw", [16, P, KO, P], bf16, isOutput=False)
    kvw_d = dp("kvw", [2, KO, P, DIM], bf16, isOutput=False)
    outw_d = dp("outw", [KO, P, KO, P], bf16, isOutput=False)
    w1a_d = dp("w1a", [MFF, P, KO, P], bf16, isOutput=False)
    w1g_d = dp("w1g", [MFF, P, KO, P], bf16, isOutput=False)
    w2_d = dp("w2", [KO, P, MFF, P], bf16, isOutput=False)
    mtw_d = dp("mtw", [KO, P, DIM], bf16, isOutput=False)
    pmv_d = dp("pmv", [HEADS, NPM, DH], bf16, isOutput=False)
    pmk_d = dp("pmk", [HEADS, DH, NPM], bf16, isOutput=False)
    b1a_d = dp("b1a", [P, MFF], f32, isOutput=False)
    b1g_d = dp("b1g", [P, MFF], f32, isOutput=False)
    b2_d = dp("b2", [P, KO], f32, isOutput=False)
    anw_d = dp("anw", [P, KO], f32, isOutput=False)
    fnw_d = dp("fnw", [P, KO], f32, isOutput=False)
    mpnw_d = dp("mpnw", [1, DIM], f32, isOutput=False)
    yT_d = dp("yT", [DIM, SEG], f32, isOutput=True)

    def _emit(nc):
      with tile.TileContext(nc) as tc, \
            nc.allow_low_precision(reason="bf16 matmul rounding"):
        from contextlib import ExitStack
        ctx = ExitStack()
        with ctx:
            persist = ctx.enter_context(tc.tile_pool(name="persist", bufs=1))
            wpool = ctx.enter_context(tc.tile_pool(name="wpool", bufs=3))
            kvpool = ctx.enter_context(tc.tile_pool(name="kvpool", bufs=2))
            w2pool = ctx.enter_context(tc.tile_pool(name="w2pool", bufs=2))
            mopool = ctx.enter_context(tc.tile_pool(name="mopool", bufs=2))
            rot = ctx.enter_context(tc.tile_pool(name="rot", bufs=2))
            epool = ctx.enter_context(tc.tile_pool(name="epool", bufs=3))
            pa = ctx.enter_context(tc.tile_pool(name="pa", bufs=4, space="PSUM"))
            psc = ctx.enter_context(tc.tile_pool(name="psc", bufs=2, space="PSUM"))
            pso = ctx.enter_context(tc.tile_pool(name="pso", bufs=2, space="PSUM"))

            cnt = [0]

            def pa_t():
                cnt[0] += 1
                return pa.tile([P, SEG], f32, tag="ps", name=f"pa{cnt[0]}")

            def psc_t():
                cnt[0] += 1
                return psc.tile([P, SEG], f32, tag="sc", name=f"sc{cnt[0]}")

            def pso_t():
                cnt[0] += 1
                return pso.tile([P, SEG], f32, tag="o", name=f"o{cnt[0]}")

            # ---------------- persistent SBUF tensors ----------------
            xT = persist.tile([P, KO, SEG], f32, tag="xT")       # x^T, later x1^T
            xnT = persist.tile([P, KO, SEG], bf16, tag="xnT")    # xn^T, later xn1^T
            kT = persist.tile([P, KO, SEG], bf16, tag="kT")      # roped k^T
            vA = persist.tile([P, 4, HEADS, DH + 1], bf16, tag="vA")  # v key-major
            vP = persist.tile([PFX, HEADS, DH + 1], bf16, tag="vP")   # prefix v rows
            kP = persist.tile([P, HEADS, PFX], bf16, tag="kP")   # prefix k^T @64*(h%2)
            oA = persist.tile([P, KO, SEG], bf16, tag="oA")      # attn o^T
            cq = persist.tile([P, SEG], f32, tag="cq")
            sq_ = persist.tile([P, SEG], f32, tag="sq")
            ck = persist.tile([P, SEG], f32, tag="ck")
            sk = persist.tile([P, SEG], f32, tag="sk")
            maskD = persist.tile([P, P], bf16, tag="maskD")
            rmat = persist.tile([P, P], bf16, tag="rmat")
            b1a = persist.tile([P, MFF], f32, tag="b1a")
            b1g = persist.tile([P, MFF], f32, tag="b1g")
            b2 = persist.tile([P, KO], f32, tag="b2")
            anw = persist.tile([P, KO], f32, tag="anw")
            fnw = persist.tile([P, KO], f32, tag="fnw")
            mpnw = persist.tile([1, DIM], f32, tag="mpnw")
            ones16 = persist.tile([1, 16], f32, tag="o16")
            ones11 = persist.tile([1, 1], f32, tag="o11")
            ones128b = persist.tile([P, 1], bf16, tag="o128b")   # lhsT K=128,M=1
            ones1xPb = persist.tile([1, P], bf16, tag="o1xPb")   # lhsT K=1,M=128
            ones1xP = persist.tile([1, P], f32, tag="o1xP")      # f32 variant
            pooledT = persist.tile([P, KO], bf16, tag="pooledT")
            memtokT = persist.tile([P, KO], bf16, tag="memtokT")
            mrow = persist.tile([1, 3 * DIM], f32, tag="mrow")
            rrow = persist.tile([1, 2 * SEG], bf16, tag="rrow")
            epsc = persist.tile([P, 1], f32, tag="epsc")
            zeroc = persist.tile([P, 1], f32, tag="zeroc")

            dma = nc.sync.dma_start
            dma(out=cq, in_=cq_d[:])
            dma(out=sq_, in_=sq_d[:])
            dma(out=ck, in_=ck_d[:])
            dma(out=sk, in_=sk_d[:])
            dma(out=maskD, in_=mask_d[:])
            dma(out=rmat, in_=rmat_d[:])
            dma(out=b1a, in_=b1a_d[:])
            dma(out=b1g, in_=b1g_d[:])
            dma(out=b2, in_=b2_d[:])
            dma(out=anw, in_=anw_d[:])
            dma(out=fnw, in_=fnw_d[:])
            dma(out=mpnw, in_=mpnw_d[:])
            nc.vector.memset(ones16, 1.0)
            nc.vector.memset(ones11, 1.0)
            nc.vector.memset(ones128b, 1.0)
            nc.vector.memset(ones1xPb, 1.0)
            nc.vector.memset(ones1xP, 1.0)
            nc.vector.memset(epsc, EPS)
            nc.vector.memset(zeroc, 0.0)
            # denominator ones-columns in the P@V stationary operands
            nc.vector.memset(vA[:, :, :, DH:DH + 1], 1.0)
            nc.vector.memset(vP[:, :, DH:DH + 1], 1.0)
            for h in range(HEADS):
                hb = DH * (h % 2)
                dma(out=kP[hb:hb + DH, h, NPM:PFX], in_=pmk_d[h])
                dma(out=vP[NPM:PFX, h, 0:DH], in_=pmv_d[h])

            if True:
              dma(out=xT, in_=xT_d.rearrange("(ko p) n -> p ko n", p=P))

              mm = nc.tensor.matmul

              def rmsnorm_into(dst, src, w_sb, sq_tag):
                  """dst[:,ko,:] = src[:,ko,:] * w[:,ko] * rsqrt(mean_dim(src^2)+eps)"""
                  ss = psc_t()  # [1,512] slice used
                  sq8 = persist.tile([P, KO, SEG], bf16, tag=sq_tag, name="sq8")
                  for ko in range(KO):
                      nc.vector.tensor_mul(sq8[:, ko, :], src[:, ko, :],
                                           src[:, ko, :])
                      mm(ss[0:1, :], ones128b, sq8[:, ko, :],
                         start=(ko == 0), stop=(ko == KO - 1))
                  nc.scalar.activation(rrow[:, 0:SEG], ss[0:1, :], AF.Sqrt,
                                       bias=epsc[0:1], scale=1.0 / DIM)
                  nc.vector.reciprocal(rrow[:, SEG:2 * SEG], rrow[:, 0:SEG])
                  bc = pso_t()  # broadcast rstd over 128 partitions
                  mm(bc, ones1xPb, rrow[:, SEG:2 * SEG],
                     start=True, stop=True)
                  for ko in range(KO):
                      nc.vector.scalar_tensor_tensor(
                          out=dst[:, ko, :], in0=src[:, ko, :],
                          scalar=w_sb[:, ko:ko + 1], in1=bc,
                          op0=OP.mult, op1=OP.mult)

              # ---------------- attn rmsnorm ----------------
              rmsnorm_into(xnT, xT, anw, "big16")
              qT = persist.tile([P, KO, SEG], bf16, tag="qT")    # roped,scaled q^T

              # ---------------- q/k projections + rope, interleaved with
              # ---------------- mem_out mean accumulation ----------------
              mean_ps = [psc_t(), psc_t()]   # two [1,512] accumulators (slices)

              def mo_mean_step(t):
                  mot = mopool.tile([P, DIM], bf16, tag="mo", name="mot")
                  dma(out=mot, in_=mo_d[t * P:(t + 1) * P, :])
                  for half in range(2):
                      mm(mean_ps[half][0:1, :], ones128b,
                         mot[:, half * SEG:(half + 1) * SEG],
                         start=(t == 0), stop=(t == 15))

              for m in range(16):
                  wt = wpool.tile([P, KO, P], bf16, tag="w8")
                  dma(out=wt, in_=qkw_d[m])
                  ps = pa_t()
                  for ko in range(KO):
                      mm(ps, wt[:, ko], xnT[:, ko, :],
                         start=(ko == 0), stop=(ko == KO - 1))
                  is_q = m < 8
                  c_t, s_t = (cq, sq_) if is_q else (ck, sk)
                  dst = qT if is_q else kT
                  ko_out = m % 8
                  qraw = rot.tile([P, SEG], bf16, tag="ropeA")
                  nc.scalar.copy(qraw, ps)
                  rps = pa_t()
                  mm(rps, rmat, qraw, start=True, stop=True)
                  At = rot.tile([P, SEG], bf16, tag="ropeB")
                  nc.vector.tensor_mul(At, ps, c_t)
                  Bt = rot.tile([P, SEG], bf16, tag="ropeA")
                  nc.vector.tensor_mul(Bt, rps, s_t)
                  nc.vector.tensor_add(dst[:, ko_out, :], At, Bt)
                  mo_mean_step(m)

              # ---------------- memory-context: pooled rmsnorm (no PE) -----
              # Emitted before v-proj so its serial ACT/DVE latency hides
              # under the v-proj matmuls; PE-touching stages are interleaved
              # into the v-proj loop below so the PE never idles long enough
              # to re-throttle.
              pooled_raw = mrow[:, 0:DIM]
              for half in range(2):
                  nc.scalar.activation(_f32r(pooled_raw[:, half * SEG:(half + 1) * SEG]),
                                       mean_ps[half][0:1, :], AF.Copy,
                                       scale=1.0 / N)
              sqr = mrow[:, DIM:2 * DIM]
              nc.vector.tensor_mul(_f32r(sqr), pooled_raw, pooled_raw)
              nc.vector.reduce_sum(_f32r(sqr[:, 0:1]), sqr, axis=AX.X)
              nc.scalar.activation(_f32r(sqr[:, 1:2]), sqr[:, 0:1], AF.Sqrt,
                                   bias=epsc[0:1], scale=1.0 / DIM)
              nc.vector.reciprocal(_f32r(sqr[:, 2:3]), sqr[:, 1:2])
              pooled = mrow[:, 2 * DIM:3 * DIM]
              nc.vector.scalar_tensor_tensor(out=_f32r(pooled), in0=pooled_raw,
                                             scalar=sqr[:, 2:3], in1=mpnw,
                                             op0=OP.mult, op1=OP.mult)

              def stage_pT():
                  pT = pa_t()
                  for ko in range(KO):
                      mm(pT[:, ko:ko + 1], pooled[0:1, ko * P:(ko + 1) * P],
                         ones11, start=True, stop=True, skip_group_check=True)
                  nc.vector.tensor_copy(out=pooledT, in_=pT[:, 0:KO])

              mt_ps = []

              def stage_mt():
                  mt_ps.extend([psc_t(), psc_t()])
                  for ko in range(KO):
                      mtw_t = kvpool.tile([P, DIM], bf16, tag="kv")
                      dma(out=mtw_t, in_=mtw_d[ko])
                      for half in range(2):
                          mm(mt_ps[half][0:1, :], pooledT[:, ko:ko + 1],
                             mtw_t[:, half * SEG:(half + 1) * SEG],
                             start=(ko == 0), stop=(ko == KO - 1))

              memtok = mrow[:, 0:DIM]

              def stage_mT():
                  for half in range(2):
                      nc.scalar.activation(_f32r(memtok[:, half * SEG:(half + 1) * SEG]),
                                           mt_ps[half][0:1, :], AF.Copy)
                  mT = pa_t()
                  for ko in range(KO):
                      mm(mT[:, ko:ko + 1], memtok[0:1, ko * P:(ko + 1) * P],
                         ones11, start=True, stop=True, skip_group_check=True)
                  nc.vector.tensor_copy(out=memtokT, in_=mT[:, 0:KO])

              kc_row = mrow[:, DIM:2 * DIM]
              vc_row = mrow[:, 2 * DIM:3 * DIM]

              def stage_kcvc(c):
                  r_ps = [psc_t(), psc_t()]
                  for ko in range(KO):
                      kv_t = kvpool.tile([P, DIM], bf16, tag="kv")
                      dma(out=kv_t, in_=kvw_d[c, ko])
                      for half in range(2):
                          mm(r_ps[half][0:1, :], memtokT[:, ko:ko + 1],
                             kv_t[:, half * SEG:(half + 1) * SEG],
                             start=(ko == 0), stop=(ko == KO - 1))
                  row = kc_row if c == 0 else vc_row
                  for half in range(2):
                      nc.scalar.activation(_f32r(row[:, half * SEG:(half + 1) * SEG]),
                                           r_ps[half][0:1, :], AF.Copy)

              def stage_kx():
                  for j in range(KO):  # 2 heads per chunk
                      kx = pa_t()
                      mm(kx[:, 0:16], _f32r(kc_row[0:1, j * P:(j + 1) * P]),
                         _f32r(ones16), start=True, stop=True,
                         skip_group_check=True)
                      nc.vector.tensor_copy(out=kP[0:DH, 2 * j, 0:NPM],
                                            in_=kx[0:DH, 0:16])
                      nc.vector.tensor_copy(out=kP[DH:P, 2 * j + 1, 0:NPM],
                                            in_=kx[DH:P, 0:16])

              def stage_vx():
                  for half in range(2):
                      vx = pa_t()
                      mm(vx[0:16, :], _f32r(ones16),
                         _f32r(vc_row[0:1, half * SEG:(half + 1) * SEG]),
                         start=True, stop=True, skip_group_check=True)
                      nc.vector.tensor_copy(
                          out=vP[0:NPM, half * 8:(half + 1) * 8, 0:DH],
                          in_=vx[0:16, :].rearrange("p (h d) -> p h d", d=DH))

              stages = [stage_pT, stage_mt, stage_mT,
                        lambda: stage_kcvc(0), lambda: stage_kcvc(1),
                        stage_kx, stage_vx]

              # ---------------- v projection (token-major), interleaved ----
              for half in range(2):
                  kvv = persist.tile([P, KO, SEG], bf16, tag="big16")
                  for ko in range(KO):
                      dma(out=kvv[:, ko, :],
                          in_=kvw_d[1, ko, :, half * SEG:(half + 1) * SEG])
                  for tc_ in range(4):
                      ps = pa_t()
                      for ko in range(KO):
                          mm(ps, xnT[:, ko, tc_ * P:(tc_ + 1) * P],
                             kvv[:, ko, :],
                             start=(ko == 0), stop=(ko == KO - 1))
                      nc.vector.tensor_copy(
                          out=vA[:, tc_, half * 8:(half + 1) * 8, 0:DH],
                          in_=ps.rearrange("p (h d) -> p h d", d=DH))
                      if (half, tc_) != (0, 0) and stages:
                          stages.pop(0)()
              while stages:
                  stages.pop(0)()

              # ---------------- attention heads ----------------
              # Scores chunk c covers keys [cP,(c+1)P) x queries [cP,SEG)
              # (block-triangular). Chunks 1 and 3 share one PSUM tile /
              # exp pass: c1 at free 0:384 (queries 128:512), c3 at free
              # 384:512 (queries 384:512). The P@V stationary has a ones
              # column, so PSUM row DH is the softmax denominator.
              for h in range(HEADS):
                  ko_h, hf = h // 2, h % 2
                  qr = DH * hf
                  q_h = qT[qr:qr + DH, ko_h, :]
                  k_h = kT[qr:qr + DH, ko_h, :]
                  # prefix scores [32, 512]
                  scp = psc_t()
                  mm(scp[0:PFX, :], kP[qr:qr + DH, h, :], q_h,
                     start=True, stop=True, skip_group_check=True)
                  eP = epool.tile([PFX, SEG], bf16, tag="eP")
                  nc.scalar.activation(eP, scp[0:PFX, :], AF.Exp,
                                       bias=zeroc[0:PFX])
                  sc0 = psc_t()
                  mm(sc0[:, :], k_h[:, 0:P], q_h,
                     start=True, stop=True, skip_group_check=True)
                  e0 = epool.tile([P, SEG], bf16, tag="e0")
                  nc.scalar.activation(e0, sc0, AF.Exp, bias=zeroc)
                  nc.vector.tensor_mul(e0[:, 0:P], e0[:, 0:P], maskD)
                  sc13 = psc_t()
                  mm(sc13[:, 0:384], k_h[:, P:2 * P], q_h[:, P:],
                     start=True, stop=True, skip_group_check=True)
                  mm(sc13[:, 384:512], k_h[:, 3 * P:4 * P], q_h[:, 3 * P:],
                     start=True, stop=True, skip_group_check=True)
                  eB = epool.tile([P, SEG], bf16, tag="eB")
                  nc.scalar.activation(eB, sc13, AF.Exp, bias=zeroc)
                  nc.vector.tensor_mul(eB[:, 0:P], eB[:, 0:P], maskD)
                  nc.vector.tensor_mul(eB[:, 384:512], eB[:, 384:512], maskD)
                  sc2 = psc_t()
                  mm(sc2[:, 0:256], k_h[:, 2 * P:3 * P], q_h[:, 2 * P:],
                     start=True, stop=True, skip_group_check=True)
                  e2 = epool.tile([P, 256], bf16, tag="e2")
                  nc.scalar.activation(e2, sc2[:, 0:256], AF.Exp, bias=zeroc)
                  nc.vector.tensor_mul(e2[:, 0:P], e2[:, 0:P], maskD)
                  # P@V with ones-column: row DH = softmax denominator
                  po = pa_t()  # 4-buf pool so heads pipeline
                  mm(po[0:DH + 1, :], vP[:, h, :], eP,
                     start=True, stop=False, skip_group_check=True)
                  mm(po[0:DH + 1, 0:], vA[:, 0, h, :], e0,
                     start=False, stop=False, skip_group_check=True)
                  mm(po[0:DH + 1, P:], vA[:, 1, h, :], eB[:, 0:384],
                     start=False, stop=False, skip_group_check=True)
                  mm(po[0:DH + 1, 2 * P:], vA[:, 2, h, :], e2,
                     start=False, stop=False, skip_group_check=True)
                  mm(po[0:DH + 1, 3 * P:], vA[:, 3, h, :], eB[:, 384:512],
                     start=False, stop=True, skip_group_check=True)
                  rvb = rot.tile([1, SEG], bf16, tag="rvb")
                  nc.vector.reciprocal(rvb, po[DH:DH + 1, :])
                  bcp = pso_t()  # broadcast 1/denom over DH partitions
                  mm(bcp[0:DH, :], ones1xPb[0:1, 0:DH], rvb,
                     start=True, stop=True, skip_group_check=True)
                  bcs = epool.tile([DH, SEG], bf16, tag="bcs")
                  nc.vector.tensor_copy(out=bcs, in_=bcp[0:DH, :])
                  nc.vector.tensor_mul(oA[qr:qr + DH, ko_h, :],
                                       po[0:DH, :], bcs)

              # ---------------- output projection + residual ----------------
              for m in range(KO):
                  wt = wpool.tile([P, KO, P], bf16, tag="w8")
                  dma(out=wt, in_=outw_d[m])
                  ps = pa_t()
                  for k in range(KO):
                      mm(ps, wt[:, k], oA[:, k, :],
                         start=(k == 0), stop=(k == KO - 1))
                  nc.vector.tensor_add(xT[:, m, :], ps, xT[:, m, :])  # x1

              # ---------------- FFN ----------------
              rmsnorm_into(xnT, xT, fnw, "big16")  # xn1^T
              u_parts = [qT, kT]  # reuse dead slots as u storage
              u_c = persist.tile([P, 6, SEG], bf16, tag="big16")

              def u_slice(k):
                  if k < 8:
                      return u_parts[0][:, k, :]
                  if k < 16:
                      return u_parts[1][:, k - 8, :]
                  return u_c[:, k - 16, :]

              for m in range(MFF):
                  wa = wpool.tile([P, KO, P], bf16, tag="w8")
                  dma(out=wa, in_=w1a_d[m])
                  wg = wpool.tile([P, KO, P], bf16, tag="w8")
                  dma(out=wg, in_=w1g_d[m])
                  psa = pa_t()
                  psg = pa_t()
                  for ko in range(KO):
                      mm(psa, wa[:, ko], xnT[:, ko, :],
                         start=(ko == 0), stop=(ko == KO - 1))
                      mm(psg, wg[:, ko], xnT[:, ko, :],
                         start=(ko == 0), stop=(ko == KO - 1))
                  sig = rot.tile([P, SEG], f32, tag="ropeA")
                  nc.scalar.activation(sig, psg, AF.Sigmoid,
                                       bias=b1g[:, m:m + 1], scale=1.0)
                  silu = rot.tile([P, SEG], f32, tag="ropeB")
                  nc.vector.scalar_tensor_tensor(
                      out=silu, in0=psg, scalar=b1g[:, m:m + 1],
                      in1=sig, op0=OP.add, op1=OP.mult)
                  nc.vector.scalar_tensor_tensor(
                      out=u_slice(m), in0=psa, scalar=b1a[:, m:m + 1],
                      in1=silu, op0=OP.add, op1=OP.mult)

              for o in range(KO):
                  ps = pa_t()
                  for half in range(2):
                      w2t = w2pool.tile([P, 11, P], bf16, tag="w2")
                      dma(out=w2t, in_=w2_d[o][:, half * 11:(half + 1) * 11, :])
                      for k2 in range(11):
                          k = half * 11 + k2
                          mm(ps, w2t[:, k2], u_slice(k),
                             start=(k == 0), stop=(k == MFF - 1))
                  outT = persist.tile([P, KO, SEG], f32, tag="outT",
                                      name=f"outT{o}")
                  nc.vector.scalar_tensor_tensor(
                      out=outT[:, o, :], in0=ps, scalar=b2[:, o:o + 1],
                      in1=xT[:, o, :], op0=OP.add, op1=OP.add)
                  dma(out=yT_d[o * P:(o + 1) * P, :], in_=outT[:, o, :])

    for _rep in range(reps):
        _emit(nc)
    nc.compile()
    return nc


# ======================= host-side preparation =======================

def _prep_shared(inputs):
    import ml_dtypes
    f32 = np.float32
    bf16 = ml_dtypes.bfloat16
    qkv = np.asarray(inputs["to_qkv_w"], f32)
    shared = {}
    shared["qkw"] = np.ascontiguousarray(
        qkv[:, :2048].reshape(KO, P, 16, P).transpose(2, 1, 0, 3)).astype(bf16)
    shared["kvw"] = np.ascontiguousarray(
        np.stack([qkv[:, 1024:2048], qkv[:, 2048:3072]])
        .reshape(2, KO, P, DIM)).astype(bf16)
    shared["outw"] = np.ascontiguousarray(
        np.asarray(inputs["to_out_w"], f32)
        .reshape(KO, P, KO, P).transpose(2, 1, 0, 3)).astype(bf16)
    w1 = np.asarray(inputs["ff_w1"], f32)
    w1a = np.zeros((DIM, DFFP), f32)
    w1g = np.zeros((DIM, DFFP), f32)
    w1a[:, :DFF] = w1[:, :DFF]
    w1g[:, :DFF] = w1[:, DFF:]
    shared["w1a"] = np.ascontiguousarray(
        w1a.reshape(KO, P, MFF, P).transpose(2, 1, 0, 3)).astype(bf16)
    shared["w1g"] = np.ascontiguousarray(
        w1g.reshape(KO, P, MFF, P).transpose(2, 1, 0, 3)).astype(bf16)
    w2 = np.zeros((DFFP, DIM), f32)
    w2[:DFF] = np.asarray(inputs["ff_w2"], f32)
    shared["w2"] = np.ascontiguousarray(
        w2.reshape(MFF, P, KO, P).transpose(2, 1, 0, 3)).astype(bf16)
    shared["mtw"] = np.ascontiguousarray(
        np.asarray(inputs["to_mem_tokens_w"], f32).reshape(KO, P, DIM)).astype(bf16)
    pm = np.asarray(inputs["persist_mem"], f32)
    shared["pmv"] = np.ascontiguousarray(pm).astype(bf16)
    shared["pmk"] = np.ascontiguousarray(pm.transpose(0, 2, 1)).astype(bf16)
    b1 = np.asarray(inputs["ff_b1"], f32)
    b1a = np.zeros(DFFP, f32)
    b1g = np.zeros(DFFP, f32)
    b1a[:DFF] = b1[:DFF]
    b1g[:DFF] = b1[DFF:]
    shared["b1a"] = np.ascontiguousarray(b1a.reshape(MFF, P).T)
    shared["b1g"] = np.ascontiguousarray(b1g.reshape(MFF, P).T)
    shared["b2"] = np.ascontiguousarray(
        np.asarray(inputs["ff_b2"], f32).reshape(KO, P).T)
    shared["anw"] = np.ascontiguousarray(
        np.asarray(inputs["attn_norm_w"], f32).reshape(KO, P).T)
    shared["fnw"] = np.ascontiguousarray(
        np.asarray(inputs["ff_norm_w"], f32).reshape(KO, P).T)
    shared["mpnw"] = np.ascontiguousarray(
        np.asarray(inputs["mem_pool_norm_w"], f32).reshape(1, DIM))
    rl = np.zeros((P, P), f32)
    ii = np.arange(0, P, 2)
    rl[ii + 1, ii] = f32(-1.0)
    rl[ii, ii + 1] = f32(1.0)
    shared["rmat"] = rl.astype(bf16)
    shared["maskD"] = np.where(
        np.arange(P)[None, :] >= np.arange(P)[:, None], f32(1.0), f32(0.0)
    ).astype(bf16)

    # per-batch bf16 mem_out
    mo = np.asarray(inputs["mem_out"], f32)
    shared["_mo"] = [np.ascontiguousarray(mo[b]).astype(bf16) for b in range(B)]

    # rope tables, float32 math to match the reference
    pos = np.arange(N, dtype=f32)
    expo = (np.arange(0, DH, 2).astype(f32) / f32(DH)).astype(f32)
    inv = (f32(1.0) / np.power(f32(10000.0), expo)).astype(f32)
    ang = np.repeat(pos[:, None] * inv[None, :], 2, axis=1).astype(f32)
    cosf, sinf = np.cos(ang).astype(f32), np.sin(ang).astype(f32)
    scale = f32(DH ** -0.5)
    shared["_cos"], shared["_sin"], shared["_scale"] = cosf, sinf, scale
    return shared


def _prep_core(inputs, shared, b, s):
    f32 = np.float32
    x = np.asarray(inputs["x"], f32)
    cosf, sinf, scale = shared["_cos"], shared["_sin"], shared["_scale"]
    seg = slice(s * SEG, (s + 1) * SEG)
    ct = np.ascontiguousarray(np.tile(cosf[seg].T, (2, 1)))
    st = np.ascontiguousarray(np.tile(sinf[seg].T, (2, 1)))
    m = {k: v for k, v in shared.items() if not k.startswith("_")}
    m["xT"] = np.ascontiguousarray(x[b, seg].T)
    m["mo"] = shared["_mo"][b]
    m["cq"] = (ct * scale).astype(f32)
    m["sq"] = (st * scale).astype(f32)
    m["ck"] = ct
    m["sk"] = st
    return m


def _get_nc():
    if "nc" not in _CACHE:
        _CACHE["nc"] = build_nc()
    return _CACHE["nc"]


def kernel(**inputs) -> np.ndarray:
    nc = _get_nc()
    shared = _prep_shared(inputs)
    cores = [(b, s) for b in range(B) for s in range(4)]
    in_maps = [_prep_core(inputs, shared, b, s) for b, s in cores]
    from concourse import bass_utils
    import os
    res = bass_utils.run_bass_kernel_spmd(
        nc, in_maps, core_ids=list(range(NCORES)),
        trace=bool(os.environ.get("MAC_TRACE")))
    _CACHE["last_results"] = res
    out = np.empty((B, N, DIM), np.float32)
    for i, (b, s) in enumerate(cores):
        out[b, s * SEG:(s + 1) * SEG, :] = res.results[i]["yT"].T
    return out


# revision 14
# speedup vs baseline: 1.4987x; 1.1442x over previous
"""Trainium2 Bass kernel for nn_MACBlock (segmented attention + GEGLU FFN).

Sharding: 8 cores = 2 batches x 4 segments of 512 queries. The segment mask
makes attention block-diagonal (plus a 32-token always-visible prefix derived
from pooled memory + persistent memory), so each core is fully independent:
no collectives.

Layout: activations are kept feature-major (x^T [dim, tokens]) on-chip, so
every matmul contraction dim lands on partitions with zero transposes.
All heavy GEMMs run in bf16 (weights pre-cast on host, activations cast
on-chip): bf16 enables Fast Weight Load and avoids the fp32-HIGH power
throttle that halves the PE clock. PSUM accumulation stays fp32.
Scores are computed key-major ([keys, queries]); softmax is max-free; the
softmax denominator comes from a ones-column folded into the P@V stationary
operand (row DH of the same PSUM tile).
"""

import sys

if "/opt/trn_rl_repo" not in sys.path:
    sys.path.insert(0, "/opt/trn_rl_repo")

import numpy as np

B, N, DIM = 2, 2048, 1024
HEADS, DH = 16, 64
SEG = 512
NPM = NM = 16
PFX = NPM + NM          # 32 prefix keys
DFF = 2730
MFF = 22                # padded dff chunks
DFFP = MFF * 128        # 2816
KO = 8                  # 1024 / 128
P = 128
NCORES = 8
EPS = 1.1920929e-07

_CACHE = {}


def _f32r(ap):
    import concourse.mybir as mybir
    return ap.bitcast(mybir.dt.float32r)


def build_nc(reps=1):
    import concourse.bass as bass
    from concourse import bacc
    import concourse.tile as tile
    import concourse.mybir as mybir

    f32 = mybir.dt.float32
    bf16 = mybir.dt.bfloat16
    AF = mybir.ActivationFunctionType
    OP = mybir.AluOpType
    AX = mybir.AxisListType

    nc = bacc.Bacc("TRN2", target_bir_lowering=False, debug=False)

    dp = nc.declare_dram_parameter
    xT_d = dp("xT", [DIM, SEG], f32, isOutput=False)
    mo_d = dp("mo", [N, DIM], bf16, isOutput=False)
    cq_d = dp("cq", [P, SEG], f32, isOutput=False)
    sq_d = dp("sq", [P, SEG], f32, isOutput=False)
    ck_d = dp("ck", [P, SEG], f32, isOutput=False)
    sk_d = dp("sk", [P, SEG], f32, isOutput=False)
    mask_d = dp("maskD", [P, P], bf16, isOutput=False)
    rmat_d = dp("rmat", [P, P], bf16, isOutput=False)
    qkw_d = dp("qkw", [16, P, KO, P], bf16, isOutput=False)
    kvw_d = dp("kvw", [2, KO, P, DIM], bf16, isOutput=False)
    outw_d = dp("outw", [KO, P, KO, P], bf16, isOutput=False)
    w1a_d = dp("w1a", [MFF, P, KO, P], bf16, isOutput=False)
    w1g_d = dp("w1g", [MFF, P, KO, P], bf16, isOutput=False)
    w2_d = dp("w2", [KO, P, MFF, P], bf16, isOutput=False)
    mtw_d = dp("mtw", [KO, P, DIM], bf16, isOutput=False)
    pmv_d = dp("pmv", [HEADS, NPM, DH], bf16, isOutput=False)
    pmk_d = dp("pmk", [HEADS, DH, NPM], bf16, isOutput=False)
    b1a_d = dp("b1a", [P, MFF], f32, isOutput=False)
    b1g_d = dp("b1g", [P, MFF], f32, isOutput=False)
    b2_d = dp("b2", [P, KO], f32, isOutput=False)
    anw_d = dp("anw", [P, KO], f32, isOutput=False)
    fnw_d = dp("fnw", [P, KO], f32, isOutput=False)
    mpnw_d = dp("mpnw", [1, DIM], f32, isOutput=False)
    yT_d = dp("yT", [DIM, SEG], f32, isOutput=True)

    def _emit(nc):
      with tile.TileContext(nc) as tc, \
            nc.allow_low_precision(reason="bf16 matmul rounding"):
        from contextlib import ExitStack
        ctx = ExitStack()
        with ctx:
            persist = ctx.enter_context(tc.tile_pool(name="persist", bufs=1))
            wpool = ctx.enter_context(tc.tile_pool(name="wpool", bufs=5))
            kvpool = ctx.enter_context(tc.tile_pool(name="kvpool", bufs=2))
            w2pool = ctx.enter_context(tc.tile_pool(name="w2pool", bufs=3))
            mopool = ctx.enter_context(tc.tile_pool(name="mopool", bufs=3))
            rot = ctx.enter_context(tc.tile_pool(name="rot", bufs=2))
            epool = ctx.enter_context(tc.tile_pool(name="epool", bufs=3))
            pa = ctx.enter_context(tc.tile_pool(name="pa", bufs=4, space="PSUM"))
            psc = ctx.enter_context(tc.tile_pool(name="psc", bufs=2, space="PSUM"))
            pso = ctx.enter_context(tc.tile_pool(name="pso", bufs=2, space="PSUM"))

            cnt = [0]

            def pa_t():
                cnt[0] += 1
                return pa.tile([P, SEG], f32, tag="ps", name=f"pa{cnt[0]}")

            def psc_t():
                cnt[0] += 1
                return psc.tile([P, SEG], f32, tag="sc", name=f"sc{cnt[0]}")

            def pso_t():
                cnt[0] += 1
                return pso.tile([P, SEG], f32, tag="o", name=f"o{cnt[0]}")

            # ---------------- persistent SBUF tensors ----------------
            xT = persist.tile([P, KO, SEG], f32, tag="xT")       # x^T, later x1^T
            xnT = persist.tile([P, KO, SEG], bf16, tag="xnT")    # xn^T, later xn1^T
            kT = persist.tile([P, KO, SEG], bf16, tag="kT")      # roped k^T
            vA = persist.tile([P, 4, HEADS, DH + 1], bf16, tag="vA")  # v key-major
            vP = persist.tile([PFX, HEADS, DH + 1], bf16, tag="vP")   # prefix v rows
            kP = persist.tile([P, HEADS, PFX], bf16, tag="kP")   # prefix k^T @64*(h%2)
            oA = persist.tile([P, KO, SEG], bf16, tag="oA")      # attn o^T
            cq = persist.tile([P, SEG], f32, tag="cq")
            sq_ = persist.tile([P, SEG], f32, tag="sq")
            ck = persist.tile([P, SEG], f32, tag="ck")
            sk = persist.tile([P, SEG], f32, tag="sk")
            maskD = persist.tile([P, P], bf16, tag="maskD")
            rmat = persist.tile([P, P], bf16, tag="rmat")
            b1a = persist.tile([P, MFF], f32, tag="b1a")
            b1g = persist.tile([P, MFF], f32, tag="b1g")
            b2 = persist.tile([P, KO], f32, tag="b2")
            anw = persist.tile([P, KO], f32, tag="anw")
            fnw = persist.tile([P, KO], f32, tag="fnw")
            mpnw = persist.tile([1, DIM], f32, tag="mpnw")
            ones16 = persist.tile([1, 16], f32, tag="o16")
            ones11 = persist.tile([1, 1], f32, tag="o11")
            ones128b = persist.tile([P, 1], bf16, tag="o128b")   # lhsT K=128,M=1
            ones1xPb = persist.tile([1, P], bf16, tag="o1xPb")   # lhsT K=1,M=128
            ones1xP = persist.tile([1, P], f32, tag="o1xP")      # f32 variant
            pooledT = persist.tile([P, KO], bf16, tag="pooledT")
            memtokT = persist.tile([P, KO], bf16, tag="memtokT")
            mrow = persist.tile([1, 3 * DIM], f32, tag="mrow")
            rrow = persist.tile([1, 2 * SEG], bf16, tag="rrow")
            epsc = persist.tile([P, 1], f32, tag="epsc")
            zeroc = persist.tile([P, 1], f32, tag="zeroc")

            dma = nc.sync.dma_start
            dma(out=cq, in_=cq_d[:])
            dma(out=sq_, in_=sq_d[:])
            dma(out=ck, in_=ck_d[:])
            dma(out=sk, in_=sk_d[:])
            dma(out=maskD, in_=mask_d[:])
            dma(out=rmat, in_=rmat_d[:])
            dma(out=b1a, in_=b1a_d[:])
            dma(out=b1g, in_=b1g_d[:])
            dma(out=b2, in_=b2_d[:])
            dma(out=anw, in_=anw_d[:])
            dma(out=fnw, in_=fnw_d[:])
            dma(out=mpnw, in_=mpnw_d[:])
            nc.vector.memset(ones16, 1.0)
            nc.vector.memset(ones11, 1.0)
            nc.vector.memset(ones128b, 1.0)
            nc.vector.memset(ones1xPb, 1.0)
            nc.vector.memset(ones1xP, 1.0)
            nc.vector.memset(epsc, EPS)
            nc.vector.memset(zeroc, 0.0)
            # denominator ones-columns in the P@V stationary operands
            nc.vector.memset(vA[:, :, :, DH:DH + 1], 1.0)
            nc.vector.memset(vP[:, :, DH:DH + 1], 1.0)
            for h in range(HEADS):
                hb = DH * (h % 2)
                dma(out=kP[hb:hb + DH, h, NPM:PFX], in_=pmk_d[h])
                dma(out=vP[NPM:PFX, h, 0:DH], in_=pmv_d[h])

            if True:
              dma(out=xT, in_=xT_d.rearrange("(ko p) n -> p ko n", p=P))

              mm = nc.tensor.matmul

              def rmsnorm_into(dst, src, w_sb, sq_tag):
                  """dst[:,ko,:] = src[:,ko,:] * w[:,ko] * rsqrt(mean_dim(src^2)+eps)"""
                  ss = psc_t()  # [1,512] slice used
                  sq8 = persist.tile([P, KO, SEG], bf16, tag=sq_tag, name="sq8")
                  for ko in range(KO):
                      nc.vector.tensor_mul(sq8[:, ko, :], src[:, ko, :],
                                           src[:, ko, :])
                      mm(ss[0:1, :], ones128b, sq8[:, ko, :],
                         start=(ko == 0), stop=(ko == KO - 1))
                  nc.scalar.activation(rrow[:, 0:SEG], ss[0:1, :], AF.Sqrt,
                                       bias=epsc[0:1], scale=1.0 / DIM)
                  nc.vector.reciprocal(rrow[:, SEG:2 * SEG], rrow[:, 0:SEG])
                  bc = pso_t()  # broadcast rstd over 128 partitions
                  mm(bc, ones1xPb, rrow[:, SEG:2 * SEG],
                     start=True, stop=True)
                  for ko in range(KO):
                      nc.vector.scalar_tensor_tensor(
                          out=dst[:, ko, :], in0=src[:, ko, :],
                          scalar=w_sb[:, ko:ko + 1], in1=bc,
                          op0=OP.mult, op1=OP.mult)

              # ---------------- attn rmsnorm ----------------
              rmsnorm_into(xnT, xT, anw, "big16")
              qT = persist.tile([P, KO, SEG], bf16, tag="qT")    # roped,scaled q^T

              # ---------------- q/k projections + rope, interleaved with
              # ---------------- mem_out mean accumulation ----------------
              mean_ps = [psc_t(), psc_t()]   # two [1,512] accumulators (slices)

              def mo_mean_step(t):
                  mot = mopool.tile([P, DIM], bf16, tag="mo", name="mot")
                  dma(out=mot, in_=mo_d[t * P:(t + 1) * P, :])
                  for half in range(2):
                      mm(mean_ps[half][0:1, :], ones128b,
                         mot[:, half * SEG:(half + 1) * SEG],
                         start=(t == 0), stop=(t == 15))

              # pooled rmsnorm (pure ACT/DVE): emitted mid-qk-loop so its
              # serial latency hides under the remaining projection matmuls
              pooled_raw = mrow[:, 0:DIM]
              sqr = mrow[:, DIM:2 * DIM]
              pooled = mrow[:, 2 * DIM:3 * DIM]

              def pooled_chain():
                  for half in range(2):
                      nc.scalar.activation(_f32r(pooled_raw[:, half * SEG:(half + 1) * SEG]),
                                           mean_ps[half][0:1, :], AF.Copy,
                                           scale=1.0 / N)
                  nc.vector.tensor_mul(_f32r(sqr), pooled_raw, pooled_raw)
                  nc.vector.reduce_sum(_f32r(sqr[:, 0:1]), sqr, axis=AX.X)
                  nc.scalar.activation(_f32r(sqr[:, 1:2]), sqr[:, 0:1], AF.Sqrt,
                                       bias=epsc[0:1], scale=1.0 / DIM)
                  nc.vector.reciprocal(_f32r(sqr[:, 2:3]), sqr[:, 1:2])
                  nc.vector.scalar_tensor_tensor(out=_f32r(pooled), in0=pooled_raw,
                                                 scalar=sqr[:, 2:3], in1=mpnw,
                                                 op0=OP.mult, op1=OP.mult)

              for m in range(16):
                  wt = wpool.tile([P, KO, P], bf16, tag="w8")
                  dma(out=wt, in_=qkw_d[m])
                  ps = pa_t()
                  for ko in range(KO):
                      mm(ps, wt[:, ko], xnT[:, ko, :],
                         start=(ko == 0), stop=(ko == KO - 1))
                  is_q = m < 8
                  c_t, s_t = (cq, sq_) if is_q else (ck, sk)
                  dst = qT if is_q else kT
                  ko_out = m % 8
                  qraw = rot.tile([P, SEG], bf16, tag="ropeA")
                  nc.scalar.copy(qraw, ps)
                  rps = pa_t()
                  mm(rps, rmat, qraw, start=True, stop=True)
                  At = rot.tile([P, SEG], bf16, tag="ropeB")
                  nc.vector.tensor_mul(At, ps, c_t)
                  Bt = rot.tile([P, SEG], bf16, tag="ropeA")
                  nc.vector.tensor_mul(Bt, rps, s_t)
                  nc.vector.tensor_add(dst[:, ko_out, :], At, Bt)
                  if 4 <= m < 12:
                      mo_mean_step(2 * (m - 4))
                      mo_mean_step(2 * (m - 4) + 1)
                  if m == 12:
                      pooled_chain()

              def stage_pT():
                  pT = pa_t()
                  for ko in range(KO):
                      mm(pT[:, ko:ko + 1], pooled[0:1, ko * P:(ko + 1) * P],
                         ones11, start=True, stop=True, skip_group_check=True)
                  nc.vector.tensor_copy(out=pooledT, in_=pT[:, 0:KO])

              mt_ps = []

              def stage_mt():
                  mt_ps.extend([psc_t(), psc_t()])
                  for ko in range(KO):
                      mtw_t = kvpool.tile([P, DIM], bf16, tag="kv")
                      dma(out=mtw_t, in_=mtw_d[ko])
                      for half in range(2):
                          mm(mt_ps[half][0:1, :], pooledT[:, ko:ko + 1],
                             mtw_t[:, half * SEG:(half + 1) * SEG],
                             start=(ko == 0), stop=(ko == KO - 1))

              memtok = mrow[:, 0:DIM]

              def stage_mT():
                  for half in range(2):
                      nc.scalar.activation(_f32r(memtok[:, half * SEG:(half + 1) * SEG]),
                                           mt_ps[half][0:1, :], AF.Copy)
                  mT = pa_t()
                  for ko in range(KO):
                      mm(mT[:, ko:ko + 1], memtok[0:1, ko * P:(ko + 1) * P],
                         ones11, start=True, stop=True, skip_group_check=True)
                  nc.vector.tensor_copy(out=memtokT, in_=mT[:, 0:KO])

              kc_row = mrow[:, DIM:2 * DIM]
              vc_row = mrow[:, 2 * DIM:3 * DIM]

              def stage_kcvc(c):
                  r_ps = [psc_t(), psc_t()]
                  for ko in range(KO):
                      kv_t = kvpool.tile([P, DIM], bf16, tag="kv")
                      dma(out=kv_t, in_=kvw_d[c, ko])
                      for half in range(2):
                          mm(r_ps[half][0:1, :], memtokT[:, ko:ko + 1],
                             kv_t[:, half * SEG:(half + 1) * SEG],
                             start=(ko == 0), stop=(ko == KO - 1))
                  row = kc_row if c == 0 else vc_row
                  for half in range(2):
                      nc.scalar.activation(_f32r(row[:, half * SEG:(half + 1) * SEG]),
                                           r_ps[half][0:1, :], AF.Copy)

              def stage_kx():
                  for j in range(KO):  # 2 heads per chunk
                      kx = pa_t()
                      mm(kx[:, 0:16], _f32r(kc_row[0:1, j * P:(j + 1) * P]),
                         _f32r(ones16), start=True, stop=True,
                         skip_group_check=True)
                      nc.vector.tensor_copy(out=kP[0:DH, 2 * j, 0:NPM],
                                            in_=kx[0:DH, 0:16])
                      nc.vector.tensor_copy(out=kP[DH:P, 2 * j + 1, 0:NPM],
                                            in_=kx[DH:P, 0:16])

              def stage_vx():
                  for half in range(2):
                      vx = pa_t()
                      mm(vx[0:16, :], _f32r(ones16),
                         _f32r(vc_row[0:1, half * SEG:(half + 1) * SEG]),
                         start=True, stop=True, skip_group_check=True)
                      nc.vector.tensor_copy(
                          out=vP[0:NPM, half * 8:(half + 1) * 8, 0:DH],
                          in_=vx[0:16, :].rearrange("p (h d) -> p h d", d=DH))

              stage_pT()  # pooled ready ~3 qk-iterations ago
              stages = [stage_mt, stage_mT,
                        lambda: stage_kcvc(0), lambda: stage_kcvc(1),
                        stage_kx, stage_vx]

              # ---------------- v projection (token-major), interleaved ----
              for half in range(2):
                  kvv = persist.tile([P, KO, SEG], bf16, tag="big16")
                  for ko in range(KO):
                      dma(out=kvv[:, ko, :],
                          in_=kvw_d[1, ko, :, half * SEG:(half + 1) * SEG])
                  for tc_ in range(4):
                      ps = pa_t()
                      for ko in range(KO):
                          mm(ps, xnT[:, ko, tc_ * P:(tc_ + 1) * P],
                             kvv[:, ko, :],
                             start=(ko == 0), stop=(ko == KO - 1))
                      nc.vector.tensor_copy(
                          out=vA[:, tc_, half * 8:(half + 1) * 8, 0:DH],
                          in_=ps.rearrange("p (h d) -> p h d", d=DH))
                      if (half, tc_) != (0, 0) and stages:
                          stages.pop(0)()
              while stages:
                  stages.pop(0)()

              # ---------------- attention heads ----------------
              # Scores chunk c covers keys [cP,(c+1)P) x queries [cP,SEG)
              # (block-triangular). Chunks 1 and 3 share one PSUM tile /
              # exp pass: c1 at free 0:384 (queries 128:512), c3 at free
              # 384:512 (queries 384:512). The P@V stationary has a ones
              # column, so PSUM row DH is the softmax denominator.
              for h in range(HEADS):
                  ko_h, hf = h // 2, h % 2
                  qr = DH * hf
                  q_h = qT[qr:qr + DH, ko_h, :]
                  k_h = kT[qr:qr + DH, ko_h, :]
                  # prefix scores [32, 512]
                  scp = psc_t()
                  mm(scp[0:PFX, :], kP[qr:qr + DH, h, :], q_h,
                     start=True, stop=True, skip_group_check=True)
                  eP = epool.tile([PFX, SEG], bf16, tag="eP")
                  nc.scalar.activation(eP, scp[0:PFX, :], AF.Exp,
                                       bias=zeroc[0:PFX])
                  sc0 = psc_t()
                  mm(sc0[:, :], k_h[:, 0:P], q_h,
                     start=True, stop=True, skip_group_check=True)
                  e0 = epool.tile([P, SEG], bf16, tag="e0")
                  nc.scalar.activation(e0, sc0, AF.Exp, bias=zeroc)
                  nc.vector.tensor_mul(e0[:, 0:P], e0[:, 0:P], maskD)
                  sc13 = psc_t()
                  mm(sc13[:, 0:384], k_h[:, P:2 * P], q_h[:, P:],
                     start=True, stop=True, skip_group_check=True)
                  mm(sc13[:, 384:512], k_h[:, 3 * P:4 * P], q_h[:, 3 * P:],
                     start=True, stop=True, skip_group_check=True)
                  eB = epool.tile([P, SEG], bf16, tag="eB")
                  nc.scalar.activation(eB, sc13, AF.Exp, bias=zeroc)
                  nc.vector.tensor_mul(eB[:, 0:P], eB[:, 0:P], maskD)
                  nc.vector.tensor_mul(eB[:, 384:512], eB[:, 384:512], maskD)
                  sc2 = psc_t()
                  mm(sc2[:, 0:256], k_h[:, 2 * P:3 * P], q_h[:, 2 * P:],
                     start=True, stop=True, skip_group_check=True)
                  e2 = epool.tile([P, 256], bf16, tag="e2")
                  nc.scalar.activation(e2, sc2[:, 0:256], AF.Exp, bias=zeroc)
                  nc.vector.tensor_mul(e2[:, 0:P], e2[:, 0:P], maskD)
                  # P@V with ones-column: row DH = softmax denominator
                  po = pa_t()  # 4-buf pool so heads pipeline
                  mm(po[0:DH + 1, :], vP[:, h, :], eP,
                     start=True, stop=False, skip_group_check=True)
                  mm(po[0:DH + 1, 0:], vA[:, 0, h, :], e0,
                     start=False, stop=False, skip_group_check=True)
                  mm(po[0:DH + 1, P:], vA[:, 1, h, :], eB[:, 0:384],
                     start=False, stop=False, skip_group_check=True)
                  mm(po[0:DH + 1, 2 * P:], vA[:, 2, h, :], e2,
                     start=False, stop=False, skip_group_check=True)
                  mm(po[0:DH + 1, 3 * P:], vA[:, 3, h, :], eB[:, 384:512],
                     start=False, stop=True, skip_group_check=True)
                  drow = rot.tile([1, SEG], f32, tag="dr")
                  nc.scalar.copy(drow, po[DH:DH + 1, :])
                  rvf = rot.tile([1, SEG], f32, tag="rv")
                  nc.vector.reciprocal_approx_fast(out=rvf, in_=drow)
                  rvb = rot.tile([1, SEG], bf16, tag="rvb")
                  nc.vector.tensor_copy(out=rvb, in_=rvf)
                  bcp = pso_t()  # broadcast 1/denom over DH partitions
                  mm(bcp[0:DH, :], ones1xPb[0:1, 0:DH], rvb,
                     start=True, stop=True, skip_group_check=True)
                  bcs = epool.tile([DH, SEG], bf16, tag="bcs")
                  nc.vector.tensor_copy(out=bcs, in_=bcp[0:DH, :])
                  nc.vector.tensor_mul(oA[qr:qr + DH, ko_h, :],
                                       po[0:DH, :], bcs)

              # ---------------- output projection + residual ----------------
              for m in range(KO):
                  wt = wpool.tile([P, KO, P], bf16, tag="w8")
                  dma(out=wt, in_=outw_d[m])
                  ps = pa_t()
                  for k in range(KO):
                      mm(ps, wt[:, k], oA[:, k, :],
                         start=(k == 0), stop=(k == KO - 1))
                  nc.vector.tensor_add(xT[:, m, :], ps, xT[:, m, :])  # x1

              # ---------------- FFN ----------------
              rmsnorm_into(xnT, xT, fnw, "big16")  # xn1^T
              u_parts = [qT, kT]  # reuse dead slots as u storage
              u_c = persist.tile([P, 6, SEG], bf16, tag="big16")

              def u_slice(k):
                  if k < 8:
                      return u_parts[0][:, k, :]
                  if k < 16:
                      return u_parts[1][:, k - 8, :]
                  return u_c[:, k - 16, :]

              for m in range(MFF):
                  wa = wpool.tile([P, KO, P], bf16, tag="w8")
                  dma(out=wa, in_=w1a_d[m])
                  wg = wpool.tile([P, KO, P], bf16, tag="w8")
                  dma(out=wg, in_=w1g_d[m])
                  psa = pa_t()
                  psg = pa_t()
                  for ko in range(KO):
                      mm(psa, wa[:, ko], xnT[:, ko, :],
                         start=(ko == 0), stop=(ko == KO - 1))
                      mm(psg, wg[:, ko], xnT[:, ko, :],
                         start=(ko == 0), stop=(ko == KO - 1))
                  sig = rot.tile([P, SEG], f32, tag="ropeA")
                  nc.scalar.activation(sig, psg, AF.Sigmoid,
                                       bias=b1g[:, m:m + 1], scale=1.0)
                  silu = rot.tile([P, SEG], f32, tag="ropeB")
                  nc.vector.scalar_tensor_tensor(
                      out=silu, in0=psg, scalar=b1g[:, m:m + 1],
                      in1=sig, op0=OP.add, op1=OP.mult)
                  nc.vector.scalar_tensor_tensor(
                      out=u_slice(m), in0=psa, scalar=b1a[:, m:m + 1],
                      in1=silu, op0=OP.add, op1=OP.mult)

              for o in range(KO):
                  ps = pa_t()
                  for half in range(2):
                      w2t = w2pool.tile([P, 11, P], bf16, tag="w2")
                      dma(out=w2t, in_=w2_d[o][:, half * 11:(half + 1) * 11, :])
                      for k2 in range(11):
                          k = half * 11 + k2
                          mm(ps, w2t[:, k2], u_slice(k),
                             start=(k == 0), stop=(k == MFF - 1))
                  outT = persist.tile([P, KO, SEG], f32, tag="outT",
                                      name=f"outT{o}")
                  nc.vector.scalar_tensor_tensor(
                      out=outT[:, o, :], in0=ps, scalar=b2[:, o:o + 1],
                      in1=xT[:, o, :], op0=OP.add, op1=OP.add)
                  dma(out=yT_d[o * P:(o + 1) * P, :], in_=outT[:, o, :])

    for _rep in range(reps):
        _emit(nc)
    nc.compile()
    return nc


# ======================= host-side preparation =======================

def _prep_shared(inputs):
    import ml_dtypes
    f32 = np.float32
    bf16 = ml_dtypes.bfloat16
    qkv = np.asarray(inputs["to_qkv_w"], f32)
    shared = {}
    shared["qkw"] = np.ascontiguousarray(
        qkv[:, :2048].reshape(KO, P, 16, P).transpose(2, 1, 0, 3)).astype(bf16)
    shared["kvw"] = np.ascontiguousarray(
        np.stack([qkv[:, 1024:2048], qkv[:, 2048:3072]])
        .reshape(2, KO, P, DIM)).astype(bf16)
    shared["outw"] = np.ascontiguousarray(
        np.asarray(inputs["to_out_w"], f32)
        .reshape(KO, P, KO, P).transpose(2, 1, 0, 3)).astype(bf16)
    w1 = np.asarray(inputs["ff_w1"], f32)
    w1a = np.zeros((DIM, DFFP), f32)
    w1g = np.zeros((DIM, DFFP), f32)
    w1a[:, :DFF] = w1[:, :DFF]
    w1g[:, :DFF] = w1[:, DFF:]
    shared["w1a"] = np.ascontiguousarray(
        w1a.reshape(KO, P, MFF, P).transpose(2, 1, 0, 3)).astype(bf16)
    shared["w1g"] = np.ascontiguousarray(
        w1g.reshape(KO, P, MFF, P).transpose(2, 1, 0, 3)).astype(bf16)
    w2 = np.zeros((DFFP, DIM), f32)
    w2[:DFF] = np.asarray(inputs["ff_w2"], f32)
    shared["w2"] = np.ascontiguousarray(
        w2.reshape(MFF, P, KO, P).transpose(2, 1, 0, 3)).astype(bf16)
    shared["mtw"] = np.ascontiguousarray(
        np.asarray(inputs["to_mem_tokens_w"], f32).reshape(KO, P, DIM)).astype(bf16)
    pm = np.asarray(inputs["persist_mem"], f32)
    shared["pmv"] = np.ascontiguousarray(pm).astype(bf16)
    shared["pmk"] = np.ascontiguousarray(pm.transpose(0, 2, 1)).astype(bf16)
    b1 = np.asarray(inputs["ff_b1"], f32)
    b1a = np.zeros(DFFP, f32)
    b1g = np.zeros(DFFP, f32)
    b1a[:DFF] = b1[:DFF]
    b1g[:DFF] = b1[DFF:]
    shared["b1a"] = np.ascontiguousarray(b1a.reshape(MFF, P).T)
    shared["b1g"] = np.ascontiguousarray(b1g.reshape(MFF, P).T)
    shared["b2"] = np.ascontiguousarray(
        np.asarray(inputs["ff_b2"], f32).reshape(KO, P).T)
    shared["anw"] = np.ascontiguousarray(
        np.asarray(inputs["attn_norm_w"], f32).reshape(KO, P).T)
    shared["fnw"] = np.ascontiguousarray(
        np.asarray(inputs["ff_norm_w"], f32).reshape(KO, P).T)
    shared["mpnw"] = np.ascontiguousarray(
        np.asarray(inputs["mem_pool_norm_w"], f32).reshape(1, DIM))
    rl = np.zeros((P, P), f32)
    ii = np.arange(0, P, 2)
    rl[ii + 1, ii] = f32(-1.0)
    rl[ii, ii + 1] = f32(1.0)
    shared["rmat"] = rl.astype(bf16)
    shared["maskD"] = np.where(
        np.arange(P)[None, :] >= np.arange(P)[:, None], f32(1.0), f32(0.0)
    ).astype(bf16)

    # per-batch bf16 mem_out
    mo = np.asarray(inputs["mem_out"], f32)
    shared["_mo"] = [np.ascontiguousarray(mo[b]).astype(bf16) for b in range(B)]

    # rope tables, float32 math to match the reference
    pos = np.arange(N, dtype=f32)
    expo = (np.arange(0, DH, 2).astype(f32) / f32(DH)).astype(f32)
    inv = (f32(1.0) / np.power(f32(10000.0), expo)).astype(f32)
    ang = np.repeat(pos[:, None] * inv[None, :], 2, axis=1).astype(f32)
    cosf, sinf = np.cos(ang).astype(f32), np.sin(ang).astype(f32)
    scale = f32(DH ** -0.5)
    shared["_cos"], shared["_sin"], shared["_scale"] = cosf, sinf, scale
    return shared


def _prep_core(inputs, shared, b, s):
    f32 = np.float32
    x = np.asarray(inputs["x"], f32)
    cosf, sinf, scale = shared["_cos"], shared["_sin"], shared["_scale"]
    seg = slice(s * SEG, (s + 1) * SEG)
    ct = np.ascontiguousarray(np.tile(cosf[seg].T, (2, 1)))
    st = np.ascontiguousarray(np.tile(sinf[seg].T, (2, 1)))
    m = {k: v for k, v in shared.items() if not k.startswith("_")}
    m["xT"] = np.ascontiguousarray(x[b, seg].T)
    m["mo"] = shared["_mo"][b]
    m["cq"] = (ct * scale).astype(f32)
    m["sq"] = (st * scale).astype(f32)
    m["ck"] = ct
    m["sk"] = st
    return m


def _get_nc():
    if "nc" not in _CACHE:
        _CACHE["nc"] = build_nc()
    return _CACHE["nc"]


def kernel(**inputs) -> np.ndarray:
    nc = _get_nc()
    shared = _prep_shared(inputs)
    cores = [(b, s) for b in range(B) for s in range(4)]
    in_maps = [_prep_core(inputs, shared, b, s) for b, s in cores]
    from concourse import bass_utils
    import os
    res = bass_utils.run_bass_kernel_spmd(
        nc, in_maps, core_ids=list(range(NCORES)),
        trace=bool(os.environ.get("MAC_TRACE")))
    _CACHE["last_results"] = res
    out = np.empty((B, N, DIM), np.float32)
    for i, (b, s) in enumerate(cores):
        out[b, s * SEG:(s + 1) * SEG, :] = res.results[i]["yT"].T
    return out


# revision 21
# speedup vs baseline: 1.5736x; 1.0499x over previous
"""Trainium2 Bass kernel for nn_MACBlock (segmented attention + GEGLU FFN).

Sharding: 8 cores = 2 batches x 4 segments of 512 queries. The segment mask
makes attention block-diagonal (plus a 32-token always-visible prefix derived
from pooled memory + persistent memory), so each core is fully independent:
no collectives.

Layout: activations are kept feature-major (x^T [dim, tokens]) on-chip, so
every matmul contraction dim lands on partitions with zero transposes.
All heavy GEMMs run in bf16 (weights pre-cast on host, activations cast
on-chip): bf16 enables Fast Weight Load and avoids the fp32-HIGH power
throttle that halves the PE clock. PSUM accumulation stays fp32.
Scores are computed key-major ([keys, queries]); softmax is max-free; the
softmax denominator comes from a ones-column folded into the P@V stationary
operand (row DH of the same PSUM tile).
"""

import sys

if "/opt/trn_rl_repo" not in sys.path:
    sys.path.insert(0, "/opt/trn_rl_repo")

import numpy as np

B, N, DIM = 2, 2048, 1024
HEADS, DH = 16, 64
SEG = 512
NPM = NM = 16
PFX = NPM + NM          # 32 prefix keys
DFF = 2730
MFF = 22                # padded dff chunks
DFFP = MFF * 128        # 2816
KO = 8                  # 1024 / 128
P = 128
NCORES = 8
EPS = 1.1920929e-07

_CACHE = {}


def _f32r(ap):
    import concourse.mybir as mybir
    return ap.bitcast(mybir.dt.float32r)


def build_nc(reps=1):
    import concourse.bass as bass
    from concourse import bacc
    import concourse.tile as tile
    import concourse.mybir as mybir

    f32 = mybir.dt.float32
    bf16 = mybir.dt.bfloat16
    AF = mybir.ActivationFunctionType
    OP = mybir.AluOpType
    AX = mybir.AxisListType

    nc = bacc.Bacc("TRN2", target_bir_lowering=False, debug=False)

    dp = nc.declare_dram_parameter
    xT_d = dp("xT", [DIM, SEG], f32, isOutput=False)
    mo_d = dp("mo", [N, DIM], bf16, isOutput=False)
    cq_d = dp("cq", [P, SEG], f32, isOutput=False)
    sq_d = dp("sq", [P, SEG], f32, isOutput=False)
    ck_d = dp("ck", [P, SEG], f32, isOutput=False)
    sk_d = dp("sk", [P, SEG], f32, isOutput=False)
    mask_d = dp("maskD", [P, P], bf16, isOutput=False)
    rmat_d = dp("rmat", [P, P], bf16, isOutput=False)
    qkw_d = dp("qkw", [16, P, KO, P], bf16, isOutput=False)
    kvw_d = dp("kvw", [2, KO, P, DIM], bf16, isOutput=False)
    outw_d = dp("outw", [KO, P, KO, P], bf16, isOutput=False)
    w1a_d = dp("w1a", [MFF, P, KO, P], bf16, isOutput=False)
    w1g_d = dp("w1g", [MFF, P, KO, P], bf16, isOutput=False)
    w2_d = dp("w2", [KO, P, MFF, P], bf16, isOutput=False)
    mtw_d = dp("mtw", [KO, P, DIM], bf16, isOutput=False)
    pmv_d = dp("pmv", [HEADS, NPM, DH], bf16, isOutput=False)
    pmk_d = dp("pmk", [HEADS, DH, NPM], bf16, isOutput=False)
    b1a_d = dp("b1a", [P, MFF], f32, isOutput=False)
    b1g_d = dp("b1g", [P, MFF], f32, isOutput=False)
    b2_d = dp("b2", [P, KO], f32, isOutput=False)
    anw_d = dp("anw", [P, KO], f32, isOutput=False)
    fnw_d = dp("fnw", [P, KO], f32, isOutput=False)
    mpnw_d = dp("mpnw", [1, DIM], f32, isOutput=False)
    yT_d = dp("yT", [DIM, SEG], f32, isOutput=True)

    def _emit(nc):
      with tile.TileContext(nc) as tc, \
            nc.allow_low_precision(reason="bf16 matmul rounding"):
        from contextlib import ExitStack
        ctx = ExitStack()
        with ctx:
            persist = ctx.enter_context(tc.tile_pool(name="persist", bufs=1))
            wpool = ctx.enter_context(tc.tile_pool(name="wpool", bufs=5))
            kvpool = ctx.enter_context(tc.tile_pool(name="kvpool", bufs=2))
            w2pool = ctx.enter_context(tc.tile_pool(name="w2pool", bufs=3))
            mopool = ctx.enter_context(tc.tile_pool(name="mopool", bufs=6))
            rot = ctx.enter_context(tc.tile_pool(name="rot", bufs=2))
            epool = ctx.enter_context(tc.tile_pool(name="epool", bufs=3))
            pa = ctx.enter_context(tc.tile_pool(name="pa", bufs=4, space="PSUM"))
            psc = ctx.enter_context(tc.tile_pool(name="psc", bufs=2, space="PSUM"))
            pso = ctx.enter_context(tc.tile_pool(name="pso", bufs=2, space="PSUM"))

            cnt = [0]

            def pa_t():
                cnt[0] += 1
                return pa.tile([P, SEG], f32, tag="ps", name=f"pa{cnt[0]}")

            def psc_t():
                cnt[0] += 1
                return psc.tile([P, SEG], f32, tag="sc", name=f"sc{cnt[0]}")

            def pso_t():
                cnt[0] += 1
                return pso.tile([P, SEG], f32, tag="o", name=f"o{cnt[0]}")

            # ---------------- persistent SBUF tensors ----------------
            xT = persist.tile([P, KO, SEG], f32, tag="xT")       # x^T, later x1^T
            xnT = persist.tile([P, KO, SEG], bf16, tag="xnT")    # xn^T, later xn1^T
            kT = persist.tile([P, KO, SEG], bf16, tag="kT")      # roped k^T
            vA = persist.tile([P, 4, HEADS, DH + 1], bf16, tag="vA")  # v key-major
            vP = persist.tile([PFX, HEADS, DH + 1], bf16, tag="vP")   # prefix v rows
            kP = persist.tile([P, HEADS, PFX], bf16, tag="kP")   # prefix k^T @64*(h%2)
            oA = persist.tile([P, KO, SEG], bf16, tag="oA")      # attn o^T
            cq = persist.tile([P, SEG], f32, tag="cq")
            sq_ = persist.tile([P, SEG], f32, tag="sq")
            ck = persist.tile([P, SEG], f32, tag="ck")
            sk = persist.tile([P, SEG], f32, tag="sk")
            maskD = persist.tile([P, P], bf16, tag="maskD")
            rmat = persist.tile([P, P], bf16, tag="rmat")
            b1a = persist.tile([P, MFF], f32, tag="b1a")
            b1g = persist.tile([P, MFF], f32, tag="b1g")
            b2 = persist.tile([P, KO], f32, tag="b2")
            anw = persist.tile([P, KO], f32, tag="anw")
            fnw = persist.tile([P, KO], f32, tag="fnw")
            mpnw = persist.tile([1, DIM], f32, tag="mpnw")
            ones16 = persist.tile([1, 16], f32, tag="o16")
            ones11 = persist.tile([1, 1], f32, tag="o11")
            ones128b = persist.tile([P, 1], bf16, tag="o128b")   # lhsT K=128,M=1
            ones1xPb = persist.tile([1, P], bf16, tag="o1xPb")   # lhsT K=1,M=128
            ones1xP = persist.tile([1, P], f32, tag="o1xP")      # f32 variant
            pooledT = persist.tile([P, KO], bf16, tag="pooledT")
            memtokT = persist.tile([P, KO], bf16, tag="memtokT")
            mrow = persist.tile([1, 3 * DIM], f32, tag="mrow")
            rrow = persist.tile([1, 2 * SEG], f32, tag="rrow")
            rrowb = persist.tile([1, SEG], bf16, tag="rrowb")
            epsc = persist.tile([P, 1], f32, tag="epsc")
            zeroc = persist.tile([P, 1], f32, tag="zeroc")

            dma = nc.sync.dma_start
            dma(out=cq, in_=cq_d[:])
            dma(out=sq_, in_=sq_d[:])
            dma(out=ck, in_=ck_d[:])
            dma(out=sk, in_=sk_d[:])
            dma(out=maskD, in_=mask_d[:])
            dma(out=rmat, in_=rmat_d[:])
            dma(out=b1a, in_=b1a_d[:])
            dma(out=b1g, in_=b1g_d[:])
            dma(out=b2, in_=b2_d[:])
            dma(out=anw, in_=anw_d[:])
            dma(out=fnw, in_=fnw_d[:])
            dma(out=mpnw, in_=mpnw_d[:])
            nc.vector.memset(ones16, 1.0)
            nc.vector.memset(ones11, 1.0)
            nc.vector.memset(ones128b, 1.0)
            nc.vector.memset(ones1xPb, 1.0)
            nc.vector.memset(ones1xP, 1.0)
            nc.vector.memset(epsc, EPS)
            nc.vector.memset(zeroc, 0.0)
            # denominator ones-columns in the P@V stationary operands
            nc.vector.memset(vA[:, :, :, DH:DH + 1], 1.0)
            nc.vector.memset(vP[:, :, DH:DH + 1], 1.0)
            for h in range(HEADS):
                hb = DH * (h % 2)
                dma(out=kP[hb:hb + DH, h, NPM:PFX], in_=pmk_d[h])
                dma(out=vP[NPM:PFX, h, 0:DH], in_=pmv_d[h])

            if True:
              for ko in range(KO):
                  dma(out=xT[:, ko, :], in_=xT_d[ko * P:(ko + 1) * P, :])

              mm = nc.tensor.matmul

              def rmsnorm_into(dst, src, w_sb, sq_tag):
                  """dst[:,ko,:] = src[:,ko,:] * w[:,ko] * rsqrt(mean_dim(src^2)+eps)"""
                  ss = psc_t()  # [1,512] slice used
                  sq8 = persist.tile([P, KO, SEG], bf16, tag=sq_tag, name="sq8")
                  for ko in range(KO):
                      nc.vector.tensor_mul(sq8[:, ko, :], src[:, ko, :],
                                           src[:, ko, :])
                      mm(ss[0:1, :], ones128b, sq8[:, ko, :],
                         start=(ko == 0), stop=(ko == KO - 1))
                  nc.scalar.activation(rrow[:, 0:SEG], ss[0:1, :], AF.Sqrt,
                                       bias=epsc[0:1], scale=1.0 / DIM)
                  nc.vector.reciprocal_approx_fast(out=rrow[:, SEG:2 * SEG],
                                                   in_=rrow[:, 0:SEG])
                  nc.vector.tensor_copy(out=rrowb, in_=rrow[:, SEG:2 * SEG])
                  bc = pso_t()  # broadcast rstd over 128 partitions
                  mm(bc, ones1xPb, rrowb,
                     start=True, stop=True)
                  for ko in range(KO):
                      nc.vector.scalar_tensor_tensor(
                          out=dst[:, ko, :], in0=src[:, ko, :],
                          scalar=w_sb[:, ko:ko + 1], in1=bc,
                          op0=OP.mult, op1=OP.mult)

              # ---------------- attn rmsnorm ----------------
              rmsnorm_into(xnT, xT, anw, "big16")
              qT = persist.tile([P, KO, SEG], bf16, tag="qT")    # roped,scaled q^T

              # ---------------- q/k projections + rope, interleaved with
              # ---------------- mem_out mean accumulation ----------------
              mean_ps = [psc_t(), psc_t()]   # two [1,512] accumulators (slices)

              def mo_mean_step(t):
                  mot = mopool.tile([P, DIM], bf16, tag="mo", name="mot")
                  dma(out=mot, in_=mo_d[t * P:(t + 1) * P, :])
                  for half in range(2):
                      mm(mean_ps[half][0:1, :], ones128b,
                         mot[:, half * SEG:(half + 1) * SEG],
                         start=(t == 0), stop=(t == 15))

              # pooled rmsnorm (pure ACT/DVE): emitted mid-qk-loop so its
              # serial latency hides under the remaining projection matmuls
              pooled_raw = mrow[:, 0:DIM]
              sqr = mrow[:, DIM:2 * DIM]
              pooled = mrow[:, 2 * DIM:3 * DIM]

              def pooled_chain():
                  for half in range(2):
                      nc.scalar.activation(_f32r(pooled_raw[:, half * SEG:(half + 1) * SEG]),
                                           mean_ps[half][0:1, :], AF.Copy,
                                           scale=1.0 / N)
                  nc.vector.tensor_mul(_f32r(sqr), pooled_raw, pooled_raw)
                  nc.vector.reduce_sum(_f32r(sqr[:, 0:1]), sqr, axis=AX.X)
                  nc.scalar.activation(_f32r(sqr[:, 1:2]), sqr[:, 0:1], AF.Sqrt,
                                       bias=epsc[0:1], scale=1.0 / DIM)
                  nc.vector.reciprocal(_f32r(sqr[:, 2:3]), sqr[:, 1:2])
                  nc.vector.scalar_tensor_tensor(out=_f32r(pooled), in0=pooled_raw,
                                                 scalar=sqr[:, 2:3], in1=mpnw,
                                                 op0=OP.mult, op1=OP.mult)

              for m in range(16):
                  wt = wpool.tile([P, KO, P], bf16, tag="w8")
                  dma(out=wt, in_=qkw_d[m])
                  ps = pa_t()
                  for ko in range(KO):
                      mm(ps, wt[:, ko], xnT[:, ko, :],
                         start=(ko == 0), stop=(ko == KO - 1))
                  is_q = m < 8
                  c_t, s_t = (cq, sq_) if is_q else (ck, sk)
                  dst = qT if is_q else kT
                  ko_out = m % 8
                  qraw = rot.tile([P, SEG], bf16, tag="ropeA")
                  nc.scalar.copy(qraw, ps)
                  rps = pa_t()
                  mm(rps, rmat, qraw, start=True, stop=True)
                  At = rot.tile([P, SEG], bf16, tag="ropeB")
                  nc.vector.tensor_mul(At, ps, c_t)
                  Bt = rot.tile([P, SEG], bf16, tag="ropeA")
                  nc.vector.tensor_mul(Bt, rps, s_t)
                  nc.vector.tensor_add(dst[:, ko_out, :], At, Bt)
                  if 1 <= m < 9:
                      mo_mean_step(2 * (m - 1))
                      mo_mean_step(2 * (m - 1) + 1)
                  if m == 10:
                      pooled_chain()

              def stage_pT():
                  pT = pa_t()
                  for ko in range(KO):
                      mm(pT[:, ko:ko + 1], pooled[0:1, ko * P:(ko + 1) * P],
                         ones11, start=True, stop=True, skip_group_check=True)
                  nc.vector.tensor_copy(out=pooledT, in_=pT[:, 0:KO])

              mt_ps = []

              def stage_mt():
                  mt_ps.extend([psc_t(), psc_t()])
                  for ko in range(KO):
                      mtw_t = kvpool.tile([P, DIM], bf16, tag="kv")
                      dma(out=mtw_t, in_=mtw_d[ko])
                      for half in range(2):
                          mm(mt_ps[half][0:1, :], pooledT[:, ko:ko + 1],
                             mtw_t[:, half * SEG:(half + 1) * SEG],
                             start=(ko == 0), stop=(ko == KO - 1))

              memtok = mrow[:, 0:DIM]

              def stage_mT():
                  for half in range(2):
                      nc.scalar.activation(_f32r(memtok[:, half * SEG:(half + 1) * SEG]),
                                           mt_ps[half][0:1, :], AF.Copy)
                  mT = pa_t()
                  for ko in range(KO):
                      mm(mT[:, ko:ko + 1], memtok[0:1, ko * P:(ko + 1) * P],
                         ones11, start=True, stop=True, skip_group_check=True)
                  nc.vector.tensor_copy(out=memtokT, in_=mT[:, 0:KO])

              kc_row = mrow[:, DIM:2 * DIM]
              vc_row = mrow[:, 2 * DIM:3 * DIM]

              def stage_kcvc(c):
                  r_ps = [psc_t(), psc_t()]
                  for ko in range(KO):
                      kv_t = kvpool.tile([P, DIM], bf16, tag="kv")
                      dma(out=kv_t, in_=kvw_d[c, ko])
                      for half in range(2):
                          mm(r_ps[half][0:1, :], memtokT[:, ko:ko + 1],
                             kv_t[:, half * SEG:(half + 1) * SEG],
                             start=(ko == 0), stop=(ko == KO - 1))
                  row = kc_row if c == 0 else vc_row
                  for half in range(2):
                      nc.scalar.activation(_f32r(row[:, half * SEG:(half + 1) * SEG]),
                                           r_ps[half][0:1, :], AF.Copy)

              def stage_kx():
                  for j in range(KO):  # 2 heads per chunk
                      kx = pa_t()
                      mm(kx[:, 0:16], _f32r(kc_row[0:1, j * P:(j + 1) * P]),
                         _f32r(ones16), start=True, stop=True,
                         skip_group_check=True)
                      nc.vector.tensor_copy(out=kP[0:DH, 2 * j, 0:NPM],
                                            in_=kx[0:DH, 0:16])
                      nc.vector.tensor_copy(out=kP[DH:P, 2 * j + 1, 0:NPM],
                                            in_=kx[DH:P, 0:16])

              def stage_vx():
                  for half in range(2):
                      vx = pa_t()
                      mm(vx[0:16, :], _f32r(ones16),
                         _f32r(vc_row[0:1, half * SEG:(half + 1) * SEG]),
                         start=True, stop=True, skip_group_check=True)
                      nc.vector.tensor_copy(
                          out=vP[0:NPM, half * 8:(half + 1) * 8, 0:DH],
                          in_=vx[0:16, :].rearrange("p (h d) -> p h d", d=DH))

              stage_pT()  # pooled ready ~3 qk-iterations ago
              stages = [stage_mt, stage_mT,
                        lambda: stage_kcvc(0), lambda: stage_kcvc(1),
                        stage_kx, stage_vx]

              # ---------------- v projection (token-major), interleaved ----
              for half in range(2):
                  kvv = persist.tile([P, KO, SEG], bf16, tag="big16")
                  for ko in range(KO):
                      dma(out=kvv[:, ko, :],
                          in_=kvw_d[1, ko, :, half * SEG:(half + 1) * SEG])
                  for tc_ in range(4):
                      ps = pa_t()
                      for ko in range(KO):
                          mm(ps, xnT[:, ko, tc_ * P:(tc_ + 1) * P],
                             kvv[:, ko, :],
                             start=(ko == 0), stop=(ko == KO - 1))
                      nc.vector.tensor_copy(
                          out=vA[:, tc_, half * 8:(half + 1) * 8, 0:DH],
                          in_=ps.rearrange("p (h d) -> p h d", d=DH))
                      if (half, tc_) != (0, 0) and stages:
                          stages.pop(0)()
              while stages:
                  stages.pop(0)()

              # ---------------- attention heads ----------------
              # Scores chunk c covers keys [cP,(c+1)P) x queries [cP,SEG)
              # (block-triangular). Chunks 1 and 3 share one PSUM tile /
              # exp pass: c1 at free 0:384 (queries 128:512), c3 at free
              # 384:512 (queries 384:512). The P@V stationary has a ones
              # column, so PSUM row DH is the softmax denominator.
              for h in range(HEADS):
                  ko_h, hf = h // 2, h % 2
                  qr = DH * hf
                  q_h = qT[qr:qr + DH, ko_h, :]
                  k_h = kT[qr:qr + DH, ko_h, :]
                  # prefix scores [32, 512]
                  scp = psc_t()
                  mm(scp[0:PFX, :], kP[qr:qr + DH, h, :], q_h,
                     start=True, stop=True, skip_group_check=True)
                  eP = epool.tile([PFX, SEG], bf16, tag="eP")
                  nc.scalar.activation(eP, scp[0:PFX, :], AF.Exp,
                                       bias=zeroc[0:PFX])
                  sc0 = psc_t()
                  mm(sc0[:, :], k_h[:, 0:P], q_h,
                     start=True, stop=True, skip_group_check=True)
                  e0 = epool.tile([P, SEG], bf16, tag="e0")
                  nc.scalar.activation(e0, sc0, AF.Exp, bias=zeroc)
                  nc.vector.tensor_mul(e0[:, 0:P], e0[:, 0:P], maskD)
                  sc13 = psc_t()
                  mm(sc13[:, 0:384], k_h[:, P:2 * P], q_h[:, P:],
                     start=True, stop=True, skip_group_check=True)
                  mm(sc13[:, 384:512], k_h[:, 3 * P:4 * P], q_h[:, 3 * P:],
                     start=True, stop=True, skip_group_check=True)
                  eB = epool.tile([P, SEG], bf16, tag="eB")
                  nc.scalar.activation(eB, sc13, AF.Exp, bias=zeroc)
                  nc.vector.tensor_mul(eB[:, 0:P], eB[:, 0:P], maskD)
                  nc.vector.tensor_mul(eB[:, 384:512], eB[:, 384:512], maskD)
                  sc2 = psc_t()
                  mm(sc2[:, 0:256], k_h[:, 2 * P:3 * P], q_h[:, 2 * P:],
                     start=True, stop=True, skip_group_check=True)
                  e2 = epool.tile([P, 256], bf16, tag="e2")
                  nc.scalar.activation(e2, sc2[:, 0:256], AF.Exp, bias=zeroc)
                  nc.vector.tensor_mul(e2[:, 0:P], e2[:, 0:P], maskD)
                  # P@V with ones-column: row DH = softmax denominator
                  po = pa_t()  # 4-buf pool so heads pipeline
                  mm(po[0:DH + 1, :], vP[:, h, :], eP,
                     start=True, stop=False, skip_group_check=True)
                  mm(po[0:DH + 1, 0:], vA[:, 0, h, :], e0,
                     start=False, stop=False, skip_group_check=True)
                  mm(po[0:DH + 1, P:], vA[:, 1, h, :], eB[:, 0:384],
                     start=False, stop=False, skip_group_check=True)
                  mm(po[0:DH + 1, 2 * P:], vA[:, 2, h, :], e2,
                     start=False, stop=False, skip_group_check=True)
                  mm(po[0:DH + 1, 3 * P:], vA[:, 3, h, :], eB[:, 384:512],
                     start=False, stop=True, skip_group_check=True)
                  drow = rot.tile([1, SEG], f32, tag="dr")
                  nc.scalar.copy(drow, po[DH:DH + 1, :])
                  rvf = rot.tile([1, SEG], f32, tag="rv")
                  nc.vector.reciprocal_approx_fast(out=rvf, in_=drow)
                  rvb = rot.tile([1, SEG], bf16, tag="rvb")
                  nc.vector.tensor_copy(out=rvb, in_=rvf)
                  bcp = pso_t()  # broadcast 1/denom over DH partitions
                  mm(bcp[0:DH, :], ones1xPb[0:1, 0:DH], rvb,
                     start=True, stop=True, skip_group_check=True)
                  bcs = epool.tile([DH, SEG], bf16, tag="bcs")
                  nc.vector.tensor_copy(out=bcs, in_=bcp[0:DH, :])
                  nc.vector.tensor_mul(oA[qr:qr + DH, ko_h, :],
                                       po[0:DH, :], bcs)

              # ---------------- output projection + residual ----------------
              for m in range(KO):
                  wt = wpool.tile([P, KO, P], bf16, tag="w8")
                  dma(out=wt, in_=outw_d[m])
                  ps = pa_t()
                  for k in range(KO):
                      mm(ps, wt[:, k], oA[:, k, :],
                         start=(k == 0), stop=(k == KO - 1))
                  nc.vector.tensor_add(xT[:, m, :], ps, xT[:, m, :])  # x1

              # ---------------- FFN ----------------
              rmsnorm_into(xnT, xT, fnw, "big16")  # xn1^T
              u_parts = [qT, kT]  # reuse dead slots as u storage
              u_c = persist.tile([P, 6, SEG], bf16, tag="big16")

              def u_slice(k):
                  if k < 8:
                      return u_parts[0][:, k, :]
                  if k < 16:
                      return u_parts[1][:, k - 8, :]
                  return u_c[:, k - 16, :]

              for m in range(MFF):
                  wa = wpool.tile([P, KO, P], bf16, tag="w8")
                  dma(out=wa, in_=w1a_d[m])
                  wg = wpool.tile([P, KO, P], bf16, tag="w8")
                  dma(out=wg, in_=w1g_d[m])
                  psa = pa_t()
                  psg = pa_t()
                  for ko in range(KO):
                      mm(psa, wa[:, ko], xnT[:, ko, :],
                         start=(ko == 0), stop=(ko == KO - 1))
                      mm(psg, wg[:, ko], xnT[:, ko, :],
                         start=(ko == 0), stop=(ko == KO - 1))
                  sig = rot.tile([P, SEG], f32, tag="ropeA")
                  nc.scalar.activation(sig, psg, AF.Sigmoid,
                                       bias=b1g[:, m:m + 1], scale=1.0)
                  silu = rot.tile([P, SEG], f32, tag="ropeB")
                  nc.vector.scalar_tensor_tensor(
                      out=silu, in0=psg, scalar=b1g[:, m:m + 1],
                      in1=sig, op0=OP.add, op1=OP.mult)
                  nc.vector.scalar_tensor_tensor(
                      out=u_slice(m), in0=psa, scalar=b1a[:, m:m + 1],
                      in1=silu, op0=OP.add, op1=OP.mult)

              for o in range(KO):
                  ps = pa_t()
                  for half in range(2):
                      w2t = w2pool.tile([P, 11, P], bf16, tag="w2")
                      dma(out=w2t, in_=w2_d[o][:, half * 11:(half + 1) * 11, :])
                      for k2 in range(11):
                          k = half * 11 + k2
                          mm(ps, w2t[:, k2], u_slice(k),
                             start=(k == 0), stop=(k == MFF - 1))
                  outT = persist.tile([P, KO, SEG], f32, tag="outT",
                                      name=f"outT{o}")
                  nc.vector.scalar_tensor_tensor(
                      out=outT[:, o, :], in0=ps, scalar=b2[:, o:o + 1],
                      in1=xT[:, o, :], op0=OP.add, op1=OP.add)
                  dma(out=yT_d[o * P:(o + 1) * P, :], in_=outT[:, o, :])

    for _rep in range(reps):
        _emit(nc)
    nc.compile()
    return nc


# ======================= host-side preparation =======================

def _prep_shared(inputs):
    import ml_dtypes
    f32 = np.float32
    bf16 = ml_dtypes.bfloat16
    qkv = np.asarray(inputs["to_qkv_w"], f32)
    shared = {}
    shared["qkw"] = np.ascontiguousarray(
        qkv[:, :2048].reshape(KO, P, 16, P).transpose(2, 1, 0, 3)).astype(bf16)
    shared["kvw"] = np.ascontiguousarray(
        np.stack([qkv[:, 1024:2048], qkv[:, 2048:3072]])
        .reshape(2, KO, P, DIM)).astype(bf16)
    shared["outw"] = np.ascontiguousarray(
        np.asarray(inputs["to_out_w"], f32)
        .reshape(KO, P, KO, P).transpose(2, 1, 0, 3)).astype(bf16)
    w1 = np.asarray(inputs["ff_w1"], f32)
    w1a = np.zeros((DIM, DFFP), f32)
    w1g = np.zeros((DIM, DFFP), f32)
    w1a[:, :DFF] = w1[:, :DFF]
    w1g[:, :DFF] = w1[:, DFF:]
    shared["w1a"] = np.ascontiguousarray(
        w1a.reshape(KO, P, MFF, P).transpose(2, 1, 0, 3)).astype(bf16)
    shared["w1g"] = np.ascontiguousarray(
        w1g.reshape(KO, P, MFF, P).transpose(2, 1, 0, 3)).astype(bf16)
    w2 = np.zeros((DFFP, DIM), f32)
    w2[:DFF] = np.asarray(inputs["ff_w2"], f32)
    shared["w2"] = np.ascontiguousarray(
        w2.reshape(MFF, P, KO, P).transpose(2, 1, 0, 3)).astype(bf16)
    shared["mtw"] = np.ascontiguousarray(
        np.asarray(inputs["to_mem_tokens_w"], f32).reshape(KO, P, DIM)).astype(bf16)
    pm = np.asarray(inputs["persist_mem"], f32)
    shared["pmv"] = np.ascontiguousarray(pm).astype(bf16)
    shared["pmk"] = np.ascontiguousarray(pm.transpose(0, 2, 1)).astype(bf16)
    b1 = np.asarray(inputs["ff_b1"], f32)
    b1a = np.zeros(DFFP, f32)
    b1g = np.zeros(DFFP, f32)
    b1a[:DFF] = b1[:DFF]
    b1g[:DFF] = b1[DFF:]
    shared["b1a"] = np.ascontiguousarray(b1a.reshape(MFF, P).T)
    shared["b1g"] = np.ascontiguousarray(b1g.reshape(MFF, P).T)
    shared["b2"] = np.ascontiguousarray(
        np.asarray(inputs["ff_b2"], f32).reshape(KO, P).T)
    shared["anw"] = np.ascontiguousarray(
        np.asarray(inputs["attn_norm_w"], f32).reshape(KO, P).T)
    shared["fnw"] = np.ascontiguousarray(
        np.asarray(inputs["ff_norm_w"], f32).reshape(KO, P).T)
    shared["mpnw"] = np.ascontiguousarray(
        np.asarray(inputs["mem_pool_norm_w"], f32).reshape(1, DIM))
    rl = np.zeros((P, P), f32)
    ii = np.arange(0, P, 2)
    rl[ii + 1, ii] = f32(-1.0)
    rl[ii, ii + 1] = f32(1.0)
    shared["rmat"] = rl.astype(bf16)
    shared["maskD"] = np.where(
        np.arange(P)[None, :] >= np.arange(P)[:, None], f32(1.0), f32(0.0)
    ).astype(bf16)

    # per-batch bf16 mem_out
    mo = np.asarray(inputs["mem_out"], f32)
    shared["_mo"] = [np.ascontiguousarray(mo[b]).astype(bf16) for b in range(B)]

    # rope tables, float32 math to match the reference
    pos = np.arange(N, dtype=f32)
    expo = (np.arange(0, DH, 2).astype(f32) / f32(DH)).astype(f32)
    inv = (f32(1.0) / np.power(f32(10000.0), expo)).astype(f32)
    ang = np.repeat(pos[:, None] * inv[None, :], 2, axis=1).astype(f32)
    cosf, sinf = np.cos(ang).astype(f32), np.sin(ang).astype(f32)
    scale = f32(DH ** -0.5)
    shared["_cos"], shared["_sin"], shared["_scale"] = cosf, sinf, scale
    return shared


def _prep_core(inputs, shared, b, s):
    f32 = np.float32
    x = np.asarray(inputs["x"], f32)
    cosf, sinf, scale = shared["_cos"], shared["_sin"], shared["_scale"]
    seg = slice(s * SEG, (s + 1) * SEG)
    ct = np.ascontiguousarray(np.tile(cosf[seg].T, (2, 1)))
    st = np.ascontiguousarray(np.tile(sinf[seg].T, (2, 1)))
    m = {k: v for k, v in shared.items() if not k.startswith("_")}
    m["xT"] = np.ascontiguousarray(x[b, seg].T)
    m["mo"] = shared["_mo"][b]
    m["cq"] = (ct * scale).astype(f32)
    m["sq"] = (st * scale).astype(f32)
    m["ck"] = ct
    m["sk"] = st
    return m


def _get_nc():
    if "nc" not in _CACHE:
        _CACHE["nc"] = build_nc()
    return _CACHE["nc"]


def kernel(**inputs) -> np.ndarray:
    nc = _get_nc()
    shared = _prep_shared(inputs)
    cores = [(b, s) for b in range(B) for s in range(4)]
    in_maps = [_prep_core(inputs, shared, b, s) for b, s in cores]
    from concourse import bass_utils
    import os
    res = bass_utils.run_bass_kernel_spmd(
        nc, in_maps, core_ids=list(range(NCORES)),
        trace=bool(os.environ.get("MAC_TRACE")))
    _CACHE["last_results"] = res
    out = np.empty((B, N, DIM), np.float32)
    for i, (b, s) in enumerate(cores):
        out[b, s * SEG:(s + 1) * SEG, :] = res.results[i]["yT"].T
    return out


# revision 28
# speedup vs baseline: 1.6191x; 1.0290x over previous
"""Trainium2 Bass kernel for nn_MACBlock (segmented attention + GEGLU FFN).

Sharding: 8 cores = 2 batches x 4 segments of 512 queries. The segment mask
makes attention block-diagonal (plus a 32-token always-visible prefix derived
from pooled memory + persistent memory), so each core is fully independent:
no collectives.

Layout: activations are kept feature-major (x^T [dim, tokens]) on-chip, so
every matmul contraction dim lands on partitions with zero transposes.
All heavy GEMMs run in bf16 (weights pre-cast on host, activations cast
on-chip): bf16 enables Fast Weight Load and avoids the fp32-HIGH power
throttle that halves the PE clock. PSUM accumulation stays fp32.
Scores are computed key-major ([keys, queries]); softmax is max-free; the
softmax denominator comes from a ones-column folded into the P@V stationary
operand (row DH of the same PSUM tile).
"""

import sys

if "/opt/trn_rl_repo" not in sys.path:
    sys.path.insert(0, "/opt/trn_rl_repo")

import numpy as np

B, N, DIM = 2, 2048, 1024
HEADS, DH = 16, 64
SEG = 512
NPM = NM = 16
PFX = NPM + NM          # 32 prefix keys
DFF = 2730
MFF = 22                # padded dff chunks
DFFP = MFF * 128        # 2816
KO = 8                  # 1024 / 128
P = 128
NCORES = 8
EPS = 1.1920929e-07

_CACHE = {}


def _f32r(ap):
    import concourse.mybir as mybir
    return ap.bitcast(mybir.dt.float32r)


def build_nc(reps=1):
    import concourse.bass as bass
    from concourse import bacc
    import concourse.tile as tile
    import concourse.mybir as mybir

    f32 = mybir.dt.float32
    bf16 = mybir.dt.bfloat16
    AF = mybir.ActivationFunctionType
    OP = mybir.AluOpType
    AX = mybir.AxisListType

    nc = bacc.Bacc("TRN2", target_bir_lowering=False, debug=False)

    dp = nc.declare_dram_parameter
    xT_d = dp("xT", [DIM, SEG], f32, isOutput=False)
    mo_d = dp("mo", [N, DIM], bf16, isOutput=False)
    cq_d = dp("cq", [P, SEG], f32, isOutput=False)
    sq_d = dp("sq", [P, SEG], f32, isOutput=False)
    ck_d = dp("ck", [P, SEG], f32, isOutput=False)
    sk_d = dp("sk", [P, SEG], f32, isOutput=False)
    mask_d = dp("maskD", [P, P], bf16, isOutput=False)
    rmat_d = dp("rmat", [P, P], bf16, isOutput=False)
    qkw_d = dp("qkw", [16, P, KO, P], bf16, isOutput=False)
    kvw_d = dp("kvw", [2, KO, P, DIM], bf16, isOutput=False)
    outw_d = dp("outw", [KO, P, KO, P], bf16, isOutput=False)
    w1a_d = dp("w1a", [MFF, P, KO, P], bf16, isOutput=False)
    w1g_d = dp("w1g", [MFF, P, KO, P], bf16, isOutput=False)
    w2_d = dp("w2", [KO, P, MFF, P], bf16, isOutput=False)
    mtw_d = dp("mtw", [KO, P, DIM], bf16, isOutput=False)
    pmv_d = dp("pmv", [HEADS, NPM, DH], bf16, isOutput=False)
    pmk_d = dp("pmk", [HEADS, DH, NPM], bf16, isOutput=False)
    b1a_d = dp("b1a", [P, MFF], f32, isOutput=False)
    b1g_d = dp("b1g", [P, MFF], f32, isOutput=False)
    b2_d = dp("b2", [P, KO], f32, isOutput=False)
    anw_d = dp("anw", [P, KO], f32, isOutput=False)
    fnw_d = dp("fnw", [P, KO], f32, isOutput=False)
    mpnw_d = dp("mpnw", [1, DIM], f32, isOutput=False)
    yT_d = dp("yT", [DIM, SEG], f32, isOutput=True)

    def _emit(nc):
      with tile.TileContext(nc) as tc, \
            nc.allow_low_precision(reason="bf16 matmul rounding"):
        from contextlib import ExitStack
        ctx = ExitStack()
        with ctx:
            persist = ctx.enter_context(tc.tile_pool(name="persist", bufs=1))
            wpool = ctx.enter_context(tc.tile_pool(name="wpool", bufs=5))
            kvpool = ctx.enter_context(tc.tile_pool(name="kvpool", bufs=2))
            w2pool = ctx.enter_context(tc.tile_pool(name="w2pool", bufs=3))
            mopool = ctx.enter_context(tc.tile_pool(name="mopool", bufs=6))
            rot = ctx.enter_context(tc.tile_pool(name="rot", bufs=2))
            epool = ctx.enter_context(tc.tile_pool(name="epool", bufs=3))
            pa = ctx.enter_context(tc.tile_pool(name="pa", bufs=4, space="PSUM"))
            psc = ctx.enter_context(tc.tile_pool(name="psc", bufs=2, space="PSUM"))
            pso = ctx.enter_context(tc.tile_pool(name="pso", bufs=2, space="PSUM"))

            cnt = [0]

            def pa_t():
                cnt[0] += 1
                return pa.tile([P, SEG], f32, tag="ps", name=f"pa{cnt[0]}")

            def psc_t():
                cnt[0] += 1
                return psc.tile([P, SEG], f32, tag="sc", name=f"sc{cnt[0]}")

            def pso_t():
                cnt[0] += 1
                return pso.tile([P, SEG], f32, tag="o", name=f"o{cnt[0]}")

            # ---------------- persistent SBUF tensors ----------------
            xT = persist.tile([P, KO, SEG], f32, tag="xT")       # x^T, later x1^T
            xnT = persist.tile([P, KO, SEG], bf16, tag="xnT")    # xn^T, later xn1^T
            kT = persist.tile([P, KO, SEG], bf16, tag="kT")      # roped k^T
            vA = persist.tile([P, 4, HEADS, DH + 1], bf16, tag="vA")  # v key-major
            vP = persist.tile([PFX, HEADS, DH + 1], bf16, tag="vP")   # prefix v rows
            kP = persist.tile([P, HEADS, PFX], bf16, tag="kP")   # prefix k^T @64*(h%2)
            oA = persist.tile([P, KO, SEG], bf16, tag="oA")      # attn o^T
            cq = persist.tile([P, SEG], f32, tag="cq")
            sq_ = persist.tile([P, SEG], f32, tag="sq")
            ck = persist.tile([P, SEG], f32, tag="ck")
            sk = persist.tile([P, SEG], f32, tag="sk")
            maskD = persist.tile([P, P], bf16, tag="maskD")
            rmat = persist.tile([P, P], bf16, tag="rmat")
            b1a = persist.tile([P, MFF], f32, tag="b1a")
            b1g = persist.tile([P, MFF], f32, tag="b1g")
            b2 = persist.tile([P, KO], f32, tag="b2")
            anw = persist.tile([P, KO], f32, tag="anw")
            fnw = persist.tile([P, KO], f32, tag="fnw")
            mpnw = persist.tile([1, DIM], f32, tag="mpnw")
            ones16 = persist.tile([1, 16], f32, tag="o16")
            ones11 = persist.tile([1, 1], f32, tag="o11")
            ones128b = persist.tile([P, 1], bf16, tag="o128b")   # lhsT K=128,M=1
            ones1xPb = persist.tile([1, P], bf16, tag="o1xPb")   # lhsT K=1,M=128
            ones1xP = persist.tile([1, P], f32, tag="o1xP")      # f32 variant
            pooledT = persist.tile([P, KO], bf16, tag="pooledT")
            memtokT = persist.tile([P, KO], bf16, tag="memtokT")
            mrow = persist.tile([1, 3 * DIM], f32, tag="mrow")
            rrow = persist.tile([1, 2 * SEG], f32, tag="rrow")
            rrowb = persist.tile([1, SEG], bf16, tag="rrowb")
            epsc = persist.tile([P, 1], f32, tag="epsc")
            zeroc = persist.tile([P, 1], f32, tag="zeroc")

            dma = nc.sync.dma_start
            dma(out=cq, in_=cq_d[:])
            dma(out=sq_, in_=sq_d[:])
            dma(out=ck, in_=ck_d[:])
            dma(out=sk, in_=sk_d[:])
            dma(out=maskD, in_=mask_d[:])
            dma(out=rmat, in_=rmat_d[:])
            dma(out=b1a, in_=b1a_d[:])
            dma(out=b1g, in_=b1g_d[:])
            dma(out=b2, in_=b2_d[:])
            dma(out=anw, in_=anw_d[:])
            dma(out=fnw, in_=fnw_d[:])
            dma(out=mpnw, in_=mpnw_d[:])
            nc.vector.memset(ones16, 1.0)
            nc.vector.memset(ones11, 1.0)
            nc.vector.memset(ones128b, 1.0)
            nc.vector.memset(ones1xPb, 1.0)
            nc.vector.memset(ones1xP, 1.0)
            nc.vector.memset(epsc, EPS)
            nc.vector.memset(zeroc, 0.0)
            # denominator ones-columns in the P@V stationary operands
            nc.vector.memset(vA[:, :, :, DH:DH + 1], 1.0)
            nc.vector.memset(vP[:, :, DH:DH + 1], 1.0)
            for h in range(HEADS):
                hb = DH * (h % 2)
                dma(out=kP[hb:hb + DH, h, NPM:PFX], in_=pmk_d[h])
                dma(out=vP[NPM:PFX, h, 0:DH], in_=pmv_d[h])

            if True:
              for ko in range(KO):
                  dma(out=xT[:, ko, :], in_=xT_d[ko * P:(ko + 1) * P, :])

              mm = nc.tensor.matmul

              def rmsnorm_into(dst, src, w_sb, sq_tag):
                  """dst[:,ko,:] = src[:,ko,:] * w[:,ko] * rsqrt(mean_dim(src^2)+eps)"""
                  ss = psc_t()  # [1,512] slice used
                  sq8 = persist.tile([P, KO, SEG], bf16, tag=sq_tag, name="sq8")
                  for ko in range(KO):
                      nc.vector.tensor_mul(sq8[:, ko, :], src[:, ko, :],
                                           src[:, ko, :])
                      mm(ss[0:1, :], ones128b, sq8[:, ko, :],
                         start=(ko == 0), stop=(ko == KO - 1))
                  nc.scalar.activation(rrow[:, 0:SEG], ss[0:1, :], AF.Sqrt,
                                       bias=epsc[0:1], scale=1.0 / DIM)
                  nc.vector.reciprocal_approx_fast(out=rrow[:, SEG:2 * SEG],
                                                   in_=rrow[:, 0:SEG])
                  nc.vector.tensor_copy(out=rrowb, in_=rrow[:, SEG:2 * SEG])
                  bc = pso_t()  # broadcast rstd over 128 partitions
                  mm(bc, ones1xPb, rrowb,
                     start=True, stop=True)
                  for ko in range(KO):
                      nc.vector.scalar_tensor_tensor(
                          out=dst[:, ko, :], in0=src[:, ko, :],
                          scalar=w_sb[:, ko:ko + 1], in1=bc,
                          op0=OP.mult, op1=OP.mult)

              # ---------------- attn rmsnorm ----------------
              rmsnorm_into(xnT, xT, anw, "big16")
              qT = persist.tile([P, KO, SEG], bf16, tag="qT")    # roped,scaled q^T

              # ---------------- q/k projections + rope, interleaved with
              # ---------------- mem_out mean accumulation ----------------
              mean_ps = [psc_t(), psc_t()]   # two [1,512] accumulators (slices)

              def mo_mean_step(t):
                  mot = mopool.tile([P, DIM], bf16, tag="mo", name="mot")
                  dma(out=mot, in_=mo_d[t * P:(t + 1) * P, :])
                  for half in range(2):
                      mm(mean_ps[half][0:1, :], ones128b,
                         mot[:, half * SEG:(half + 1) * SEG],
                         start=(t == 0), stop=(t == 15))

              # pooled rmsnorm (pure ACT/DVE): emitted mid-qk-loop so its
              # serial latency hides under the remaining projection matmuls
              pooled_raw = mrow[:, 0:DIM]
              sqr = mrow[:, DIM:2 * DIM]
              pooled = mrow[:, 2 * DIM:3 * DIM]

              def pooled_chain():
                  for half in range(2):
                      nc.scalar.activation(_f32r(pooled_raw[:, half * SEG:(half + 1) * SEG]),
                                           mean_ps[half][0:1, :], AF.Copy,
                                           scale=1.0 / N)
                  nc.vector.tensor_mul(_f32r(sqr), pooled_raw, pooled_raw)
                  nc.vector.reduce_sum(_f32r(sqr[:, 0:1]), sqr, axis=AX.X)
                  nc.scalar.activation(_f32r(sqr[:, 1:2]), sqr[:, 0:1], AF.Sqrt,
                                       bias=epsc[0:1], scale=1.0 / DIM)
                  nc.vector.reciprocal(_f32r(sqr[:, 2:3]), sqr[:, 1:2])
                  nc.vector.scalar_tensor_tensor(out=_f32r(pooled), in0=pooled_raw,
                                                 scalar=sqr[:, 2:3], in1=mpnw,
                                                 op0=OP.mult, op1=OP.mult)

              # software-pipelined: the rope finish (rmat matmul + DVE
              # combine) for iteration m is emitted during iteration m+1 so
              # the ACT qraw copy never stalls the PE FIFO
              def rope_finish(ps, qraw, m):
                  is_q = m < 8
                  c_t, s_t = (cq, sq_) if is_q else (ck, sk)
                  dst = qT if is_q else kT
                  rps = pa_t()
                  mm(rps, rmat, qraw, start=True, stop=True)
                  At = rot.tile([P, SEG], bf16, tag="ropeB")
                  nc.vector.tensor_mul(At, ps, c_t)
                  Bt = rot.tile([P, SEG], bf16, tag="ropeA")
                  nc.vector.tensor_mul(Bt, rps, s_t)
                  nc.vector.tensor_add(dst[:, m % 8, :], At, Bt)

              pend = None
              for m in range(16):
                  wt = wpool.tile([P, KO, P], bf16, tag="w8")
                  dma(out=wt, in_=qkw_d[m])
                  ps = pa_t()
                  for ko in range(KO):
                      mm(ps, wt[:, ko], xnT[:, ko, :],
                         start=(ko == 0), stop=(ko == KO - 1))
                  qraw = rot.tile([P, SEG], bf16, tag="qraw")
                  nc.scalar.copy(qraw, ps)
                  if pend is not None:
                      rope_finish(*pend)
                  pend = (ps, qraw, m)
                  if 1 <= m < 9:
                      mo_mean_step(2 * (m - 1))
                      mo_mean_step(2 * (m - 1) + 1)
                  if m == 10:
                      pooled_chain()
              rope_finish(*pend)

              def stage_pT():
                  pT = pa_t()
                  for ko in range(KO):
                      mm(pT[:, ko:ko + 1], pooled[0:1, ko * P:(ko + 1) * P],
                         ones11, start=True, stop=True, skip_group_check=True)
                  nc.vector.tensor_copy(out=pooledT, in_=pT[:, 0:KO])

              mt_ps = []

              def stage_mt():
                  mt_ps.extend([psc_t(), psc_t()])
                  for ko in range(KO):
                      mtw_t = kvpool.tile([P, DIM], bf16, tag="kv")
                      dma(out=mtw_t, in_=mtw_d[ko])
                      for half in range(2):
                          mm(mt_ps[half][0:1, :], pooledT[:, ko:ko + 1],
                             mtw_t[:, half * SEG:(half + 1) * SEG],
                             start=(ko == 0), stop=(ko == KO - 1))

              memtok = mrow[:, 0:DIM]

              def stage_mT():
                  for half in range(2):
                      nc.scalar.activation(_f32r(memtok[:, half * SEG:(half + 1) * SEG]),
                                           mt_ps[half][0:1, :], AF.Copy)
                  mT = pa_t()
                  for ko in range(KO):
                      mm(mT[:, ko:ko + 1], memtok[0:1, ko * P:(ko + 1) * P],
                         ones11, start=True, stop=True, skip_group_check=True)
                  nc.vector.tensor_copy(out=memtokT, in_=mT[:, 0:KO])

              kc_row = mrow[:, DIM:2 * DIM]
              vc_row = mrow[:, 2 * DIM:3 * DIM]

              def stage_kcvc(c):
                  r_ps = [psc_t(), psc_t()]
                  for ko in range(KO):
                      kv_t = kvpool.tile([P, DIM], bf16, tag="kv")
                      dma(out=kv_t, in_=kvw_d[c, ko])
                      for half in range(2):
                          mm(r_ps[half][0:1, :], memtokT[:, ko:ko + 1],
                             kv_t[:, half * SEG:(half + 1) * SEG],
                             start=(ko == 0), stop=(ko == KO - 1))
                  row = kc_row if c == 0 else vc_row
                  for half in range(2):
                      nc.scalar.activation(_f32r(row[:, half * SEG:(half + 1) * SEG]),
                                           r_ps[half][0:1, :], AF.Copy)

              def stage_kx_j(j):  # 2 heads per chunk
                  kx = pa_t()
                  mm(kx[:, 0:16], _f32r(kc_row[0:1, j * P:(j + 1) * P]),
                     _f32r(ones16), start=True, stop=True,
                     skip_group_check=True)
                  nc.vector.tensor_copy(out=kP[0:DH, 2 * j, 0:NPM],
                                        in_=kx[0:DH, 0:16])
                  nc.vector.tensor_copy(out=kP[DH:P, 2 * j + 1, 0:NPM],
                                        in_=kx[DH:P, 0:16])

              def stage_vx_half(half):
                  vx = pa_t()
                  mm(vx[0:16, :], _f32r(ones16),
                     _f32r(vc_row[0:1, half * SEG:(half + 1) * SEG]),
                     start=True, stop=True, skip_group_check=True)
                  nc.vector.tensor_copy(
                      out=vP[0:NPM, half * 8:(half + 1) * 8, 0:DH],
                      in_=vx[0:16, :].rearrange("p (h d) -> p h d", d=DH))

              stage_pT()  # pooled ready ~3 qk-iterations ago
              stage_sched = {1: stage_mt, 3: stage_mT,
                             5: lambda: stage_kcvc(0),
                             6: lambda: stage_kcvc(1)}

              # ---------------- v projection (token-major), interleaved ----
              # (kx/vx stages are woven into the attention head loop below)
              for half in range(2):
                  kvv = persist.tile([P, KO, SEG], bf16, tag="big16")
                  for ko in range(KO):
                      dma(out=kvv[:, ko, :],
                          in_=kvw_d[1, ko, :, half * SEG:(half + 1) * SEG])
                  for tc_ in range(4):
                      ps = pa_t()
                      for ko in range(KO):
                          mm(ps, xnT[:, ko, tc_ * P:(tc_ + 1) * P],
                             kvv[:, ko, :],
                             start=(ko == 0), stop=(ko == KO - 1))
                      nc.vector.tensor_copy(
                          out=vA[:, tc_, half * 8:(half + 1) * 8, 0:DH],
                          in_=ps.rearrange("p (h d) -> p h d", d=DH))
                      blk = half * 4 + tc_
                      if blk in stage_sched:
                          stage_sched[blk]()

              # ---------------- attention heads ----------------
              # Scores chunk c covers keys [cP,(c+1)P) x queries [cP,SEG)
              # (block-triangular). Chunks 1 and 3 share one PSUM tile /
              # exp pass: c1 at free 0:384 (queries 128:512), c3 at free
              # 384:512 (queries 384:512). The P@V stationary has a ones
              # column, so PSUM row DH is the softmax denominator.
              # The divide chain for head h is emitted during head h+1
              # (software pipelining) so its serial ACT/DVE latency never
              # head-of-line-blocks the engine FIFOs.
              def divide(h, po):
                  ko_h, hf = h // 2, h % 2
                  qr = DH * hf
                  drow = rot.tile([1, SEG], f32, tag="dr")
                  nc.scalar.copy(drow, po[DH:DH + 1, :])
                  rvf = rot.tile([1, SEG], f32, tag="rv")
                  nc.vector.reciprocal_approx_fast(out=rvf, in_=drow)
                  rvb = rot.tile([1, SEG], bf16, tag="rvb")
                  nc.vector.tensor_copy(out=rvb, in_=rvf)
                  bcp = pso_t()  # broadcast 1/denom over DH partitions
                  mm(bcp[0:DH, :], ones1xPb[0:1, 0:DH], rvb,
                     start=True, stop=True, skip_group_check=True)
                  bcs = epool.tile([DH, SEG], bf16, tag="bcs")
                  nc.vector.tensor_copy(out=bcs, in_=bcp[0:DH, :])
                  nc.vector.tensor_mul(oA[qr:qr + DH, ko_h, :],
                                       po[0:DH, :], bcs)

              pend_h = None
              for h in range(HEADS):
                  if h % 2 == 0:
                      stage_kx_j(h // 2)
                  if h == 0:
                      stage_vx_half(0)
                  if h == 6:
                      stage_vx_half(1)
                  ko_h, hf = h // 2, h % 2
                  qr = DH * hf
                  q_h = qT[qr:qr + DH, ko_h, :]
                  k_h = kT[qr:qr + DH, ko_h, :]
                  # prefix scores [32, 512]
                  scp = psc_t()
                  mm(scp[0:PFX, :], kP[qr:qr + DH, h, :], q_h,
                     start=True, stop=True, skip_group_check=True)
                  eP = epool.tile([PFX, SEG], bf16, tag="eP")
                  nc.scalar.activation(eP, scp[0:PFX, :], AF.Exp,
                                       bias=zeroc[0:PFX])
                  sc0 = psc_t()
                  mm(sc0[:, :], k_h[:, 0:P], q_h,
                     start=True, stop=True, skip_group_check=True)
                  e0 = epool.tile([P, SEG], bf16, tag="e0")
                  nc.scalar.activation(e0, sc0, AF.Exp, bias=zeroc)
                  nc.vector.tensor_mul(e0[:, 0:P], e0[:, 0:P], maskD)
                  sc13 = psc_t()
                  mm(sc13[:, 0:384], k_h[:, P:2 * P], q_h[:, P:],
                     start=True, stop=True, skip_group_check=True)
                  mm(sc13[:, 384:512], k_h[:, 3 * P:4 * P], q_h[:, 3 * P:],
                     start=True, stop=True, skip_group_check=True)
                  eB = epool.tile([P, SEG], bf16, tag="eB")
                  nc.scalar.activation(eB, sc13, AF.Exp, bias=zeroc)
                  nc.vector.tensor_mul(eB[:, 0:P], eB[:, 0:P], maskD)
                  nc.vector.tensor_mul(eB[:, 384:512], eB[:, 384:512], maskD)
                  sc2 = psc_t()
                  mm(sc2[:, 0:256], k_h[:, 2 * P:3 * P], q_h[:, 2 * P:],
                     start=True, stop=True, skip_group_check=True)
                  e2 = epool.tile([P, 256], bf16, tag="e2")
                  nc.scalar.activation(e2, sc2[:, 0:256], AF.Exp, bias=zeroc)
                  nc.vector.tensor_mul(e2[:, 0:P], e2[:, 0:P], maskD)
                  # P@V with ones-column: row DH = softmax denominator
                  po = pa_t()  # 4-buf pool so heads pipeline
                  mm(po[0:DH + 1, :], vP[:, h, :], eP,
                     start=True, stop=False, skip_group_check=True)
                  mm(po[0:DH + 1, 0:], vA[:, 0, h, :], e0,
                     start=False, stop=False, skip_group_check=True)
                  mm(po[0:DH + 1, P:], vA[:, 1, h, :], eB[:, 0:384],
                     start=False, stop=False, skip_group_check=True)
                  mm(po[0:DH + 1, 2 * P:], vA[:, 2, h, :], e2,
                     start=False, stop=False, skip_group_check=True)
                  mm(po[0:DH + 1, 3 * P:], vA[:, 3, h, :], eB[:, 384:512],
                     start=False, stop=True, skip_group_check=True)
                  if pend_h is not None:
                      divide(*pend_h)
                  pend_h = (h, po)
              divide(*pend_h)

              # ---------------- output projection + residual ----------------
              # FFN rmsnorm statistics are accumulated in the same loop so
              # the second norm adds no serial latency.
              ss2 = psc_t()
              sq8b = persist.tile([P, KO, SEG], bf16, tag="big16", name="sq8b")
              for m in range(KO):
                  wt = wpool.tile([P, KO, P], bf16, tag="w8")
                  dma(out=wt, in_=outw_d[m])
                  ps = pa_t()
                  for k in range(KO):
                      mm(ps, wt[:, k], oA[:, k, :],
                         start=(k == 0), stop=(k == KO - 1))
                  nc.vector.tensor_add(xT[:, m, :], ps, xT[:, m, :])  # x1
                  nc.vector.tensor_mul(sq8b[:, m, :], xT[:, m, :], xT[:, m, :])
                  mm(ss2[0:1, :], ones128b, sq8b[:, m, :],
                     start=(m == 0), stop=(m == KO - 1))

              # ---------------- FFN ----------------
              nc.scalar.activation(rrow[:, 0:SEG], ss2[0:1, :], AF.Sqrt,
                                   bias=epsc[0:1], scale=1.0 / DIM)
              nc.vector.reciprocal_approx_fast(out=rrow[:, SEG:2 * SEG],
                                               in_=rrow[:, 0:SEG])
              nc.vector.tensor_copy(out=rrowb, in_=rrow[:, SEG:2 * SEG])
              bc2 = pso_t()
              mm(bc2, ones1xPb, rrowb, start=True, stop=True)
              for ko in range(KO):
                  nc.vector.scalar_tensor_tensor(
                      out=xnT[:, ko, :], in0=xT[:, ko, :],
                      scalar=fnw[:, ko:ko + 1], in1=bc2,
                      op0=OP.mult, op1=OP.mult)
              u_parts = [qT, kT]  # reuse dead slots as u storage
              u_c = persist.tile([P, 6, SEG], bf16, tag="big16")

              def u_slice(k):
                  if k < 8:
                      return u_parts[0][:, k, :]
                  if k < 16:
                      return u_parts[1][:, k - 8, :]
                  return u_c[:, k - 16, :]

              for m in range(MFF):
                  wa = wpool.tile([P, KO, P], bf16, tag="w8")
                  dma(out=wa, in_=w1a_d[m])
                  wg = wpool.tile([P, KO, P], bf16, tag="w8")
                  dma(out=wg, in_=w1g_d[m])
                  psa = pa_t()
                  psg = pa_t()
                  for ko in range(KO):
                      mm(psa, wa[:, ko], xnT[:, ko, :],
                         start=(ko == 0), stop=(ko == KO - 1))
                      mm(psg, wg[:, ko], xnT[:, ko, :],
                         start=(ko == 0), stop=(ko == KO - 1))
                  sig = rot.tile([P, SEG], f32, tag="ropeA")
                  nc.scalar.activation(sig, psg, AF.Sigmoid,
                                       bias=b1g[:, m:m + 1], scale=1.0)
                  silu = rot.tile([P, SEG], f32, tag="ropeB")
                  nc.vector.scalar_tensor_tensor(
                      out=silu, in0=psg, scalar=b1g[:, m:m + 1],
                      in1=sig, op0=OP.add, op1=OP.mult)
                  nc.vector.scalar_tensor_tensor(
                      out=u_slice(m), in0=psa, scalar=b1a[:, m:m + 1],
                      in1=silu, op0=OP.add, op1=OP.mult)

              for o in range(KO):
                  ps = pa_t()
                  for half in range(2):
                      w2t = w2pool.tile([P, 11, P], bf16, tag="w2")
                      dma(out=w2t, in_=w2_d[o][:, half * 11:(half + 1) * 11, :])
                      for k2 in range(11):
                          k = half * 11 + k2
                          mm(ps, w2t[:, k2], u_slice(k),
                             start=(k == 0), stop=(k == MFF - 1))
                  outT = persist.tile([P, KO, SEG], f32, tag="outT",
                                      name=f"outT{o}")
                  nc.vector.scalar_tensor_tensor(
                      out=outT[:, o, :], in0=ps, scalar=b2[:, o:o + 1],
                      in1=xT[:, o, :], op0=OP.add, op1=OP.add)
                  dma(out=yT_d[o * P:(o + 1) * P, :], in_=outT[:, o, :])

    for _rep in range(reps):
        _emit(nc)
    nc.compile()
    return nc


# ======================= host-side preparation =======================

def _prep_shared(inputs):
    import ml_dtypes
    f32 = np.float32
    bf16 = ml_dtypes.bfloat16
    qkv = np.asarray(inputs["to_qkv_w"], f32)
    shared = {}
    shared["qkw"] = np.ascontiguousarray(
        qkv[:, :2048].reshape(KO, P, 16, P).transpose(2, 1, 0, 3)).astype(bf16)
    shared["kvw"] = np.ascontiguousarray(
        np.stack([qkv[:, 1024:2048], qkv[:, 2048:3072]])
        .reshape(2, KO, P, DIM)).astype(bf16)
    shared["outw"] = np.ascontiguousarray(
        np.asarray(inputs["to_out_w"], f32)
        .reshape(KO, P, KO, P).transpose(2, 1, 0, 3)).astype(bf16)
    w1 = np.asarray(inputs["ff_w1"], f32)
    w1a = np.zeros((DIM, DFFP), f32)
    w1g = np.zeros((DIM, DFFP), f32)
    w1a[:, :DFF] = w1[:, :DFF]
    w1g[:, :DFF] = w1[:, DFF:]
    shared["w1a"] = np.ascontiguousarray(
        w1a.reshape(KO, P, MFF, P).transpose(2, 1, 0, 3)).astype(bf16)
    shared["w1g"] = np.ascontiguousarray(
        w1g.reshape(KO, P, MFF, P).transpose(2, 1, 0, 3)).astype(bf16)
    w2 = np.zeros((DFFP, DIM), f32)
    w2[:DFF] = np.asarray(inputs["ff_w2"], f32)
    shared["w2"] = np.ascontiguousarray(
        w2.reshape(MFF, P, KO, P).transpose(2, 1, 0, 3)).astype(bf16)
    shared["mtw"] = np.ascontiguousarray(
        np.asarray(inputs["to_mem_tokens_w"], f32).reshape(KO, P, DIM)).astype(bf16)
    pm = np.asarray(inputs["persist_mem"], f32)
    shared["pmv"] = np.ascontiguousarray(pm).astype(bf16)
    shared["pmk"] = np.ascontiguousarray(pm.transpose(0, 2, 1)).astype(bf16)
    b1 = np.asarray(inputs["ff_b1"], f32)
    b1a = np.zeros(DFFP, f32)
    b1g = np.zeros(DFFP, f32)
    b1a[:DFF] = b1[:DFF]
    b1g[:DFF] = b1[DFF:]
    shared["b1a"] = np.ascontiguousarray(b1a.reshape(MFF, P).T)
    shared["b1g"] = np.ascontiguousarray(b1g.reshape(MFF, P).T)
    shared["b2"] = np.ascontiguousarray(
        np.asarray(inputs["ff_b2"], f32).reshape(KO, P).T)
    shared["anw"] = np.ascontiguousarray(
        np.asarray(inputs["attn_norm_w"], f32).reshape(KO, P).T)
    shared["fnw"] = np.ascontiguousarray(
        np.asarray(inputs["ff_norm_w"], f32).reshape(KO, P).T)
    shared["mpnw"] = np.ascontiguousarray(
        np.asarray(inputs["mem_pool_norm_w"], f32).reshape(1, DIM))
    rl = np.zeros((P, P), f32)
    ii = np.arange(0, P, 2)
    rl[ii + 1, ii] = f32(-1.0)
    rl[ii, ii + 1] = f32(1.0)
    shared["rmat"] = rl.astype(bf16)
    shared["maskD"] = np.where(
        np.arange(P)[None, :] >= np.arange(P)[:, None], f32(1.0), f32(0.0)
    ).astype(bf16)

    # per-batch bf16 mem_out
    mo = np.asarray(inputs["mem_out"], f32)
    shared["_mo"] = [np.ascontiguousarray(mo[b]).astype(bf16) for b in range(B)]

    # rope tables, float32 math to match the reference
    pos = np.arange(N, dtype=f32)
    expo = (np.arange(0, DH, 2).astype(f32) / f32(DH)).astype(f32)
    inv = (f32(1.0) / np.power(f32(10000.0), expo)).astype(f32)
    ang = np.repeat(pos[:, None] * inv[None, :], 2, axis=1).astype(f32)
    cosf, sinf = np.cos(ang).astype(f32), np.sin(ang).astype(f32)
    scale = f32(DH ** -0.5)
    shared["_cos"], shared["_sin"], shared["_scale"] = cosf, sinf, scale
    return shared


def _prep_core(inputs, shared, b, s):
    f32 = np.float32
    x = np.asarray(inputs["x"], f32)
    cosf, sinf, scale = shared["_cos"], shared["_sin"], shared["_scale"]
    seg = slice(s * SEG, (s + 1) * SEG)
    ct = np.ascontiguousarray(np.tile(cosf[seg].T, (2, 1)))
    st = np.ascontiguousarray(np.tile(sinf[seg].T, (2, 1)))
    m = {k: v for k, v in shared.items() if not k.startswith("_")}
    m["xT"] = np.ascontiguousarray(x[b, seg].T)
    m["mo"] = shared["_mo"][b]
    m["cq"] = (ct * scale).astype(f32)
    m["sq"] = (st * scale).astype(f32)
    m["ck"] = ct
    m["sk"] = st
    return m


def _get_nc():
    if "nc" not in _CACHE:
        _CACHE["nc"] = build_nc()
    return _CACHE["nc"]


def kernel(**inputs) -> np.ndarray:
    nc = _get_nc()
    shared = _prep_shared(inputs)
    cores = [(b, s) for b in range(B) for s in range(4)]
    in_maps = [_prep_core(inputs, shared, b, s) for b, s in cores]
    from concourse import bass_utils
    import os
    res = bass_utils.run_bass_kernel_spmd(
        nc, in_maps, core_ids=list(range(NCORES)),
        trace=bool(os.environ.get("MAC_TRACE")))
    _CACHE["last_results"] = res
    out = np.empty((B, N, DIM), np.float32)
    for i, (b, s) in enumerate(cores):
        out[b, s * SEG:(s + 1) * SEG, :] = res.results[i]["yT"].T
    return out


# revision 62
# speedup vs baseline: 1.7204x; 1.0625x over previous
"""Trainium2 Bass kernel for nn_MACBlock (segmented attention + GEGLU FFN).

Sharding: 8 cores = 2 batches x 4 segments of 512 queries. The segment mask
makes attention block-diagonal (plus a 32-token always-visible prefix derived
from pooled memory + persistent memory), so each core is fully independent:
no collectives.

Layout: activations are kept feature-major (x^T [dim, tokens]) on-chip, so
every matmul contraction dim lands on partitions with zero transposes.
All heavy GEMMs run in bf16 (weights pre-cast on host, activations cast
on-chip): bf16 enables Fast Weight Load and avoids the fp32-HIGH power
throttle that halves the PE clock. PSUM accumulation stays fp32.
Scores are computed key-major ([keys, queries]); softmax is max-free; the
softmax denominator comes from a ones-column folded into the P@V stationary
operand (row DH of the same PSUM tile).
"""

import sys

if "/opt/trn_rl_repo" not in sys.path:
    sys.path.insert(0, "/opt/trn_rl_repo")

import numpy as np

B, N, DIM = 2, 2048, 1024
HEADS, DH = 16, 64
SEG = 512
NPM = NM = 16
PFX = NPM + NM          # 32 prefix keys
DFF = 2730
MFF = 22                # padded dff chunks
DFFP = MFF * 128        # 2816
KO = 8                  # 1024 / 128
P = 128
NCORES = 8
EPS = 1.1920929e-07

_CACHE = {}


def _f32r(ap):
    import concourse.mybir as mybir
    return ap.bitcast(mybir.dt.float32r)


def build_nc(reps=1):
    import concourse.bass as bass
    from concourse import bacc
    import concourse.tile as tile
    import concourse.mybir as mybir

    f32 = mybir.dt.float32
    bf16 = mybir.dt.bfloat16
    AF = mybir.ActivationFunctionType
    OP = mybir.AluOpType
    AX = mybir.AxisListType

    nc = bacc.Bacc("TRN2", target_bir_lowering=False, debug=False)

    dp = nc.declare_dram_parameter
    xT_d = dp("xT", [DIM, SEG], f32, isOutput=False)
    mo_d = dp("mo", [N, DIM], bf16, isOutput=False)
    cq_d = dp("cq", [P, SEG], f32, isOutput=False)
    sq_d = dp("sq", [P, SEG], f32, isOutput=False)
    ck_d = dp("ck", [P, SEG], f32, isOutput=False)
    sk_d = dp("sk", [P, SEG], f32, isOutput=False)
    mask_d = dp("maskD", [P, P], bf16, isOutput=False)
    rmat_d = dp("rmat", [P, P], bf16, isOutput=False)
    qkw_d = dp("qkw", [16, P, KO, P], bf16, isOutput=False)
    kvw_d = dp("kvw", [2, KO, P, DIM], bf16, isOutput=False)
    outw_d = dp("outw", [KO, P, KO, P], bf16, isOutput=False)
    w1a_d = dp("w1a", [MFF, P, KO, P], bf16, isOutput=False)
    w1g_d = dp("w1g", [MFF, P, KO, P], bf16, isOutput=False)
    w2_d = dp("w2", [KO, P, MFF, P], bf16, isOutput=False)
    mtw_d = dp("mtw", [KO, P, DIM], bf16, isOutput=False)
    pmv_d = dp("pmv", [HEADS, NPM, DH], bf16, isOutput=False)
    pmk_d = dp("pmk", [HEADS, DH, NPM], bf16, isOutput=False)
    b1a_d = dp("b1a", [P, MFF], f32, isOutput=False)
    b1g_d = dp("b1g", [P, MFF], f32, isOutput=False)
    b2_d = dp("b2", [P, KO], f32, isOutput=False)
    anw_d = dp("anw", [P, KO], f32, isOutput=False)
    fnw_d = dp("fnw", [P, KO], f32, isOutput=False)
    mpnw_d = dp("mpnw", [1, DIM], f32, isOutput=False)
    yT_d = dp("yT", [DIM, SEG], f32, isOutput=True)

    def _emit(nc):
      with tile.TileContext(nc) as tc, \
            nc.allow_low_precision(reason="bf16 matmul rounding"):
        from contextlib import ExitStack
        ctx = ExitStack()
        with ctx:
            persist = ctx.enter_context(tc.tile_pool(name="persist", bufs=1))
            wpool = ctx.enter_context(tc.tile_pool(name="wpool", bufs=5))
            kvpool = ctx.enter_context(tc.tile_pool(name="kvpool", bufs=6))
            w2pool = ctx.enter_context(tc.tile_pool(name="w2pool", bufs=3))
            mopool = ctx.enter_context(tc.tile_pool(name="mopool", bufs=4))
            rot = ctx.enter_context(tc.tile_pool(name="rot", bufs=2))
            epool = ctx.enter_context(tc.tile_pool(name="epool", bufs=3))
            pa = ctx.enter_context(tc.tile_pool(name="pa", bufs=3, space="PSUM"))
            psc = ctx.enter_context(tc.tile_pool(name="psc", bufs=2, space="PSUM"))
            ppo = ctx.enter_context(tc.tile_pool(name="ppo", bufs=3, space="PSUM"))

            cnt = [0]

            def pa_t():
                cnt[0] += 1
                return pa.tile([P, SEG], f32, tag="ps", name=f"pa{cnt[0]}")

            def psc_t():
                cnt[0] += 1
                return psc.tile([P, SEG], f32, tag="sc", name=f"sc{cnt[0]}")

            def ppo_t():
                cnt[0] += 1
                return ppo.tile([P, SEG], f32, tag="o", name=f"o{cnt[0]}")

            # ---------------- persistent SBUF tensors ----------------
            xT = persist.tile([P, KO, SEG], f32, tag="xT")       # x^T, later x1^T
            xnT = persist.tile([P, KO, SEG], bf16, tag="xnT")    # xn^T, later xn1^T
            kT = persist.tile([P, KO, SEG], bf16, tag="kT")      # roped k^T
            vA = persist.tile([P, 4, HEADS, DH + 1], bf16, tag="vA")  # v key-major
            vP = persist.tile([PFX, HEADS, DH + 1], bf16, tag="vP")   # prefix v rows
            kP = persist.tile([P, HEADS, PFX], bf16, tag="kP")   # prefix k^T @64*(h%2)
            oA = persist.tile([P, KO, SEG], bf16, tag="oA")      # attn o^T
            cq = persist.tile([P, SEG], f32, tag="cq")
            sq_ = persist.tile([P, SEG], f32, tag="sq")
            ck = persist.tile([P, SEG], f32, tag="ck")
            sk = persist.tile([P, SEG], f32, tag="sk")
            maskD = persist.tile([P, P], bf16, tag="maskD")
            rmat = persist.tile([P, P], bf16, tag="rmat")
            b1a = persist.tile([P, MFF], f32, tag="b1a")
            b1g = persist.tile([P, MFF], f32, tag="b1g")
            b2 = persist.tile([P, KO], f32, tag="b2")
            anw = persist.tile([P, KO], f32, tag="anw")
            fnw = persist.tile([P, KO], f32, tag="fnw")
            mpnw = persist.tile([1, DIM], f32, tag="mpnw")
            ones16 = persist.tile([1, 16], bf16, tag="o16")
            ones11 = persist.tile([1, 1], bf16, tag="o11")
            onesHH = persist.tile([DH, P], bf16, tag="oHH")  # pair bcast lhsT
            ones128b = persist.tile([P, 1], bf16, tag="o128b")   # lhsT K=128,M=1
            ones1xPb = persist.tile([1, P], bf16, tag="o1xPb")   # lhsT K=1,M=128
            ones1xP = persist.tile([1, P], f32, tag="o1xP")      # f32 variant
            pooledT = persist.tile([P, KO], bf16, tag="pooledT")
            memtokT = persist.tile([P, KO], bf16, tag="memtokT")
            mrow = persist.tile([1, 3 * DIM], f32, tag="mrow")
            mrowb = persist.tile([1, 3 * DIM], bf16, tag="mrowb")
            rrow = persist.tile([1, 2 * SEG], f32, tag="rrow")
            rrowb = persist.tile([1, SEG], bf16, tag="rrowb")
            epsc = persist.tile([P, 1], f32, tag="epsc")
            zeroc = persist.tile([P, 1], f32, tag="zeroc")

            dma = nc.sync.dma_start
            dma(out=cq, in_=cq_d[:])
            dma(out=sq_, in_=sq_d[:])
            dma(out=ck, in_=ck_d[:])
            dma(out=sk, in_=sk_d[:])
            dma(out=maskD, in_=mask_d[:])
            dma(out=rmat, in_=rmat_d[:])
            dma(out=b1a, in_=b1a_d[:])
            dma(out=b1g, in_=b1g_d[:])
            dma(out=b2, in_=b2_d[:])
            dma(out=anw, in_=anw_d[:])
            dma(out=fnw, in_=fnw_d[:])
            dma(out=mpnw, in_=mpnw_d[:])
            nc.vector.memset(ones16, 1.0)
            nc.vector.memset(ones11, 1.0)
            # pair-broadcast stationary: row 0 -> out rows 0-63, row 32 ->
            # out rows 64-127; all other contraction rows are zero, and the
            # matching moving-tile rows are zero-primed below, so they
            # contribute exactly 0 to the K=64 contraction.
            nc.vector.memset(onesHH, 0.0)
            nc.vector.memset(onesHH[0:1, 0:DH], 1.0)
            nc.vector.memset(onesHH[32:33, DH:P], 1.0)
            # zero-prime both rot-pool buffers of the pair-reciprocal moving
            # tile: its rows other than 0/32 are never written afterwards
            for _ in range(2):
                _rz = rot.tile([DH, SEG], bf16, tag="rvb")
                nc.vector.memset(_rz, 0.0)
            nc.vector.memset(ones128b, 1.0)
            nc.vector.memset(ones1xPb, 1.0)
            nc.vector.memset(ones1xP, 1.0)
            nc.vector.memset(epsc, EPS)
            nc.vector.memset(zeroc, 0.0)
            # denominator ones-columns in the P@V stationary operands
            nc.vector.memset(vA[:, :, :, DH:DH + 1], 1.0)
            nc.vector.memset(vP[:, :, DH:DH + 1], 1.0)
            for h in range(HEADS):
                hb = DH * (h % 2)
                dma(out=kP[hb:hb + DH, h, NPM:PFX], in_=pmk_d[h])
                dma(out=vP[NPM:PFX, h, 0:DH], in_=pmv_d[h])

            if True:
              for ko in range(KO):
                  dma(out=xT[:, ko, :], in_=xT_d[ko * P:(ko + 1) * P, :])

              mm = nc.tensor.matmul

              def rmsnorm_into(dst, src, w_sb, sq_tag):
                  """dst[:,ko,:] = src[:,ko,:] * w[:,ko] * rsqrt(mean_dim(src^2)+eps)"""
                  ss = psc_t()  # [1,512] slice used
                  sq8 = persist.tile([P, KO, SEG], bf16, tag=sq_tag, name="sq8")
                  for ko in range(KO):
                      nc.vector.tensor_mul(sq8[:, ko, :], src[:, ko, :],
                                           src[:, ko, :])
                      mm(ss[0:1, :], ones128b, sq8[:, ko, :],
                         start=(ko == 0), stop=(ko == KO - 1))
                  nc.scalar.activation(rrow[:, 0:SEG], ss[0:1, :], AF.Sqrt,
                                       bias=epsc[0:1], scale=1.0 / DIM)
                  nc.vector.reciprocal_approx_fast(out=rrow[:, SEG:2 * SEG],
                                                   in_=rrow[:, 0:SEG])
                  nc.vector.tensor_copy(out=rrowb, in_=rrow[:, SEG:2 * SEG])
                  bc = psc_t()  # broadcast rstd over 128 partitions
                  mm(bc, ones1xPb, rrowb,
                     start=True, stop=True)
                  for ko in range(KO):
                      nc.vector.scalar_tensor_tensor(
                          out=dst[:, ko, :], in0=src[:, ko, :],
                          scalar=w_sb[:, ko:ko + 1], in1=bc,
                          op0=OP.mult, op1=OP.mult)

              # ---------------- attn rmsnorm ----------------
              rmsnorm_into(xnT, xT, anw, "big16")
              qT = persist.tile([P, KO, SEG], bf16, tag="qT")    # roped,scaled q^T

              # ---------------- q/k projections + rope, interleaved with
              # ---------------- mem_out mean accumulation ----------------
              # [1,512] accumulator pairs live at partition rows 0 and 32 of
              # a single PSUM tile (matmul out base partition must be 0/32/64)
              mean_ps = psc_t()

              def mo_mean_step(t):
                  mot = mopool.tile([P, DIM], bf16, tag="mo", name="mot")
                  dma(out=mot, in_=mo_d[t * P:(t + 1) * P, :])
                  for half in range(2):
                      r = 32 * half
                      mm(mean_ps[r:r + 1, :], ones128b,
                         mot[:, half * SEG:(half + 1) * SEG],
                         start=(t == 0), stop=(t == 15))

              # pooled rmsnorm (pure ACT/DVE): emitted mid-qk-loop so its
              # serial latency hides under the remaining projection matmuls
              pooled_raw = mrow[:, 0:DIM]
              sqr = mrow[:, DIM:2 * DIM]
              pooled = mrowb[:, 2 * DIM:3 * DIM]  # bf16 row for transposes

              def pooled_chain():
                  for half in range(2):
                      r = 32 * half
                      nc.scalar.activation(_f32r(pooled_raw[:, half * SEG:(half + 1) * SEG]),
                                           mean_ps[r:r + 1, :], AF.Copy,
                                           scale=1.0 / N)
                  nc.vector.tensor_mul(_f32r(sqr), pooled_raw, pooled_raw)
                  nc.vector.reduce_sum(_f32r(sqr[:, 0:1]), sqr, axis=AX.X)
                  nc.scalar.activation(_f32r(sqr[:, 1:2]), sqr[:, 0:1], AF.Sqrt,
                                       bias=epsc[0:1], scale=1.0 / DIM)
                  nc.vector.reciprocal(_f32r(sqr[:, 2:3]), sqr[:, 1:2])
                  nc.vector.scalar_tensor_tensor(out=pooled, in0=pooled_raw,
                                                 scalar=sqr[:, 2:3], in1=mpnw,
                                                 op0=OP.mult, op1=OP.mult)

              # software-pipelined: the rope finish (rmat matmul + DVE
              # combine) for iteration m is emitted during iteration m+1 so
              # the ACT qraw copy never stalls the PE FIFO
              def rope_finish(ps, qraw, m):
                  is_q = m < 8
                  c_t, s_t = (cq, sq_) if is_q else (ck, sk)
                  dst = qT if is_q else kT
                  At = rot.tile([P, SEG], bf16, tag="ropeB")
                  nc.vector.tensor_mul(At, ps, c_t)  # before rps: frees ps
                  rps = pa_t()
                  mm(rps, rmat, qraw, start=True, stop=True)
                  Bt = rot.tile([P, SEG], bf16, tag="ropeA")
                  nc.vector.tensor_mul(Bt, rps, s_t)
                  nc.vector.tensor_add(dst[:, m % 8, :], At, Bt)

              pend = None
              for m in range(16):
                  wt = wpool.tile([P, KO, P], bf16, tag="w8")
                  dma(out=wt, in_=qkw_d[m])
                  ps = pa_t()
                  for ko in range(KO):
                      mm(ps, wt[:, ko], xnT[:, ko, :],
                         start=(ko == 0), stop=(ko == KO - 1))
                  qraw = rot.tile([P, SEG], bf16, tag="qraw")
                  nc.scalar.copy(qraw, ps)
                  if pend is not None:
                      rope_finish(*pend)
                  pend = (ps, qraw, m)
                  if 1 <= m < 9:
                      mo_mean_step(2 * (m - 1))
                      mo_mean_step(2 * (m - 1) + 1)
                  if m == 10:
                      pooled_chain()
              rope_finish(*pend)

              def stage_pT():
                  pT = pa_t()
                  for ko in range(KO):
                      mm(pT[:, ko:ko + 1], pooled[0:1, ko * P:(ko + 1) * P],
                         ones11, start=True, stop=True, skip_group_check=True)
                  nc.vector.tensor_copy(out=pooledT, in_=pT[:, 0:KO])

              kvvs = []  # v-proj weights, hoisted so DMAs overlap qk tail

              def load_kvv():
                  for half in range(2):
                      kvv = persist.tile([P, KO, SEG], bf16,
                                         tag=("kvv0" if half == 0 else "big16"))
                      for ko in range(KO):
                          dma(out=kvv[:, ko, :],
                              in_=kvw_d[1, ko, :, half * SEG:(half + 1) * SEG])
                      kvvs.append(kvv)

              mt_ps = []

              def stage_mt():
                  mt_ps.append(psc_t())
                  for ko in range(KO):
                      mtw_t = kvpool.tile([P, DIM], bf16, tag="kv")
                      dma(out=mtw_t, in_=mtw_d[ko])
                      for half in range(2):
                          r = 32 * half
                          mm(mt_ps[0][r:r + 1, :], pooledT[:, ko:ko + 1],
                             mtw_t[:, half * SEG:(half + 1) * SEG],
                             start=(ko == 0), stop=(ko == KO - 1))

              memtok = mrowb[:, 0:DIM]

              def stage_mT():
                  for half in range(2):
                      r = 32 * half
                      nc.scalar.activation(memtok[:, half * SEG:(half + 1) * SEG],
                                           mt_ps[0][r:r + 1, :], AF.Copy)
                  mT = pa_t()
                  for ko in range(KO):
                      mm(mT[:, ko:ko + 1], memtok[0:1, ko * P:(ko + 1) * P],
                         ones11, start=True, stop=True, skip_group_check=True)
                  nc.vector.tensor_copy(out=memtokT, in_=mT[:, 0:KO])

              kc_row = mrowb[:, DIM:2 * DIM]
              vc_row = mrowb[:, 2 * DIM:3 * DIM]

              def stage_kcvc(c):
                  r_ps = psc_t()
                  for ko in range(KO):
                      kv_t = kvpool.tile([P, DIM], bf16, tag="kv")
                      dma(out=kv_t, in_=kvw_d[c, ko])
                      for half in range(2):
                          r = 32 * half
                          mm(r_ps[r:r + 1, :], memtokT[:, ko:ko + 1],
                             kv_t[:, half * SEG:(half + 1) * SEG],
                             start=(ko == 0), stop=(ko == KO - 1))
                  row = kc_row if c == 0 else vc_row
                  for half in range(2):
                      r = 32 * half
                      nc.scalar.activation(row[:, half * SEG:(half + 1) * SEG],
                                           r_ps[r:r + 1, :], AF.Copy)

              def stage_kx_j(j):  # 2 heads per chunk
                  kx = pa_t()
                  mm(kx[:, 0:16], kc_row[0:1, j * P:(j + 1) * P],
                     ones16, start=True, stop=True,
                     skip_group_check=True)
                  nc.vector.tensor_copy(out=kP[0:DH, 2 * j, 0:NPM],
                                        in_=kx[0:DH, 0:16])
                  nc.vector.tensor_copy(out=kP[DH:P, 2 * j + 1, 0:NPM],
                                        in_=kx[DH:P, 0:16])

              def stage_vx_half(half):
                  vx = pa_t()
                  mm(vx[0:16, :], ones16,
                     vc_row[0:1, half * SEG:(half + 1) * SEG],
                     start=True, stop=True, skip_group_check=True)
                  nc.vector.tensor_copy(
                      out=vP[0:NPM, half * 8:(half + 1) * 8, 0:DH],
                      in_=vx[0:16, :].rearrange("p (h d) -> p h d", d=DH))

              load_kvv()  # all v-weight DMAs issued up front
              stage_pT()  # pooled ready ~3 qk-iterations ago
              stage_sched = {1: stage_mt, 3: stage_mT,
                             5: lambda: stage_kcvc(0),
                             6: lambda: stage_kcvc(1)}

              # ---------------- v projection (token-major), interleaved ----
              # (kx/vx stages are woven into the attention head loop below)
              for half in range(2):
                  kvv = kvvs[half]
                  for tc_ in range(4):
                      ps = pa_t()
                      for ko in range(KO):
                          mm(ps, xnT[:, ko, tc_ * P:(tc_ + 1) * P],
                             kvv[:, ko, :],
                             start=(ko == 0), stop=(ko == KO - 1))
                      nc.vector.tensor_copy(
                          out=vA[:, tc_, half * 8:(half + 1) * 8, 0:DH],
                          in_=ps.rearrange("p (h d) -> p h d", d=DH))
                      blk = half * 4 + tc_
                      if blk in stage_sched:
                          stage_sched[blk]()

              # ---------------- attention heads ----------------
              # Scores chunk c covers keys [cP,(c+1)P) x queries [cP,SEG)
              # (block-triangular). Chunks 1 and 3 share one PSUM tile /
              # exp pass: c1 at free 0:384 (queries 128:512), c3 at free
              # 384:512 (queries 384:512). The P@V stationary has a ones
              # column, so PSUM row DH is the softmax denominator.
              # The divide chain for a head PAIR (2h, 2h+1) is emitted two
              # heads later (software pipelining), with one reciprocal /
              # broadcast / copy serving both heads, so its serial ACT/DVE
              # latency never head-of-line-blocks the engine FIFOs.
              def divide_pair(h2, po_a, po_b):
                  ko_h = h2 // 2
                  drow2 = rot.tile([DH, SEG], f32, tag="dr")
                  nc.scalar.copy(drow2[0:1, :], po_a[DH:DH + 1, :])
                  nc.vector.tensor_copy(out=drow2[32:33, :],
                                        in_=po_b[DH:DH + 1, :])
                  rvf2 = rot.tile([DH, SEG], f32, tag="rv")
                  nc.vector.reciprocal_approx_fast(out=rvf2[0:33, :],
                                                   in_=drow2[0:33, :])
                  rvb2 = rot.tile([DH, SEG], bf16, tag="rvb")
                  nc.vector.tensor_copy(out=rvb2[0:1, :], in_=rvf2[0:1, :])
                  nc.vector.tensor_copy(out=rvb2[32:33, :], in_=rvf2[32:33, :])
                  bcp2 = psc_t()  # rows 0-63 = 1/d_a, rows 64-127 = 1/d_b
                  mm(bcp2, onesHH, rvb2,
                     start=True, stop=True, skip_group_check=True)
                  bcs2 = epool.tile([P, SEG], bf16, tag="bcs")
                  nc.vector.tensor_copy(out=bcs2, in_=bcp2)
                  nc.vector.tensor_mul(oA[0:DH, ko_h, :],
                                       po_a[0:DH, :], bcs2[0:DH, :])
                  nc.vector.tensor_mul(oA[DH:P, ko_h, :],
                                       po_b[0:DH, :], bcs2[DH:P, :])

              pend_pair = None
              po_prev = None
              for h in range(HEADS):
                  if h % 2 == 1 and pend_pair is not None:
                      # emitted BEFORE this head's po allocation so the
                      # 3-buf ppo rotation never recycles an unread tile
                      divide_pair(*pend_pair)
                      pend_pair = None
                  if h % 2 == 0:
                      stage_kx_j(h // 2)
                  if h == 0:
                      stage_vx_half(0)
                  if h == 6:
                      stage_vx_half(1)
                  ko_h, hf = h // 2, h % 2
                  qr = DH * hf
                  q_h = qT[qr:qr + DH, ko_h, :]
                  k_h = kT[qr:qr + DH, ko_h, :]
                  # prefix scores [32, 512]
                  scp = psc_t()
                  mm(scp[0:PFX, :], kP[qr:qr + DH, h, :], q_h,
                     start=True, stop=True, skip_group_check=True)
                  eP = epool.tile([PFX, SEG], bf16, tag="eP")
                  nc.scalar.activation(eP, scp[0:PFX, :], AF.Exp,
                                       bias=zeroc[0:PFX])
                  sc0 = pa_t()  # pa is near-idle during attention
                  mm(sc0[:, :], k_h[:, 0:P], q_h,
                     start=True, stop=True, skip_group_check=True)
                  e0 = epool.tile([P, SEG], bf16, tag="e0")
                  nc.scalar.activation(e0, sc0, AF.Exp, bias=zeroc)
                  nc.vector.tensor_mul(e0[:, 0:P], e0[:, 0:P], maskD)
                  sc13 = psc_t()
                  mm(sc13[:, 0:384], k_h[:, P:2 * P], q_h[:, P:],
                     start=True, stop=True, skip_group_check=True)
                  mm(sc13[:, 384:512], k_h[:, 3 * P:4 * P], q_h[:, 3 * P:],
                     start=True, stop=True, skip_group_check=True)
                  eB = epool.tile([P, SEG], bf16, tag="eB")
                  nc.scalar.activation(eB, sc13, AF.Exp, bias=zeroc)
                  nc.vector.tensor_mul(eB[:, 0:P], eB[:, 0:P], maskD)
                  nc.vector.tensor_mul(eB[:, 384:512], eB[:, 384:512], maskD)
                  sc2 = pa_t()
                  mm(sc2[:, 0:256], k_h[:, 2 * P:3 * P], q_h[:, 2 * P:],
                     start=True, stop=True, skip_group_check=True)
                  e2 = epool.tile([P, 256], bf16, tag="e2")
                  nc.scalar.activation(e2, sc2[:, 0:256], AF.Exp, bias=zeroc)
                  nc.vector.tensor_mul(e2[:, 0:P], e2[:, 0:P], maskD)
                  # P@V with ones-column: row DH = softmax denominator
                  po = ppo_t()  # dedicated pool so heads pipeline
                  mm(po[0:DH + 1, :], vP[:, h, :], eP,
                     start=True, stop=False, skip_group_check=True)
                  mm(po[0:DH + 1, 0:], vA[:, 0, h, :], e0,
                     start=False, stop=False, skip_group_check=True)
                  mm(po[0:DH + 1, P:], vA[:, 1, h, :], eB[:, 0:384],
                     start=False, stop=False, skip_group_check=True)
                  mm(po[0:DH + 1, 2 * P:], vA[:, 2, h, :], e2,
                     start=False, stop=False, skip_group_check=True)
                  mm(po[0:DH + 1, 3 * P:], vA[:, 3, h, :], eB[:, 384:512],
                     start=False, stop=True, skip_group_check=True)
                  if h % 2 == 0:
                      po_prev = po
                  else:
                      pend_pair = (h - 1, po_prev, po)
              divide_pair(*pend_pair)

              # ---------------- output projection + residual ----------------
              # FFN rmsnorm statistics are accumulated in the same loop so
              # the second norm adds no serial latency.
              ss2 = psc_t()
              sq8b = persist.tile([P, KO, SEG], bf16, tag="big16", name="sq8b")
              for m in range(KO):
                  wt = wpool.tile([P, KO, P], bf16, tag="w8")
                  dma(out=wt, in_=outw_d[m])
                  ps = pa_t()
                  for k in range(KO):
                      mm(ps, wt[:, k], oA[:, k, :],
                         start=(k == 0), stop=(k == KO - 1))
                  nc.vector.tensor_add(xT[:, m, :], ps, xT[:, m, :])  # x1
                  nc.vector.tensor_mul(sq8b[:, m, :], xT[:, m, :], xT[:, m, :])
                  mm(ss2[0:1, :], ones128b, sq8b[:, m, :],
                     start=(m == 0), stop=(m == KO - 1))

              # ---------------- FFN ----------------
              nc.scalar.activation(rrow[:, 0:SEG], ss2[0:1, :], AF.Sqrt,
                                   bias=epsc[0:1], scale=1.0 / DIM)
              nc.vector.reciprocal_approx_fast(out=rrow[:, SEG:2 * SEG],
                                               in_=rrow[:, 0:SEG])
              nc.vector.tensor_copy(out=rrowb, in_=rrow[:, SEG:2 * SEG])
              bc2 = psc_t()
              mm(bc2, ones1xPb, rrowb, start=True, stop=True)
              for ko in range(KO):
                  nc.vector.scalar_tensor_tensor(
                      out=xnT[:, ko, :], in0=xT[:, ko, :],
                      scalar=fnw[:, ko:ko + 1], in1=bc2,
                      op0=OP.mult, op1=OP.mult)
              u_parts = [qT, kT]  # reuse dead slots as u storage
              u_c = persist.tile([P, 6, SEG], bf16, tag="big16")

              def u_slice(k):
                  if k < 8:
                      return u_parts[0][:, k, :]
                  if k < 16:
                      return u_parts[1][:, k - 8, :]
                  return u_c[:, k - 16, :]

              for m in range(MFF):
                  wa = wpool.tile([P, KO, P], bf16, tag="w8")
                  dma(out=wa, in_=w1a_d[m])
                  wg = wpool.tile([P, KO, P], bf16, tag="w8")
                  dma(out=wg, in_=w1g_d[m])
                  psa = pa_t()
                  psg = ppo_t()  # ppo idle during FFN; full double-buffering
                  for ko in range(KO):
                      mm(psa, wa[:, ko], xnT[:, ko, :],
                         start=(ko == 0), stop=(ko == KO - 1))
                      mm(psg, wg[:, ko], xnT[:, ko, :],
                         start=(ko == 0), stop=(ko == KO - 1))
                  sig = rot.tile([P, SEG], f32, tag="ropeA")
                  nc.scalar.activation(sig, psg, AF.Sigmoid,
                                       bias=b1g[:, m:m + 1], scale=1.0)
                  silu = rot.tile([P, SEG], f32, tag="ropeB")
                  nc.vector.scalar_tensor_tensor(
                      out=silu, in0=psg, scalar=b1g[:, m:m + 1],
                      in1=sig, op0=OP.add, op1=OP.mult)
                  nc.vector.scalar_tensor_tensor(
                      out=u_slice(m), in0=psa, scalar=b1a[:, m:m + 1],
                      in1=silu, op0=OP.add, op1=OP.mult)

              for o in range(KO):
                  ps = pa_t()
                  for half in range(2):
                      w2t = w2pool.tile([P, 11, P], bf16, tag="w2")
                      dma(out=w2t, in_=w2_d[o][:, half * 11:(half + 1) * 11, :])
                      for k2 in range(11):
                          k = half * 11 + k2
                          mm(ps, w2t[:, k2], u_slice(k),
                             start=(k == 0), stop=(k == MFF - 1))
                  outT = persist.tile([P, KO, SEG], f32, tag="kvv0",
                                      name=f"outT{o}")
                  nc.vector.scalar_tensor_tensor(
                      out=outT[:, o, :], in0=ps, scalar=b2[:, o:o + 1],
                      in1=xT[:, o, :], op0=OP.add, op1=OP.add)
                  dma(out=yT_d[o * P:(o + 1) * P, :], in_=outT[:, o, :])

    for _rep in range(reps):
        _emit(nc)
    nc.compile()
    return nc


# ======================= host-side preparation =======================

def _prep_shared(inputs):
    import ml_dtypes
    f32 = np.float32
    bf16 = ml_dtypes.bfloat16
    qkv = np.asarray(inputs["to_qkv_w"], f32)
    shared = {}
    shared["qkw"] = np.ascontiguousarray(
        qkv[:, :2048].reshape(KO, P, 16, P).transpose(2, 1, 0, 3)).astype(bf16)
    shared["kvw"] = np.ascontiguousarray(
        np.stack([qkv[:, 1024:2048], qkv[:, 2048:3072]])
        .reshape(2, KO, P, DIM)).astype(bf16)
    shared["outw"] = np.ascontiguousarray(
        np.asarray(inputs["to_out_w"], f32)
        .reshape(KO, P, KO, P).transpose(2, 1, 0, 3)).astype(bf16)
    w1 = np.asarray(inputs["ff_w1"], f32)
    w1a = np.zeros((DIM, DFFP), f32)
    w1g = np.zeros((DIM, DFFP), f32)
    w1a[:, :DFF] = w1[:, :DFF]
    w1g[:, :DFF] = w1[:, DFF:]
    shared["w1a"] = np.ascontiguousarray(
        w1a.reshape(KO, P, MFF, P).transpose(2, 1, 0, 3)).astype(bf16)
    shared["w1g"] = np.ascontiguousarray(
        w1g.reshape(KO, P, MFF, P).transpose(2, 1, 0, 3)).astype(bf16)
    w2 = np.zeros((DFFP, DIM), f32)
    w2[:DFF] = np.asarray(inputs["ff_w2"], f32)
    shared["w2"] = np.ascontiguousarray(
        w2.reshape(MFF, P, KO, P).transpose(2, 1, 0, 3)).astype(bf16)
    shared["mtw"] = np.ascontiguousarray(
        np.asarray(inputs["to_mem_tokens_w"], f32).reshape(KO, P, DIM)).astype(bf16)
    pm = np.asarray(inputs["persist_mem"], f32)
    shared["pmv"] = np.ascontiguousarray(pm).astype(bf16)
    shared["pmk"] = np.ascontiguousarray(pm.transpose(0, 2, 1)).astype(bf16)
    b1 = np.asarray(inputs["ff_b1"], f32)
    b1a = np.zeros(DFFP, f32)
    b1g = np.zeros(DFFP, f32)
    b1a[:DFF] = b1[:DFF]
    b1g[:DFF] = b1[DFF:]
    shared["b1a"] = np.ascontiguousarray(b1a.reshape(MFF, P).T)
    shared["b1g"] = np.ascontiguousarray(b1g.reshape(MFF, P).T)
    shared["b2"] = np.ascontiguousarray(
        np.asarray(inputs["ff_b2"], f32).reshape(KO, P).T)
    shared["anw"] = np.ascontiguousarray(
        np.asarray(inputs["attn_norm_w"], f32).reshape(KO, P).T)
    shared["fnw"] = np.ascontiguousarray(
        np.asarray(inputs["ff_norm_w"], f32).reshape(KO, P).T)
    shared["mpnw"] = np.ascontiguousarray(
        np.asarray(inputs["mem_pool_norm_w"], f32).reshape(1, DIM))
    rl = np.zeros((P, P), f32)
    ii = np.arange(0, P, 2)
    rl[ii + 1, ii] = f32(-1.0)
    rl[ii, ii + 1] = f32(1.0)
    shared["rmat"] = rl.astype(bf16)
    shared["maskD"] = np.where(
        np.arange(P)[None, :] >= np.arange(P)[:, None], f32(1.0), f32(0.0)
    ).astype(bf16)

    # per-batch bf16 mem_out
    mo = np.asarray(inputs["mem_out"], f32)
    shared["_mo"] = [np.ascontiguousarray(mo[b]).astype(bf16) for b in range(B)]

    # rope tables, float32 math to match the reference
    pos = np.arange(N, dtype=f32)
    expo = (np.arange(0, DH, 2).astype(f32) / f32(DH)).astype(f32)
    inv = (f32(1.0) / np.power(f32(10000.0), expo)).astype(f32)
    ang = np.repeat(pos[:, None] * inv[None, :], 2, axis=1).astype(f32)
    cosf, sinf = np.cos(ang).astype(f32), np.sin(ang).astype(f32)
    scale = f32(DH ** -0.5)
    shared["_cos"], shared["_sin"], shared["_scale"] = cosf, sinf, scale
    return shared


def _prep_core(inputs, shared, b, s):
    f32 = np.float32
    x = np.asarray(inputs["x"], f32)
    cosf, sinf, scale = shared["_cos"], shared["_sin"], shared["_scale"]
    seg = slice(s * SEG, (s + 1) * SEG)
    ct = np.ascontiguousarray(np.tile(cosf[seg].T, (2, 1)))
    st = np.ascontiguousarray(np.tile(sinf[seg].T, (2, 1)))
    m = {k: v for k, v in shared.items() if not k.startswith("_")}
    m["xT"] = np.ascontiguousarray(x[b, seg].T)
    m["mo"] = shared["_mo"][b]
    m["cq"] = (ct * scale).astype(f32)
    m["sq"] = (st * scale).astype(f32)
    m["ck"] = ct
    m["sk"] = st
    return m


def _get_nc():
    if "nc" not in _CACHE:
        _CACHE["nc"] = build_nc()
    return _CACHE["nc"]


def kernel(**inputs) -> np.ndarray:
    nc = _get_nc()
    shared = _prep_shared(inputs)
    cores = [(b, s) for b in range(B) for s in range(4)]
    in_maps = [_prep_core(inputs, shared, b, s) for b, s in cores]
    from concourse import bass_utils
    import os
    res = bass_utils.run_bass_kernel_spmd(
        nc, in_maps, core_ids=list(range(NCORES)),
        trace=bool(os.environ.get("MAC_TRACE")))
    _CACHE["last_results"] = res
    out = np.empty((B, N, DIM), np.float32)
    for i, (b, s) in enumerate(cores):
        out[b, s * SEG:(s + 1) * SEG, :] = res.results[i]["yT"].T
    return out


# revision 63
# speedup vs baseline: 1.7777x; 1.0333x over previous
"""Trainium2 Bass kernel for nn_MACBlock (segmented attention + GEGLU FFN).

Sharding: 8 cores = 2 batches x 4 segments of 512 queries. The segment mask
makes attention block-diagonal (plus a 32-token always-visible prefix derived
from pooled memory + persistent memory), so each core is fully independent:
no collectives.

Layout: activations are kept feature-major (x^T [dim, tokens]) on-chip, so
every matmul contraction dim lands on partitions with zero transposes.
All heavy GEMMs run in bf16 (weights pre-cast on host, activations cast
on-chip): bf16 enables Fast Weight Load and avoids the fp32-HIGH power
throttle that halves the PE clock. PSUM accumulation stays fp32.
Scores are computed key-major ([keys, queries]); softmax is max-free; the
softmax denominator comes from a ones-column folded into the P@V stationary
operand (row DH of the same PSUM tile).
"""

import sys

if "/opt/trn_rl_repo" not in sys.path:
    sys.path.insert(0, "/opt/trn_rl_repo")

import numpy as np

B, N, DIM = 2, 2048, 1024
HEADS, DH = 16, 64
SEG = 512
NPM = NM = 16
PFX = NPM + NM          # 32 prefix keys
DFF = 2730
MFF = 22                # padded dff chunks
DFFP = MFF * 128        # 2816
KO = 8                  # 1024 / 128
P = 128
NCORES = 8
EPS = 1.1920929e-07

_CACHE = {}


def _f32r(ap):
    import concourse.mybir as mybir
    return ap.bitcast(mybir.dt.float32r)


def build_nc(reps=1):
    import concourse.bass as bass
    from concourse import bacc
    import concourse.tile as tile
    import concourse.mybir as mybir

    f32 = mybir.dt.float32
    bf16 = mybir.dt.bfloat16
    AF = mybir.ActivationFunctionType
    OP = mybir.AluOpType
    AX = mybir.AxisListType

    nc = bacc.Bacc("TRN2", target_bir_lowering=False, debug=False)

    dp = nc.declare_dram_parameter
    xT_d = dp("xT", [DIM, SEG], f32, isOutput=False)
    mo_d = dp("mo", [N, DIM], bf16, isOutput=False)
    cq_d = dp("cq", [P, SEG], f32, isOutput=False)
    sq_d = dp("sq", [P, SEG], f32, isOutput=False)
    ck_d = dp("ck", [P, SEG], f32, isOutput=False)
    sk_d = dp("sk", [P, SEG], f32, isOutput=False)
    mask_d = dp("maskD", [P, P], bf16, isOutput=False)
    rmat_d = dp("rmat", [P, P], bf16, isOutput=False)
    qkw_d = dp("qkw", [16, P, KO, P], bf16, isOutput=False)
    kvw_d = dp("kvw", [2, KO, P, DIM], bf16, isOutput=False)
    outw_d = dp("outw", [KO, P, KO, P], bf16, isOutput=False)
    w1a_d = dp("w1a", [MFF, P, KO, P], bf16, isOutput=False)
    w1g_d = dp("w1g", [MFF, P, KO, P], bf16, isOutput=False)
    w2_d = dp("w2", [KO, P, MFF, P], bf16, isOutput=False)
    mtw_d = dp("mtw", [KO, P, DIM], bf16, isOutput=False)
    pmv_d = dp("pmv", [HEADS, NPM, DH], bf16, isOutput=False)
    pmk_d = dp("pmk", [HEADS, DH, NPM], bf16, isOutput=False)
    b1a_d = dp("b1a", [P, MFF], f32, isOutput=False)
    b1g_d = dp("b1g", [P, MFF], f32, isOutput=False)
    b2_d = dp("b2", [P, KO], f32, isOutput=False)
    anw_d = dp("anw", [P, KO], f32, isOutput=False)
    fnw_d = dp("fnw", [P, KO], f32, isOutput=False)
    mpnw_d = dp("mpnw", [1, DIM], f32, isOutput=False)
    yT_d = dp("yT", [DIM, SEG], f32, isOutput=True)

    def _emit(nc):
      with tile.TileContext(nc) as tc, \
            nc.allow_low_precision(reason="bf16 matmul rounding"):
        from contextlib import ExitStack
        ctx = ExitStack()
        with ctx:
            persist = ctx.enter_context(tc.tile_pool(name="persist", bufs=1))
            wpool = ctx.enter_context(tc.tile_pool(name="wpool", bufs=6))
            kvpool = ctx.enter_context(tc.tile_pool(name="kvpool", bufs=6))
            w2pool = ctx.enter_context(tc.tile_pool(name="w2pool", bufs=4))
            mopool = ctx.enter_context(tc.tile_pool(name="mopool", bufs=6))
            rot = ctx.enter_context(tc.tile_pool(name="rot", bufs=2))
            epool = ctx.enter_context(tc.tile_pool(name="epool", bufs=3))
            pa = ctx.enter_context(tc.tile_pool(name="pa", bufs=3, space="PSUM"))
            psc = ctx.enter_context(tc.tile_pool(name="psc", bufs=2, space="PSUM"))
            ppo = ctx.enter_context(tc.tile_pool(name="ppo", bufs=3, space="PSUM"))

            cnt = [0]

            def pa_t():
                cnt[0] += 1
                return pa.tile([P, SEG], f32, tag="ps", name=f"pa{cnt[0]}")

            def psc_t():
                cnt[0] += 1
                return psc.tile([P, SEG], f32, tag="sc", name=f"sc{cnt[0]}")

            def ppo_t():
                cnt[0] += 1
                return ppo.tile([P, SEG], f32, tag="o", name=f"o{cnt[0]}")

            # ---------------- persistent SBUF tensors ----------------
            xT = persist.tile([P, KO, SEG], f32, tag="xT")       # x^T, later x1^T
            xnT = persist.tile([P, KO, SEG], bf16, tag="xnT")    # xn^T, later xn1^T
            kT = persist.tile([P, KO, SEG], bf16, tag="kT")      # roped k^T
            vA = persist.tile([P, 4, HEADS, DH + 1], bf16, tag="vA")  # v key-major
            vP = persist.tile([PFX, HEADS, DH + 1], bf16, tag="vP")   # prefix v rows
            kP = persist.tile([P, HEADS, PFX], bf16, tag="kP")   # prefix k^T @64*(h%2)
            oA = persist.tile([P, KO, SEG], bf16, tag="oA")      # attn o^T
            cq = persist.tile([P, SEG], f32, tag="cq")
            sq_ = persist.tile([P, SEG], f32, tag="sq")
            ck = persist.tile([P, SEG], f32, tag="ck")
            sk = persist.tile([P, SEG], f32, tag="sk")
            maskD = persist.tile([P, P], bf16, tag="maskD")
            rmat = persist.tile([P, P], bf16, tag="rmat")
            b1a = persist.tile([P, MFF], f32, tag="b1a")
            b1g = persist.tile([P, MFF], f32, tag="b1g")
            b2 = persist.tile([P, KO], f32, tag="b2")
            anw = persist.tile([P, KO], f32, tag="anw")
            fnw = persist.tile([P, KO], f32, tag="fnw")
            mpnw = persist.tile([1, DIM], f32, tag="mpnw")
            ones16 = persist.tile([1, 16], bf16, tag="o16")
            ones11 = persist.tile([1, 1], bf16, tag="o11")
            onesHH = persist.tile([DH, P], bf16, tag="oHH")  # pair bcast lhsT
            ones128b = persist.tile([P, 1], bf16, tag="o128b")   # lhsT K=128,M=1
            ones1xPb = persist.tile([1, P], bf16, tag="o1xPb")   # lhsT K=1,M=128
            ones1xP = persist.tile([1, P], f32, tag="o1xP")      # f32 variant
            pooledT = persist.tile([P, KO], bf16, tag="pooledT")
            memtokT = persist.tile([P, KO], bf16, tag="memtokT")
            mrow = persist.tile([1, 3 * DIM], f32, tag="mrow")
            mrowb = persist.tile([1, 3 * DIM], bf16, tag="mrowb")
            rrow = persist.tile([1, 2 * SEG], f32, tag="rrow")
            rrowb = persist.tile([1, SEG], bf16, tag="rrowb")
            epsc = persist.tile([P, 1], f32, tag="epsc")
            zeroc = persist.tile([P, 1], f32, tag="zeroc")

            dma = nc.sync.dma_start
            dma(out=cq, in_=cq_d[:])
            dma(out=sq_, in_=sq_d[:])
            dma(out=ck, in_=ck_d[:])
            dma(out=sk, in_=sk_d[:])
            dma(out=maskD, in_=mask_d[:])
            dma(out=rmat, in_=rmat_d[:])
            dma(out=b1a, in_=b1a_d[:])
            dma(out=b1g, in_=b1g_d[:])
            dma(out=b2, in_=b2_d[:])
            dma(out=anw, in_=anw_d[:])
            dma(out=fnw, in_=fnw_d[:])
            dma(out=mpnw, in_=mpnw_d[:])
            nc.vector.memset(ones16, 1.0)
            nc.vector.memset(ones11, 1.0)
            # pair-broadcast stationary: row 0 -> out rows 0-63, row 32 ->
            # out rows 64-127; all other contraction rows are zero, and the
            # matching moving-tile rows are zero-primed below, so they
            # contribute exactly 0 to the K=64 contraction.
            nc.vector.memset(onesHH, 0.0)
            nc.vector.memset(onesHH[0:1, 0:DH], 1.0)
            nc.vector.memset(onesHH[32:33, DH:P], 1.0)
            # zero-prime both rot-pool buffers of the pair-reciprocal moving
            # tile: its rows other than 0/32 are never written afterwards
            for _ in range(2):
                _rz = rot.tile([DH, SEG], bf16, tag="rvb")
                nc.vector.memset(_rz, 0.0)
            nc.vector.memset(ones128b, 1.0)
            nc.vector.memset(ones1xPb, 1.0)
            nc.vector.memset(ones1xP, 1.0)
            nc.vector.memset(epsc, EPS)
            nc.vector.memset(zeroc, 0.0)
            # denominator ones-columns in the P@V stationary operands
            nc.vector.memset(vA[:, :, :, DH:DH + 1], 1.0)
            nc.vector.memset(vP[:, :, DH:DH + 1], 1.0)
            for h in range(HEADS):
                hb = DH * (h % 2)
                dma(out=kP[hb:hb + DH, h, NPM:PFX], in_=pmk_d[h])
                dma(out=vP[NPM:PFX, h, 0:DH], in_=pmv_d[h])

            if True:
              for ko in range(KO):
                  dma(out=xT[:, ko, :], in_=xT_d[ko * P:(ko + 1) * P, :])

              mm = nc.tensor.matmul

              def rmsnorm_into(dst, src, w_sb, sq_tag):
                  """dst[:,ko,:] = src[:,ko,:] * w[:,ko] * rsqrt(mean_dim(src^2)+eps)"""
                  ss = psc_t()  # [1,512] slice used
                  sq8 = persist.tile([P, KO, SEG], bf16, tag=sq_tag, name="sq8")
                  for ko in range(KO):
                      nc.vector.tensor_mul(sq8[:, ko, :], src[:, ko, :],
                                           src[:, ko, :])
                      mm(ss[0:1, :], ones128b, sq8[:, ko, :],
                         start=(ko == 0), stop=(ko == KO - 1))
                  nc.scalar.activation(rrow[:, 0:SEG], ss[0:1, :], AF.Sqrt,
                                       bias=epsc[0:1], scale=1.0 / DIM)
                  nc.vector.reciprocal_approx_fast(out=rrow[:, SEG:2 * SEG],
                                                   in_=rrow[:, 0:SEG])
                  nc.vector.tensor_copy(out=rrowb, in_=rrow[:, SEG:2 * SEG])
                  bc = psc_t()  # broadcast rstd over 128 partitions
                  mm(bc, ones1xPb, rrowb,
                     start=True, stop=True)
                  for ko in range(KO):
                      nc.vector.scalar_tensor_tensor(
                          out=dst[:, ko, :], in0=src[:, ko, :],
                          scalar=w_sb[:, ko:ko + 1], in1=bc,
                          op0=OP.mult, op1=OP.mult)

              # ---------------- attn rmsnorm ----------------
              rmsnorm_into(xnT, xT, anw, "big16")
              qT = persist.tile([P, KO, SEG], bf16, tag="qT")    # roped,scaled q^T

              # ---------------- q/k projections + rope, interleaved with
              # ---------------- mem_out mean accumulation ----------------
              # [1,512] accumulator pairs live at partition rows 0 and 32 of
              # a single PSUM tile (matmul out base partition must be 0/32/64)
              mean_ps = psc_t()

              def mo_mean_step(t):
                  mot = mopool.tile([P, DIM], bf16, tag="mo", name="mot")
                  dma(out=mot, in_=mo_d[t * P:(t + 1) * P, :])
                  for half in range(2):
                      r = 32 * half
                      mm(mean_ps[r:r + 1, :], ones128b,
                         mot[:, half * SEG:(half + 1) * SEG],
                         start=(t == 0), stop=(t == 15))

              # pooled rmsnorm (pure ACT/DVE): emitted mid-qk-loop so its
              # serial latency hides under the remaining projection matmuls
              pooled_raw = mrow[:, 0:DIM]
              sqr = mrow[:, DIM:2 * DIM]
              pooled = mrowb[:, 2 * DIM:3 * DIM]  # bf16 row for transposes

              def pooled_chain():
                  for half in range(2):
                      r = 32 * half
                      nc.scalar.activation(_f32r(pooled_raw[:, half * SEG:(half + 1) * SEG]),
                                           mean_ps[r:r + 1, :], AF.Copy,
                                           scale=1.0 / N)
                  nc.vector.tensor_mul(_f32r(sqr), pooled_raw, pooled_raw)
                  nc.vector.reduce_sum(_f32r(sqr[:, 0:1]), sqr, axis=AX.X)
                  nc.scalar.activation(_f32r(sqr[:, 1:2]), sqr[:, 0:1], AF.Sqrt,
                                       bias=epsc[0:1], scale=1.0 / DIM)
                  nc.vector.reciprocal(_f32r(sqr[:, 2:3]), sqr[:, 1:2])
                  nc.vector.scalar_tensor_tensor(out=pooled, in0=pooled_raw,
                                                 scalar=sqr[:, 2:3], in1=mpnw,
                                                 op0=OP.mult, op1=OP.mult)

              # software-pipelined: the rope finish (rmat matmul + DVE
              # combine) for iteration m is emitted during iteration m+1 so
              # the ACT qraw copy never stalls the PE FIFO
              def rope_finish(ps, qraw, m):
                  is_q = m < 8
                  c_t, s_t = (cq, sq_) if is_q else (ck, sk)
                  dst = qT if is_q else kT
                  At = rot.tile([P, SEG], bf16, tag="ropeB")
                  nc.vector.tensor_mul(At, ps, c_t)  # before rps: frees ps
                  rps = pa_t()
                  mm(rps, rmat, qraw, start=True, stop=True)
                  Bt = rot.tile([P, SEG], bf16, tag="ropeA")
                  nc.vector.tensor_mul(Bt, rps, s_t)
                  nc.vector.tensor_add(dst[:, m % 8, :], At, Bt)

              pend = None
              for m in range(16):
                  wt = wpool.tile([P, KO, P], bf16, tag="w8")
                  dma(out=wt, in_=qkw_d[m])
                  ps = pa_t()
                  for ko in range(KO):
                      mm(ps, wt[:, ko], xnT[:, ko, :],
                         start=(ko == 0), stop=(ko == KO - 1))
                  qraw = rot.tile([P, SEG], bf16, tag="qraw")
                  nc.scalar.copy(qraw, ps)
                  if pend is not None:
                      rope_finish(*pend)
                  pend = (ps, qraw, m)
                  if 1 <= m < 9:
                      mo_mean_step(2 * (m - 1))
                      mo_mean_step(2 * (m - 1) + 1)
                  if m == 10:
                      pooled_chain()
              rope_finish(*pend)

              def stage_pT():
                  pT = pa_t()
                  for ko in range(KO):
                      mm(pT[:, ko:ko + 1], pooled[0:1, ko * P:(ko + 1) * P],
                         ones11, start=True, stop=True, skip_group_check=True)
                  nc.vector.tensor_copy(out=pooledT, in_=pT[:, 0:KO])

              kvvs = []  # v-proj weights, hoisted so DMAs overlap qk tail

              def load_kvv():
                  for half in range(2):
                      kvv = persist.tile([P, KO, SEG], bf16,
                                         tag=("kvv0" if half == 0 else "big16"))
                      for ko in range(KO):
                          dma(out=kvv[:, ko, :],
                              in_=kvw_d[1, ko, :, half * SEG:(half + 1) * SEG])
                      kvvs.append(kvv)

              mt_ps = []

              def stage_mt():
                  mt_ps.append(psc_t())
                  for ko in range(KO):
                      mtw_t = kvpool.tile([P, DIM], bf16, tag="kv")
                      dma(out=mtw_t, in_=mtw_d[ko])
                      for half in range(2):
                          r = 32 * half
                          mm(mt_ps[0][r:r + 1, :], pooledT[:, ko:ko + 1],
                             mtw_t[:, half * SEG:(half + 1) * SEG],
                             start=(ko == 0), stop=(ko == KO - 1))

              memtok = mrowb[:, 0:DIM]

              def stage_mT():
                  for half in range(2):
                      r = 32 * half
                      nc.scalar.activation(memtok[:, half * SEG:(half + 1) * SEG],
                                           mt_ps[0][r:r + 1, :], AF.Copy)
                  mT = pa_t()
                  for ko in range(KO):
                      mm(mT[:, ko:ko + 1], memtok[0:1, ko * P:(ko + 1) * P],
                         ones11, start=True, stop=True, skip_group_check=True)
                  nc.vector.tensor_copy(out=memtokT, in_=mT[:, 0:KO])

              kc_row = mrowb[:, DIM:2 * DIM]
              vc_row = mrowb[:, 2 * DIM:3 * DIM]

              def stage_kcvc(c):
                  r_ps = psc_t()
                  for ko in range(KO):
                      kv_t = kvpool.tile([P, DIM], bf16, tag="kv")
                      dma(out=kv_t, in_=kvw_d[c, ko])
                      for half in range(2):
                          r = 32 * half
                          mm(r_ps[r:r + 1, :], memtokT[:, ko:ko + 1],
                             kv_t[:, half * SEG:(half + 1) * SEG],
                             start=(ko == 0), stop=(ko == KO - 1))
                  row = kc_row if c == 0 else vc_row
                  for half in range(2):
                      r = 32 * half
                      nc.scalar.activation(row[:, half * SEG:(half + 1) * SEG],
                                           r_ps[r:r + 1, :], AF.Copy)

              def stage_kx_j(j):  # 2 heads per chunk
                  kx = pa_t()
                  mm(kx[:, 0:16], kc_row[0:1, j * P:(j + 1) * P],
                     ones16, start=True, stop=True,
                     skip_group_check=True)
                  nc.vector.tensor_copy(out=kP[0:DH, 2 * j, 0:NPM],
                                        in_=kx[0:DH, 0:16])
                  nc.vector.tensor_copy(out=kP[DH:P, 2 * j + 1, 0:NPM],
                                        in_=kx[DH:P, 0:16])

              def stage_vx_half(half):
                  vx = pa_t()
                  mm(vx[0:16, :], ones16,
                     vc_row[0:1, half * SEG:(half + 1) * SEG],
                     start=True, stop=True, skip_group_check=True)
                  nc.vector.tensor_copy(
                      out=vP[0:NPM, half * 8:(half + 1) * 8, 0:DH],
                      in_=vx[0:16, :].rearrange("p (h d) -> p h d", d=DH))

              load_kvv()  # all v-weight DMAs issued up front
              stage_pT()  # pooled ready ~3 qk-iterations ago
              stage_sched = {1: stage_mt, 3: stage_mT,
                             5: lambda: stage_kcvc(0),
                             6: lambda: stage_kcvc(1)}

              # ---------------- v projection (token-major), interleaved ----
              # (kx/vx stages are woven into the attention head loop below)
              for half in range(2):
                  kvv = kvvs[half]
                  for tc_ in range(4):
                      ps = pa_t()
                      for ko in range(KO):
                          mm(ps, xnT[:, ko, tc_ * P:(tc_ + 1) * P],
                             kvv[:, ko, :],
                             start=(ko == 0), stop=(ko == KO - 1))
                      nc.vector.tensor_copy(
                          out=vA[:, tc_, half * 8:(half + 1) * 8, 0:DH],
                          in_=ps.rearrange("p (h d) -> p h d", d=DH))
                      blk = half * 4 + tc_
                      if blk in stage_sched:
                          stage_sched[blk]()

              # ---------------- attention heads ----------------
              # Scores chunk c covers keys [cP,(c+1)P) x queries [cP,SEG)
              # (block-triangular). Chunks 1 and 3 share one PSUM tile /
              # exp pass: c1 at free 0:384 (queries 128:512), c3 at free
              # 384:512 (queries 384:512). The P@V stationary has a ones
              # column, so PSUM row DH is the softmax denominator.
              # The divide chain for a head PAIR (2h, 2h+1) is emitted two
              # heads later (software pipelining), with one reciprocal /
              # broadcast / copy serving both heads, so its serial ACT/DVE
              # latency never head-of-line-blocks the engine FIFOs.
              def divide_pair(h2, po_a, po_b):
                  ko_h = h2 // 2
                  drow2 = rot.tile([DH, SEG], f32, tag="dr")
                  nc.scalar.copy(drow2[0:1, :], po_a[DH:DH + 1, :])
                  nc.vector.tensor_copy(out=drow2[32:33, :],
                                        in_=po_b[DH:DH + 1, :])
                  rvf2 = rot.tile([DH, SEG], f32, tag="rv")
                  nc.vector.reciprocal_approx_fast(out=rvf2[0:33, :],
                                                   in_=drow2[0:33, :])
                  rvb2 = rot.tile([DH, SEG], bf16, tag="rvb")
                  nc.vector.tensor_copy(out=rvb2[0:1, :], in_=rvf2[0:1, :])
                  nc.vector.tensor_copy(out=rvb2[32:33, :], in_=rvf2[32:33, :])
                  bcp2 = psc_t()  # rows 0-63 = 1/d_a, rows 64-127 = 1/d_b
                  mm(bcp2, onesHH, rvb2,
                     start=True, stop=True, skip_group_check=True)
                  bcs2 = epool.tile([P, SEG], bf16, tag="bcs")
                  nc.vector.tensor_copy(out=bcs2, in_=bcp2)
                  nc.vector.tensor_mul(oA[0:DH, ko_h, :],
                                       po_a[0:DH, :], bcs2[0:DH, :])
                  nc.vector.tensor_mul(oA[DH:P, ko_h, :],
                                       po_b[0:DH, :], bcs2[DH:P, :])

              pend_pair = None
              po_prev = None
              for h in range(HEADS):
                  if h % 2 == 1 and pend_pair is not None:
                      # emitted BEFORE this head's po allocation so the
                      # 3-buf ppo rotation never recycles an unread tile
                      divide_pair(*pend_pair)
                      pend_pair = None
                  if h % 2 == 0:
                      stage_kx_j(h // 2)
                  if h == 0:
                      stage_vx_half(0)
                  if h == 6:
                      stage_vx_half(1)
                  ko_h, hf = h // 2, h % 2
                  qr = DH * hf
                  q_h = qT[qr:qr + DH, ko_h, :]
                  k_h = kT[qr:qr + DH, ko_h, :]
                  # prefix scores [32, 512]
                  scp = psc_t()
                  mm(scp[0:PFX, :], kP[qr:qr + DH, h, :], q_h,
                     start=True, stop=True, skip_group_check=True)
                  eP = epool.tile([PFX, SEG], bf16, tag="eP")
                  nc.scalar.activation(eP, scp[0:PFX, :], AF.Exp,
                                       bias=zeroc[0:PFX])
                  sc0 = pa_t()  # pa is near-idle during attention
                  mm(sc0[:, :], k_h[:, 0:P], q_h,
                     start=True, stop=True, skip_group_check=True)
                  e0 = epool.tile([P, SEG], bf16, tag="e0")
                  nc.scalar.activation(e0, sc0, AF.Exp, bias=zeroc)
                  nc.gpsimd.tensor_mul(e0[:, 0:P], e0[:, 0:P], maskD)
                  sc13 = psc_t()
                  mm(sc13[:, 0:384], k_h[:, P:2 * P], q_h[:, P:],
                     start=True, stop=True, skip_group_check=True)
                  mm(sc13[:, 384:512], k_h[:, 3 * P:4 * P], q_h[:, 3 * P:],
                     start=True, stop=True, skip_group_check=True)
                  eB = epool.tile([P, SEG], bf16, tag="eB")
                  nc.scalar.activation(eB, sc13, AF.Exp, bias=zeroc)
                  nc.gpsimd.tensor_mul(eB[:, 0:P], eB[:, 0:P], maskD)
                  nc.gpsimd.tensor_mul(eB[:, 384:512], eB[:, 384:512], maskD)
                  sc2 = pa_t()
                  mm(sc2[:, 0:256], k_h[:, 2 * P:3 * P], q_h[:, 2 * P:],
                     start=True, stop=True, skip_group_check=True)
                  e2 = epool.tile([P, 256], bf16, tag="e2")
                  nc.scalar.activation(e2, sc2[:, 0:256], AF.Exp, bias=zeroc)
                  nc.gpsimd.tensor_mul(e2[:, 0:P], e2[:, 0:P], maskD)
                  # P@V with ones-column: row DH = softmax denominator
                  po = ppo_t()  # dedicated pool so heads pipeline
                  mm(po[0:DH + 1, :], vP[:, h, :], eP,
                     start=True, stop=False, skip_group_check=True)
                  mm(po[0:DH + 1, 0:], vA[:, 0, h, :], e0,
                     start=False, stop=False, skip_group_check=True)
                  mm(po[0:DH + 1, P:], vA[:, 1, h, :], eB[:, 0:384],
                     start=False, stop=False, skip_group_check=True)
                  mm(po[0:DH + 1, 2 * P:], vA[:, 2, h, :], e2,
                     start=False, stop=False, skip_group_check=True)
                  mm(po[0:DH + 1, 3 * P:], vA[:, 3, h, :], eB[:, 384:512],
                     start=False, stop=True, skip_group_check=True)
                  if h % 2 == 0:
                      po_prev = po
                  else:
                      pend_pair = (h - 1, po_prev, po)
              divide_pair(*pend_pair)

              # ---------------- output projection + residual ----------------
              # FFN rmsnorm statistics are accumulated in the same loop so
              # the second norm adds no serial latency.
              ss2 = psc_t()
              sq8b = persist.tile([P, KO, SEG], bf16, tag="big16", name="sq8b")
              for m in range(KO):
                  wt = wpool.tile([P, KO, P], bf16, tag="w8")
                  dma(out=wt, in_=outw_d[m])
                  ps = pa_t()
                  for k in range(KO):
                      mm(ps, wt[:, k], oA[:, k, :],
                         start=(k == 0), stop=(k == KO - 1))
                  nc.vector.tensor_add(xT[:, m, :], ps, xT[:, m, :])  # x1
                  nc.vector.tensor_mul(sq8b[:, m, :], xT[:, m, :], xT[:, m, :])
                  mm(ss2[0:1, :], ones128b, sq8b[:, m, :],
                     start=(m == 0), stop=(m == KO - 1))

              # ---------------- FFN ----------------
              nc.scalar.activation(rrow[:, 0:SEG], ss2[0:1, :], AF.Sqrt,
                                   bias=epsc[0:1], scale=1.0 / DIM)
              nc.vector.reciprocal_approx_fast(out=rrow[:, SEG:2 * SEG],
                                               in_=rrow[:, 0:SEG])
              nc.vector.tensor_copy(out=rrowb, in_=rrow[:, SEG:2 * SEG])
              bc2 = psc_t()
              mm(bc2, ones1xPb, rrowb, start=True, stop=True)
              for ko in range(KO):
                  nc.vector.scalar_tensor_tensor(
                      out=xnT[:, ko, :], in0=xT[:, ko, :],
                      scalar=fnw[:, ko:ko + 1], in1=bc2,
                      op0=OP.mult, op1=OP.mult)
              u_parts = [qT, kT]  # reuse dead slots as u storage
              u_c = persist.tile([P, 6, SEG], bf16, tag="big16")

              def u_slice(k):
                  if k < 8:
                      return u_parts[0][:, k, :]
                  if k < 16:
                      return u_parts[1][:, k - 8, :]
                  return u_c[:, k - 16, :]

              for m in range(MFF):
                  wa = wpool.tile([P, KO, P], bf16, tag="w8")
                  dma(out=wa, in_=w1a_d[m])
                  wg = wpool.tile([P, KO, P], bf16, tag="w8")
                  dma(out=wg, in_=w1g_d[m])
                  psa = pa_t()
                  psg = ppo_t()  # ppo idle during FFN; full double-buffering
                  for ko in range(KO):
                      mm(psa, wa[:, ko], xnT[:, ko, :],
                         start=(ko == 0), stop=(ko == KO - 1))
                      mm(psg, wg[:, ko], xnT[:, ko, :],
                         start=(ko == 0), stop=(ko == KO - 1))
                  sig = rot.tile([P, SEG], f32, tag="ropeA")
                  nc.scalar.activation(sig, psg, AF.Sigmoid,
                                       bias=b1g[:, m:m + 1], scale=1.0)
                  silu = rot.tile([P, SEG], f32, tag="ropeB")
                  nc.vector.scalar_tensor_tensor(
                      out=silu, in0=psg, scalar=b1g[:, m:m + 1],
                      in1=sig, op0=OP.add, op1=OP.mult)
                  nc.vector.scalar_tensor_tensor(
                      out=u_slice(m), in0=psa, scalar=b1a[:, m:m + 1],
                      in1=silu, op0=OP.add, op1=OP.mult)

              for o in range(KO):
                  ps = pa_t()
                  for half in range(2):
                      w2t = w2pool.tile([P, 11, P], bf16, tag="w2")
                      dma(out=w2t, in_=w2_d[o][:, half * 11:(half + 1) * 11, :])
                      for k2 in range(11):
                          k = half * 11 + k2
                          mm(ps, w2t[:, k2], u_slice(k),
                             start=(k == 0), stop=(k == MFF - 1))
                  outT = persist.tile([P, KO, SEG], f32, tag="kvv0",
                                      name=f"outT{o}")
                  nc.vector.scalar_tensor_tensor(
                      out=outT[:, o, :], in0=ps, scalar=b2[:, o:o + 1],
                      in1=xT[:, o, :], op0=OP.add, op1=OP.add)
                  dma(out=yT_d[o * P:(o + 1) * P, :], in_=outT[:, o, :])

    for _rep in range(reps):
        _emit(nc)
    nc.compile()
    return nc


# ======================= host-side preparation =======================

def _prep_shared(inputs):
    import ml_dtypes
    f32 = np.float32
    bf16 = ml_dtypes.bfloat16
    qkv = np.asarray(inputs["to_qkv_w"], f32)
    shared = {}
    shared["qkw"] = np.ascontiguousarray(
        qkv[:, :2048].reshape(KO, P, 16, P).transpose(2, 1, 0, 3)).astype(bf16)
    shared["kvw"] = np.ascontiguousarray(
        np.stack([qkv[:, 1024:2048], qkv[:, 2048:3072]])
        .reshape(2, KO, P, DIM)).astype(bf16)
    shared["outw"] = np.ascontiguousarray(
        np.asarray(inputs["to_out_w"], f32)
        .reshape(KO, P, KO, P).transpose(2, 1, 0, 3)).astype(bf16)
    w1 = np.asarray(inputs["ff_w1"], f32)
    w1a = np.zeros((DIM, DFFP), f32)
    w1g = np.zeros((DIM, DFFP), f32)
    w1a[:, :DFF] = w1[:, :DFF]
    w1g[:, :DFF] = w1[:, DFF:]
    shared["w1a"] = np.ascontiguousarray(
        w1a.reshape(KO, P, MFF, P).transpose(2, 1, 0, 3)).astype(bf16)
    shared["w1g"] = np.ascontiguousarray(
        w1g.reshape(KO, P, MFF, P).transpose(2, 1, 0, 3)).astype(bf16)
    w2 = np.zeros((DFFP, DIM), f32)
    w2[:DFF] = np.asarray(inputs["ff_w2"], f32)
    shared["w2"] = np.ascontiguousarray(
        w2.reshape(MFF, P, KO, P).transpose(2, 1, 0, 3)).astype(bf16)
    shared["mtw"] = np.ascontiguousarray(
        np.asarray(inputs["to_mem_tokens_w"], f32).reshape(KO, P, DIM)).astype(bf16)
    pm = np.asarray(inputs["persist_mem"], f32)
    shared["pmv"] = np.ascontiguousarray(pm).astype(bf16)
    shared["pmk"] = np.ascontiguousarray(pm.transpose(0, 2, 1)).astype(bf16)
    b1 = np.asarray(inputs["ff_b1"], f32)
    b1a = np.zeros(DFFP, f32)
    b1g = np.zeros(DFFP, f32)
    b1a[:DFF] = b1[:DFF]
    b1g[:DFF] = b1[DFF:]
    shared["b1a"] = np.ascontiguousarray(b1a.reshape(MFF, P).T)
    shared["b1g"] = np.ascontiguousarray(b1g.reshape(MFF, P).T)
    shared["b2"] = np.ascontiguousarray(
        np.asarray(inputs["ff_b2"], f32).reshape(KO, P).T)
    shared["anw"] = np.ascontiguousarray(
        np.asarray(inputs["attn_norm_w"], f32).reshape(KO, P).T)
    shared["fnw"] = np.ascontiguousarray(
        np.asarray(inputs["ff_norm_w"], f32).reshape(KO, P).T)
    shared["mpnw"] = np.ascontiguousarray(
        np.asarray(inputs["mem_pool_norm_w"], f32).reshape(1, DIM))
    rl = np.zeros((P, P), f32)
    ii = np.arange(0, P, 2)
    rl[ii + 1, ii] = f32(-1.0)
    rl[ii, ii + 1] = f32(1.0)
    shared["rmat"] = rl.astype(bf16)
    shared["maskD"] = np.where(
        np.arange(P)[None, :] >= np.arange(P)[:, None], f32(1.0), f32(0.0)
    ).astype(bf16)

    # per-batch bf16 mem_out
    mo = np.asarray(inputs["mem_out"], f32)
    shared["_mo"] = [np.ascontiguousarray(mo[b]).astype(bf16) for b in range(B)]

    # rope tables, float32 math to match the reference
    pos = np.arange(N, dtype=f32)
    expo = (np.arange(0, DH, 2).astype(f32) / f32(DH)).astype(f32)
    inv = (f32(1.0) / np.power(f32(10000.0), expo)).astype(f32)
    ang = np.repeat(pos[:, None] * inv[None, :], 2, axis=1).astype(f32)
    cosf, sinf = np.cos(ang).astype(f32), np.sin(ang).astype(f32)
    scale = f32(DH ** -0.5)
    shared["_cos"], shared["_sin"], shared["_scale"] = cosf, sinf, scale
    return shared


def _prep_core(inputs, shared, b, s):
    f32 = np.float32
    x = np.asarray(inputs["x"], f32)
    cosf, sinf, scale = shared["_cos"], shared["_sin"], shared["_scale"]
    seg = slice(s * SEG, (s + 1) * SEG)
    ct = np.ascontiguousarray(np.tile(cosf[seg].T, (2, 1)))
    st = np.ascontiguousarray(np.tile(sinf[seg].T, (2, 1)))
    m = {k: v for k, v in shared.items() if not k.startswith("_")}
    m["xT"] = np.ascontiguousarray(x[b, seg].T)
    m["mo"] = shared["_mo"][b]
    m["cq"] = (ct * scale).astype(f32)
    m["sq"] = (st * scale).astype(f32)
    m["ck"] = ct
    m["sk"] = st
    return m


def _get_nc():
    if "nc" not in _CACHE:
        _CACHE["nc"] = build_nc()
    return _CACHE["nc"]


def kernel(**inputs) -> np.ndarray:
    nc = _get_nc()
    shared = _prep_shared(inputs)
    cores = [(b, s) for b in range(B) for s in range(4)]
    in_maps = [_prep_core(inputs, shared, b, s) for b, s in cores]
    from concourse import bass_utils
    import os
    res = bass_utils.run_bass_kernel_spmd(
        nc, in_maps, core_ids=list(range(NCORES)),
        trace=bool(os.environ.get("MAC_TRACE")))
    _CACHE["last_results"] = res
    out = np.empty((B, N, DIM), np.float32)
    for i, (b, s) in enumerate(cores):
        out[b, s * SEG:(s + 1) * SEG, :] = res.results[i]["yT"].T
    return out


# revision 66
# speedup vs baseline: 1.8173x; 1.0223x over previous
"""Trainium2 Bass kernel for nn_MACBlock (segmented attention + GEGLU FFN).

Sharding: 8 cores = 2 batches x 4 segments of 512 queries. The segment mask
makes attention block-diagonal (plus a 32-token always-visible prefix derived
from pooled memory + persistent memory), so each core is fully independent:
no collectives.

Layout: activations are kept feature-major (x^T [dim, tokens]) on-chip, so
every matmul contraction dim lands on partitions with zero transposes.
All heavy GEMMs run in bf16 (weights pre-cast on host, activations cast
on-chip): bf16 enables Fast Weight Load and avoids the fp32-HIGH power
throttle that halves the PE clock. PSUM accumulation stays fp32.
Scores are computed key-major ([keys, queries]); softmax is max-free; the
softmax denominator comes from a ones-column folded into the P@V stationary
operand (row DH of the same PSUM tile).
"""

import sys

if "/opt/trn_rl_repo" not in sys.path:
    sys.path.insert(0, "/opt/trn_rl_repo")

import numpy as np

B, N, DIM = 2, 2048, 1024
HEADS, DH = 16, 64
SEG = 512
NPM = NM = 16
PFX = NPM + NM          # 32 prefix keys
DFF = 2730
MFF = 22                # padded dff chunks
DFFP = MFF * 128        # 2816
KO = 8                  # 1024 / 128
P = 128
NCORES = 8
EPS = 1.1920929e-07

_CACHE = {}


def _f32r(ap):
    import concourse.mybir as mybir
    return ap.bitcast(mybir.dt.float32r)


def build_nc(reps=1):
    import concourse.bass as bass
    from concourse import bacc
    import concourse.tile as tile
    import concourse.mybir as mybir

    f32 = mybir.dt.float32
    bf16 = mybir.dt.bfloat16
    AF = mybir.ActivationFunctionType
    OP = mybir.AluOpType
    AX = mybir.AxisListType

    nc = bacc.Bacc("TRN2", target_bir_lowering=False, debug=False)

    dp = nc.declare_dram_parameter
    xT_d = dp("xT", [DIM, SEG], f32, isOutput=False)
    mo_d = dp("mo", [N, DIM], bf16, isOutput=False)
    cq_d = dp("cq", [P, SEG], f32, isOutput=False)
    sq_d = dp("sq", [P, SEG], f32, isOutput=False)
    ck_d = dp("ck", [P, SEG], f32, isOutput=False)
    sk_d = dp("sk", [P, SEG], f32, isOutput=False)
    mask_d = dp("maskD", [P, P], bf16, isOutput=False)
    rmat_d = dp("rmat", [P, P], bf16, isOutput=False)
    qkw_d = dp("qkw", [16, P, KO, P], bf16, isOutput=False)
    kvw_d = dp("kvw", [2, KO, P, DIM], bf16, isOutput=False)
    outw_d = dp("outw", [KO, P, KO, P], bf16, isOutput=False)
    w1a_d = dp("w1a", [MFF, P, KO, P], bf16, isOutput=False)
    w1g_d = dp("w1g", [MFF, P, KO, P], bf16, isOutput=False)
    w2_d = dp("w2", [KO, P, MFF, P], bf16, isOutput=False)
    mtw_d = dp("mtw", [KO, P, DIM], bf16, isOutput=False)
    pmv_d = dp("pmv", [HEADS, NPM, DH], bf16, isOutput=False)
    pmk_d = dp("pmk", [HEADS, DH, NPM], bf16, isOutput=False)
    b1a_d = dp("b1a", [P, MFF], f32, isOutput=False)
    b1g_d = dp("b1g", [P, MFF], f32, isOutput=False)
    b2_d = dp("b2", [P, KO], f32, isOutput=False)
    anw_d = dp("anw", [P, KO], f32, isOutput=False)
    fnw_d = dp("fnw", [P, KO], f32, isOutput=False)
    mpnw_d = dp("mpnw", [1, DIM], f32, isOutput=False)
    yT_d = dp("yT", [DIM, SEG], f32, isOutput=True)

    def _emit(nc):
      with tile.TileContext(nc) as tc, \
            nc.allow_low_precision(reason="bf16 matmul rounding"):
        from contextlib import ExitStack
        ctx = ExitStack()
        with ctx:
            persist = ctx.enter_context(tc.tile_pool(name="persist", bufs=1))
            wpool = ctx.enter_context(tc.tile_pool(name="wpool", bufs=6))
            kvpool = ctx.enter_context(tc.tile_pool(name="kvpool", bufs=6))
            w2pool = ctx.enter_context(tc.tile_pool(name="w2pool", bufs=4))
            mopool = ctx.enter_context(tc.tile_pool(name="mopool", bufs=6))
            rot = ctx.enter_context(tc.tile_pool(name="rot", bufs=2))
            epool = ctx.enter_context(tc.tile_pool(name="epool", bufs=3))
            pa = ctx.enter_context(tc.tile_pool(name="pa", bufs=3, space="PSUM"))
            psc = ctx.enter_context(tc.tile_pool(name="psc", bufs=2, space="PSUM"))
            ppo = ctx.enter_context(tc.tile_pool(name="ppo", bufs=3, space="PSUM"))

            cnt = [0]

            def pa_t():
                cnt[0] += 1
                return pa.tile([P, SEG], f32, tag="ps", name=f"pa{cnt[0]}")

            def psc_t():
                cnt[0] += 1
                return psc.tile([P, SEG], f32, tag="sc", name=f"sc{cnt[0]}")

            def ppo_t():
                cnt[0] += 1
                return ppo.tile([P, SEG], f32, tag="o", name=f"o{cnt[0]}")

            # ---------------- persistent SBUF tensors ----------------
            xT = persist.tile([P, KO, SEG], f32, tag="xT")       # x^T, later x1^T
            xnT = persist.tile([P, KO, SEG], bf16, tag="xnT")    # xn^T, later xn1^T
            kT = persist.tile([P, KO, SEG], bf16, tag="kT")      # roped k^T
            vA = persist.tile([P, 4, HEADS, DH + 1], bf16, tag="vA")  # v key-major
            vP = persist.tile([PFX, HEADS, DH + 1], bf16, tag="vP")   # prefix v rows
            kP = persist.tile([P, HEADS, PFX], bf16, tag="kP")   # prefix k^T @64*(h%2)
            oA = persist.tile([P, KO, SEG], bf16, tag="oA")      # attn o^T
            cq = persist.tile([P, SEG], f32, tag="cq")
            sq_ = persist.tile([P, SEG], f32, tag="sq")
            ck = persist.tile([P, SEG], f32, tag="ck")
            sk = persist.tile([P, SEG], f32, tag="sk")
            maskD = persist.tile([P, P], bf16, tag="maskD")
            rmat = persist.tile([P, P], bf16, tag="rmat")
            b1a = persist.tile([P, MFF], f32, tag="b1a")
            b1g = persist.tile([P, MFF], f32, tag="b1g")
            b2 = persist.tile([P, KO], f32, tag="b2")
            anw = persist.tile([P, KO], f32, tag="anw")
            fnw = persist.tile([P, KO], f32, tag="fnw")
            mpnw = persist.tile([1, DIM], f32, tag="mpnw")
            ones16 = persist.tile([1, 16], bf16, tag="o16")
            ones11 = persist.tile([1, 1], bf16, tag="o11")
            onesHH = persist.tile([DH, P], bf16, tag="oHH")  # pair bcast lhsT
            ones128b = persist.tile([P, 1], bf16, tag="o128b")   # lhsT K=128,M=1
            ones1xPb = persist.tile([1, P], bf16, tag="o1xPb")   # lhsT K=1,M=128
            ones1xP = persist.tile([1, P], f32, tag="o1xP")      # f32 variant
            pooledT = persist.tile([P, KO], bf16, tag="pooledT")
            memtokT = persist.tile([P, KO], bf16, tag="memtokT")
            mrow = persist.tile([1, 3 * DIM], f32, tag="mrow")
            mrowb = persist.tile([1, 3 * DIM], bf16, tag="mrowb")
            rrow = persist.tile([1, 2 * SEG], f32, tag="rrow")
            rrowb = persist.tile([1, SEG], bf16, tag="rrowb")
            epsc = persist.tile([P, 1], f32, tag="epsc")
            zeroc = persist.tile([P, 1], f32, tag="zeroc")

            dma = nc.sync.dma_start
            dma(out=cq, in_=cq_d[:])
            dma(out=sq_, in_=sq_d[:])
            dma(out=ck, in_=ck_d[:])
            dma(out=sk, in_=sk_d[:])
            dma(out=maskD, in_=mask_d[:])
            dma(out=rmat, in_=rmat_d[:])
            dma(out=b1a, in_=b1a_d[:])
            dma(out=b1g, in_=b1g_d[:])
            dma(out=b2, in_=b2_d[:])
            dma(out=anw, in_=anw_d[:])
            dma(out=fnw, in_=fnw_d[:])
            dma(out=mpnw, in_=mpnw_d[:])
            nc.vector.memset(ones16, 1.0)
            nc.vector.memset(ones11, 1.0)
            # pair-broadcast stationary: row 0 -> out rows 0-63, row 32 ->
            # out rows 64-127; all other contraction rows are zero, and the
            # matching moving-tile rows are zero-primed below, so they
            # contribute exactly 0 to the K=64 contraction.
            nc.vector.memset(onesHH, 0.0)
            nc.vector.memset(onesHH[0:1, 0:DH], 1.0)
            nc.vector.memset(onesHH[32:33, DH:P], 1.0)
            # zero-prime both rot-pool buffers of the pair-reciprocal moving
            # tile: its rows other than 0/32 are never written afterwards
            for _ in range(2):
                _rz = rot.tile([DH, SEG], bf16, tag="rvb")
                nc.vector.memset(_rz, 0.0)
            nc.vector.memset(ones128b, 1.0)
            nc.vector.memset(ones1xPb, 1.0)
            nc.vector.memset(ones1xP, 1.0)
            nc.vector.memset(epsc, EPS)
            nc.vector.memset(zeroc, 0.0)
            # denominator ones-columns in the P@V stationary operands
            nc.vector.memset(vA[:, :, :, DH:DH + 1], 1.0)
            nc.vector.memset(vP[:, :, DH:DH + 1], 1.0)
            for h in range(HEADS):
                hb = DH * (h % 2)
                dma(out=kP[hb:hb + DH, h, NPM:PFX], in_=pmk_d[h])
                dma(out=vP[NPM:PFX, h, 0:DH], in_=pmv_d[h])

            if True:
              for ko in range(KO):
                  dma(out=xT[:, ko, :], in_=xT_d[ko * P:(ko + 1) * P, :])

              mm = nc.tensor.matmul

              def rmsnorm_into(dst, src, w_sb, sq_tag):
                  """dst[:,ko,:] = src[:,ko,:] * w[:,ko] * rsqrt(mean_dim(src^2)+eps)"""
                  ss = psc_t()  # [1,512] slice used
                  sq8 = persist.tile([P, KO, SEG], bf16, tag=sq_tag, name="sq8")
                  for ko in range(KO):
                      nc.vector.tensor_mul(sq8[:, ko, :], src[:, ko, :],
                                           src[:, ko, :])
                      mm(ss[0:1, :], ones128b, sq8[:, ko, :],
                         start=(ko == 0), stop=(ko == KO - 1))
                  nc.scalar.activation(rrow[:, 0:SEG], ss[0:1, :], AF.Sqrt,
                                       bias=epsc[0:1], scale=1.0 / DIM)
                  nc.vector.reciprocal_approx_fast(out=rrow[:, SEG:2 * SEG],
                                                   in_=rrow[:, 0:SEG])
                  nc.vector.tensor_copy(out=rrowb, in_=rrow[:, SEG:2 * SEG])
                  bc = psc_t()  # broadcast rstd over 128 partitions
                  mm(bc, ones1xPb, rrowb,
                     start=True, stop=True)
                  for ko in range(KO):
                      nc.vector.scalar_tensor_tensor(
                          out=dst[:, ko, :], in0=src[:, ko, :],
                          scalar=w_sb[:, ko:ko + 1], in1=bc,
                          op0=OP.mult, op1=OP.mult)

              # ---------------- mem_out mean accumulation ----------------
              # [1,512] accumulator pair lives at partition rows 0 and 32 of
              # a single PSUM tile (matmul out base partition must be 0/32/64).
              # Allocated from ppo (idle until attention); the first steps are
              # emitted BEFORE the rmsnorm so the PE has work while the xT
              # DMA + square/sum chain resolves at kernel start.
              mean_ps = ppo_t()

              def mo_mean_step(t):
                  mot = mopool.tile([P, DIM], bf16, tag="mo", name="mot")
                  dma(out=mot, in_=mo_d[t * P:(t + 1) * P, :])
                  for half in range(2):
                      r = 32 * half
                      mm(mean_ps[r:r + 1, :], ones128b,
                         mot[:, half * SEG:(half + 1) * SEG],
                         start=(t == 0), stop=(t == 15))

              for t in range(4):
                  mo_mean_step(t)

              # ---------------- attn rmsnorm ----------------
              rmsnorm_into(xnT, xT, anw, "big16")
              qT = persist.tile([P, KO, SEG], bf16, tag="qT")    # roped,scaled q^T

              # pooled rmsnorm (pure ACT/DVE): emitted mid-qk-loop so its
              # serial latency hides under the remaining projection matmuls
              pooled_raw = mrow[:, 0:DIM]
              sqr = mrow[:, DIM:2 * DIM]
              pooled = mrowb[:, 2 * DIM:3 * DIM]  # bf16 row for transposes

              def pooled_chain():
                  for half in range(2):
                      r = 32 * half
                      nc.scalar.activation(_f32r(pooled_raw[:, half * SEG:(half + 1) * SEG]),
                                           mean_ps[r:r + 1, :], AF.Copy,
                                           scale=1.0 / N)
                  nc.vector.tensor_mul(_f32r(sqr), pooled_raw, pooled_raw)
                  nc.vector.reduce_sum(_f32r(sqr[:, 0:1]), sqr, axis=AX.X)
                  nc.scalar.activation(_f32r(sqr[:, 1:2]), sqr[:, 0:1], AF.Sqrt,
                                       bias=epsc[0:1], scale=1.0 / DIM)
                  nc.vector.reciprocal(_f32r(sqr[:, 2:3]), sqr[:, 1:2])
                  nc.vector.scalar_tensor_tensor(out=pooled, in0=pooled_raw,
                                                 scalar=sqr[:, 2:3], in1=mpnw,
                                                 op0=OP.mult, op1=OP.mult)

              # software-pipelined: the rope finish (rmat matmul + DVE
              # combine) for iteration m is emitted during iteration m+1 so
              # the ACT qraw copy never stalls the PE FIFO
              def rope_finish(ps, qraw, m):
                  is_q = m < 8
                  c_t, s_t = (cq, sq_) if is_q else (ck, sk)
                  dst = qT if is_q else kT
                  At = rot.tile([P, SEG], bf16, tag="ropeB")
                  nc.vector.tensor_mul(At, ps, c_t)  # before rps: frees ps
                  rps = pa_t()
                  mm(rps, rmat, qraw, start=True, stop=True)
                  Bt = rot.tile([P, SEG], bf16, tag="ropeA")
                  nc.vector.tensor_mul(Bt, rps, s_t)
                  nc.vector.tensor_add(dst[:, m % 8, :], At, Bt)

              pend = None
              for m in range(16):
                  wt = wpool.tile([P, KO, P], bf16, tag="w8")
                  dma(out=wt, in_=qkw_d[m])
                  ps = pa_t()
                  for ko in range(KO):
                      mm(ps, wt[:, ko], xnT[:, ko, :],
                         start=(ko == 0), stop=(ko == KO - 1))
                  qraw = rot.tile([P, SEG], bf16, tag="qraw")
                  nc.scalar.copy(qraw, ps)
                  if pend is not None:
                      rope_finish(*pend)
                  pend = (ps, qraw, m)
                  if 1 <= m < 7:
                      mo_mean_step(4 + 2 * (m - 1))
                      mo_mean_step(5 + 2 * (m - 1))
                  if m == 8:
                      pooled_chain()
              rope_finish(*pend)

              def stage_pT():
                  pT = pa_t()
                  for ko in range(KO):
                      mm(pT[:, ko:ko + 1], pooled[0:1, ko * P:(ko + 1) * P],
                         ones11, start=True, stop=True, skip_group_check=True)
                  nc.vector.tensor_copy(out=pooledT, in_=pT[:, 0:KO])

              kvvs = []  # v-proj weights, hoisted so DMAs overlap qk tail

              def load_kvv():
                  for half in range(2):
                      kvv = persist.tile([P, KO, SEG], bf16,
                                         tag=("kvv0" if half == 0 else "big16"))
                      for ko in range(KO):
                          dma(out=kvv[:, ko, :],
                              in_=kvw_d[1, ko, :, half * SEG:(half + 1) * SEG])
                      kvvs.append(kvv)

              mt_ps = []

              def stage_mt():
                  mt_ps.append(psc_t())
                  for ko in range(KO):
                      mtw_t = kvpool.tile([P, DIM], bf16, tag="kv")
                      dma(out=mtw_t, in_=mtw_d[ko])
                      for half in range(2):
                          r = 32 * half
                          mm(mt_ps[0][r:r + 1, :], pooledT[:, ko:ko + 1],
                             mtw_t[:, half * SEG:(half + 1) * SEG],
                             start=(ko == 0), stop=(ko == KO - 1))

              memtok = mrowb[:, 0:DIM]

              def stage_mT():
                  for half in range(2):
                      r = 32 * half
                      nc.scalar.activation(memtok[:, half * SEG:(half + 1) * SEG],
                                           mt_ps[0][r:r + 1, :], AF.Copy)
                  mT = pa_t()
                  for ko in range(KO):
                      mm(mT[:, ko:ko + 1], memtok[0:1, ko * P:(ko + 1) * P],
                         ones11, start=True, stop=True, skip_group_check=True)
                  nc.vector.tensor_copy(out=memtokT, in_=mT[:, 0:KO])

              kc_row = mrowb[:, DIM:2 * DIM]
              vc_row = mrowb[:, 2 * DIM:3 * DIM]

              def stage_kcvc(c):
                  r_ps = psc_t()
                  for ko in range(KO):
                      kv_t = kvpool.tile([P, DIM], bf16, tag="kv")
                      dma(out=kv_t, in_=kvw_d[c, ko])
                      for half in range(2):
                          r = 32 * half
                          mm(r_ps[r:r + 1, :], memtokT[:, ko:ko + 1],
                             kv_t[:, half * SEG:(half + 1) * SEG],
                             start=(ko == 0), stop=(ko == KO - 1))
                  row = kc_row if c == 0 else vc_row
                  for half in range(2):
                      r = 32 * half
                      nc.scalar.activation(row[:, half * SEG:(half + 1) * SEG],
                                           r_ps[r:r + 1, :], AF.Copy)

              def stage_kx_j(j):  # 2 heads per chunk
                  kx = pa_t()
                  mm(kx[:, 0:16], kc_row[0:1, j * P:(j + 1) * P],
                     ones16, start=True, stop=True,
                     skip_group_check=True)
                  nc.vector.tensor_copy(out=kP[0:DH, 2 * j, 0:NPM],
                                        in_=kx[0:DH, 0:16])
                  nc.vector.tensor_copy(out=kP[DH:P, 2 * j + 1, 0:NPM],
                                        in_=kx[DH:P, 0:16])

              def stage_vx_half(half):
                  vx = pa_t()
                  mm(vx[0:16, :], ones16,
                     vc_row[0:1, half * SEG:(half + 1) * SEG],
                     start=True, stop=True, skip_group_check=True)
                  nc.vector.tensor_copy(
                      out=vP[0:NPM, half * 8:(half + 1) * 8, 0:DH],
                      in_=vx[0:16, :].rearrange("p (h d) -> p h d", d=DH))

              load_kvv()  # all v-weight DMAs issued up front
              stage_pT()  # pooled ready ~3 qk-iterations ago
              stage_sched = {1: stage_mt, 3: stage_mT,
                             5: lambda: stage_kcvc(0),
                             6: lambda: stage_kcvc(1)}

              # ---------------- v projection (token-major), interleaved ----
              # (kx/vx stages are woven into the attention head loop below)
              for half in range(2):
                  kvv = kvvs[half]
                  for tc_ in range(4):
                      ps = pa_t()
                      for ko in range(KO):
                          mm(ps, xnT[:, ko, tc_ * P:(tc_ + 1) * P],
                             kvv[:, ko, :],
                             start=(ko == 0), stop=(ko == KO - 1))
                      nc.vector.tensor_copy(
                          out=vA[:, tc_, half * 8:(half + 1) * 8, 0:DH],
                          in_=ps.rearrange("p (h d) -> p h d", d=DH))
                      blk = half * 4 + tc_
                      if blk in stage_sched:
                          stage_sched[blk]()

              # ---------------- attention heads ----------------
              # Scores chunk c covers keys [cP,(c+1)P) x queries [cP,SEG)
              # (block-triangular). Chunks 1 and 3 share one PSUM tile /
              # exp pass: c1 at free 0:384 (queries 128:512), c3 at free
              # 384:512 (queries 384:512). The P@V stationary has a ones
              # column, so PSUM row DH is the softmax denominator.
              # The divide chain for a head PAIR (2h, 2h+1) is emitted two
              # heads later (software pipelining), with one reciprocal /
              # broadcast / copy serving both heads, so its serial ACT/DVE
              # latency never head-of-line-blocks the engine FIFOs.
              def divide_pair(h2, po_a, po_b):
                  ko_h = h2 // 2
                  drow2 = rot.tile([DH, SEG], f32, tag="dr")
                  nc.vector.tensor_copy(out=drow2[0:1, :],
                                        in_=po_a[DH:DH + 1, :])
                  nc.vector.tensor_copy(out=drow2[32:33, :],
                                        in_=po_b[DH:DH + 1, :])
                  rvf2 = rot.tile([DH, SEG], f32, tag="rv")
                  nc.vector.reciprocal_approx_fast(out=rvf2[0:33, :],
                                                   in_=drow2[0:33, :])
                  rvb2 = rot.tile([DH, SEG], bf16, tag="rvb")
                  nc.vector.tensor_copy(out=rvb2[0:1, :], in_=rvf2[0:1, :])
                  nc.vector.tensor_copy(out=rvb2[32:33, :], in_=rvf2[32:33, :])
                  bcp2 = psc_t()  # rows 0-63 = 1/d_a, rows 64-127 = 1/d_b
                  mm(bcp2, onesHH, rvb2,
                     start=True, stop=True, skip_group_check=True)
                  bcs2 = epool.tile([P, SEG], bf16, tag="bcs")
                  nc.vector.tensor_copy(out=bcs2, in_=bcp2)
                  nc.vector.tensor_mul(oA[0:DH, ko_h, :],
                                       po_a[0:DH, :], bcs2[0:DH, :])
                  nc.vector.tensor_mul(oA[DH:P, ko_h, :],
                                       po_b[0:DH, :], bcs2[DH:P, :])

              pend_pair = None
              po_prev = None
              for h in range(HEADS):
                  if h % 2 == 1 and pend_pair is not None:
                      # emitted BEFORE this head's po allocation so the
                      # 3-buf ppo rotation never recycles an unread tile
                      divide_pair(*pend_pair)
                      pend_pair = None
                  if h % 2 == 0:
                      stage_kx_j(h // 2)
                  if h == 0:
                      stage_vx_half(0)
                  if h == 6:
                      stage_vx_half(1)
                  ko_h, hf = h // 2, h % 2
                  qr = DH * hf
                  q_h = qT[qr:qr + DH, ko_h, :]
                  k_h = kT[qr:qr + DH, ko_h, :]
                  # prefix scores [32, 512]
                  scp = psc_t()
                  mm(scp[0:PFX, :], kP[qr:qr + DH, h, :], q_h,
                     start=True, stop=True, skip_group_check=True)
                  eP = epool.tile([PFX, SEG], bf16, tag="eP")
                  nc.scalar.activation(eP, scp[0:PFX, :], AF.Exp,
                                       bias=zeroc[0:PFX])
                  sc0 = pa_t()  # pa is near-idle during attention
                  mm(sc0[:, :], k_h[:, 0:P], q_h,
                     start=True, stop=True, skip_group_check=True)
                  e0 = epool.tile([P, SEG], bf16, tag="e0")
                  nc.scalar.activation(e0, sc0, AF.Exp, bias=zeroc)
                  nc.gpsimd.tensor_mul(e0[:, 0:P], e0[:, 0:P], maskD)
                  sc13 = psc_t()
                  mm(sc13[:, 0:384], k_h[:, P:2 * P], q_h[:, P:],
                     start=True, stop=True, skip_group_check=True)
                  mm(sc13[:, 384:512], k_h[:, 3 * P:4 * P], q_h[:, 3 * P:],
                     start=True, stop=True, skip_group_check=True)
                  eB = epool.tile([P, SEG], bf16, tag="eB")
                  nc.scalar.activation(eB, sc13, AF.Exp, bias=zeroc)
                  nc.gpsimd.tensor_mul(eB[:, 0:P], eB[:, 0:P], maskD)
                  nc.gpsimd.tensor_mul(eB[:, 384:512], eB[:, 384:512], maskD)
                  sc2 = pa_t()
                  mm(sc2[:, 0:256], k_h[:, 2 * P:3 * P], q_h[:, 2 * P:],
                     start=True, stop=True, skip_group_check=True)
                  e2 = epool.tile([P, 256], bf16, tag="e2")
                  nc.scalar.activation(e2, sc2[:, 0:256], AF.Exp, bias=zeroc)
                  nc.gpsimd.tensor_mul(e2[:, 0:P], e2[:, 0:P], maskD)
                  # P@V with ones-column: row DH = softmax denominator
                  po = ppo_t()  # dedicated pool so heads pipeline
                  mm(po[0:DH + 1, :], vP[:, h, :], eP,
                     start=True, stop=False, skip_group_check=True)
                  mm(po[0:DH + 1, 0:], vA[:, 0, h, :], e0,
                     start=False, stop=False, skip_group_check=True)
                  mm(po[0:DH + 1, P:], vA[:, 1, h, :], eB[:, 0:384],
                     start=False, stop=False, skip_group_check=True)
                  mm(po[0:DH + 1, 2 * P:], vA[:, 2, h, :], e2,
                     start=False, stop=False, skip_group_check=True)
                  mm(po[0:DH + 1, 3 * P:], vA[:, 3, h, :], eB[:, 384:512],
                     start=False, stop=True, skip_group_check=True)
                  if h % 2 == 0:
                      po_prev = po
                  else:
                      pend_pair = (h - 1, po_prev, po)
              divide_pair(*pend_pair)

              # ---------------- output projection + residual ----------------
              # FFN rmsnorm statistics are accumulated in the same loop so
              # the second norm adds no serial latency.
              ss2 = psc_t()
              sq8b = persist.tile([P, KO, SEG], bf16, tag="big16", name="sq8b")
              for m in range(KO):
                  wt = wpool.tile([P, KO, P], bf16, tag="w8")
                  dma(out=wt, in_=outw_d[m])
                  ps = pa_t()
                  for k in range(KO):
                      mm(ps, wt[:, k], oA[:, k, :],
                         start=(k == 0), stop=(k == KO - 1))
                  nc.vector.tensor_add(xT[:, m, :], ps, xT[:, m, :])  # x1
                  nc.vector.tensor_mul(sq8b[:, m, :], xT[:, m, :], xT[:, m, :])
                  mm(ss2[0:1, :], ones128b, sq8b[:, m, :],
                     start=(m == 0), stop=(m == KO - 1))

              # ---------------- FFN ----------------
              nc.scalar.activation(rrow[:, 0:SEG], ss2[0:1, :], AF.Sqrt,
                                   bias=epsc[0:1], scale=1.0 / DIM)
              nc.vector.reciprocal_approx_fast(out=rrow[:, SEG:2 * SEG],
                                               in_=rrow[:, 0:SEG])
              nc.vector.tensor_copy(out=rrowb, in_=rrow[:, SEG:2 * SEG])
              bc2 = psc_t()
              mm(bc2, ones1xPb, rrowb, start=True, stop=True)
              for ko in range(KO):
                  nc.vector.scalar_tensor_tensor(
                      out=xnT[:, ko, :], in0=xT[:, ko, :],
                      scalar=fnw[:, ko:ko + 1], in1=bc2,
                      op0=OP.mult, op1=OP.mult)
              u_parts = [qT, kT]  # reuse dead slots as u storage
              u_c = persist.tile([P, 6, SEG], bf16, tag="big16")

              def u_slice(k):
                  if k < 8:
                      return u_parts[0][:, k, :]
                  if k < 16:
                      return u_parts[1][:, k - 8, :]
                  return u_c[:, k - 16, :]

              for m in range(MFF):
                  wa = wpool.tile([P, KO, P], bf16, tag="w8")
                  dma(out=wa, in_=w1a_d[m])
                  wg = wpool.tile([P, KO, P], bf16, tag="w8")
                  dma(out=wg, in_=w1g_d[m])
                  psa = pa_t()
                  psg = ppo_t()  # ppo idle during FFN; full double-buffering
                  for ko in range(KO):
                      mm(psa, wa[:, ko], xnT[:, ko, :],
                         start=(ko == 0), stop=(ko == KO - 1))
                      mm(psg, wg[:, ko], xnT[:, ko, :],
                         start=(ko == 0), stop=(ko == KO - 1))
                  sig = rot.tile([P, SEG], f32, tag="ropeA")
                  nc.scalar.activation(sig, psg, AF.Sigmoid,
                                       bias=b1g[:, m:m + 1], scale=1.0)
                  silu = rot.tile([P, SEG], f32, tag="ropeB")
                  nc.vector.scalar_tensor_tensor(
                      out=silu, in0=psg, scalar=b1g[:, m:m + 1],
                      in1=sig, op0=OP.add, op1=OP.mult)
                  nc.vector.scalar_tensor_tensor(
                      out=u_slice(m), in0=psa, scalar=b1a[:, m:m + 1],
                      in1=silu, op0=OP.add, op1=OP.mult)

              for o in range(KO):
                  ps = pa_t()
                  for half in range(2):
                      w2t = w2pool.tile([P, 11, P], bf16, tag="w2")
                      dma(out=w2t, in_=w2_d[o][:, half * 11:(half + 1) * 11, :])
                      for k2 in range(11):
                          k = half * 11 + k2
                          mm(ps, w2t[:, k2], u_slice(k),
                             start=(k == 0), stop=(k == MFF - 1))
                  outT = persist.tile([P, KO, SEG], f32, tag="kvv0",
                                      name=f"outT{o}")
                  nc.vector.scalar_tensor_tensor(
                      out=outT[:, o, :], in0=ps, scalar=b2[:, o:o + 1],
                      in1=xT[:, o, :], op0=OP.add, op1=OP.add)
                  dma(out=yT_d[o * P:(o + 1) * P, :], in_=outT[:, o, :])

    for _rep in range(reps):
        _emit(nc)
    nc.compile()
    return nc


# ======================= host-side preparation =======================

def _prep_shared(inputs):
    import ml_dtypes
    f32 = np.float32
    bf16 = ml_dtypes.bfloat16
    qkv = np.asarray(inputs["to_qkv_w"], f32)
    shared = {}
    shared["qkw"] = np.ascontiguousarray(
        qkv[:, :2048].reshape(KO, P, 16, P).transpose(2, 1, 0, 3)).astype(bf16)
    shared["kvw"] = np.ascontiguousarray(
        np.stack([qkv[:, 1024:2048], qkv[:, 2048:3072]])
        .reshape(2, KO, P, DIM)).astype(bf16)
    shared["outw"] = np.ascontiguousarray(
        np.asarray(inputs["to_out_w"], f32)
        .reshape(KO, P, KO, P).transpose(2, 1, 0, 3)).astype(bf16)
    w1 = np.asarray(inputs["ff_w1"], f32)
    w1a = np.zeros((DIM, DFFP), f32)
    w1g = np.zeros((DIM, DFFP), f32)
    w1a[:, :DFF] = w1[:, :DFF]
    w1g[:, :DFF] = w1[:, DFF:]
    shared["w1a"] = np.ascontiguousarray(
        w1a.reshape(KO, P, MFF, P).transpose(2, 1, 0, 3)).astype(bf16)
    shared["w1g"] = np.ascontiguousarray(
        w1g.reshape(KO, P, MFF, P).transpose(2, 1, 0, 3)).astype(bf16)
    w2 = np.zeros((DFFP, DIM), f32)
    w2[:DFF] = np.asarray(inputs["ff_w2"], f32)
    shared["w2"] = np.ascontiguousarray(
        w2.reshape(MFF, P, KO, P).transpose(2, 1, 0, 3)).astype(bf16)
    shared["mtw"] = np.ascontiguousarray(
        np.asarray(inputs["to_mem_tokens_w"], f32).reshape(KO, P, DIM)).astype(bf16)
    pm = np.asarray(inputs["persist_mem"], f32)
    shared["pmv"] = np.ascontiguousarray(pm).astype(bf16)
    shared["pmk"] = np.ascontiguousarray(pm.transpose(0, 2, 1)).astype(bf16)
    b1 = np.asarray(inputs["ff_b1"], f32)
    b1a = np.zeros(DFFP, f32)
    b1g = np.zeros(DFFP, f32)
    b1a[:DFF] = b1[:DFF]
    b1g[:DFF] = b1[DFF:]
    shared["b1a"] = np.ascontiguousarray(b1a.reshape(MFF, P).T)
    shared["b1g"] = np.ascontiguousarray(b1g.reshape(MFF, P).T)
    shared["b2"] = np.ascontiguousarray(
        np.asarray(inputs["ff_b2"], f32).reshape(KO, P).T)
    shared["anw"] = np.ascontiguousarray(
        np.asarray(inputs["attn_norm_w"], f32).reshape(KO, P).T)
    shared["fnw"] = np.ascontiguousarray(
        np.asarray(inputs["ff_norm_w"], f32).reshape(KO, P).T)
    shared["mpnw"] = np.ascontiguousarray(
        np.asarray(inputs["mem_pool_norm_w"], f32).reshape(1, DIM))
    rl = np.zeros((P, P), f32)
    ii = np.arange(0, P, 2)
    rl[ii + 1, ii] = f32(-1.0)
    rl[ii, ii + 1] = f32(1.0)
    shared["rmat"] = rl.astype(bf16)
    shared["maskD"] = np.where(
        np.arange(P)[None, :] >= np.arange(P)[:, None], f32(1.0), f32(0.0)
    ).astype(bf16)

    # per-batch bf16 mem_out
    mo = np.asarray(inputs["mem_out"], f32)
    shared["_mo"] = [np.ascontiguousarray(mo[b]).astype(bf16) for b in range(B)]

    # rope tables, float32 math to match the reference
    pos = np.arange(N, dtype=f32)
    expo = (np.arange(0, DH, 2).astype(f32) / f32(DH)).astype(f32)
    inv = (f32(1.0) / np.power(f32(10000.0), expo)).astype(f32)
    ang = np.repeat(pos[:, None] * inv[None, :], 2, axis=1).astype(f32)
    cosf, sinf = np.cos(ang).astype(f32), np.sin(ang).astype(f32)
    scale = f32(DH ** -0.5)
    shared["_cos"], shared["_sin"], shared["_scale"] = cosf, sinf, scale
    return shared


def _prep_core(inputs, shared, b, s):
    f32 = np.float32
    x = np.asarray(inputs["x"], f32)
    cosf, sinf, scale = shared["_cos"], shared["_sin"], shared["_scale"]
    seg = slice(s * SEG, (s + 1) * SEG)
    ct = np.ascontiguousarray(np.tile(cosf[seg].T, (2, 1)))
    st = np.ascontiguousarray(np.tile(sinf[seg].T, (2, 1)))
    m = {k: v for k, v in shared.items() if not k.startswith("_")}
    m["xT"] = np.ascontiguousarray(x[b, seg].T)
    m["mo"] = shared["_mo"][b]
    m["cq"] = (ct * scale).astype(f32)
    m["sq"] = (st * scale).astype(f32)
    m["ck"] = ct
    m["sk"] = st
    return m


def _get_nc():
    if "nc" not in _CACHE:
        _CACHE["nc"] = build_nc()
    return _CACHE["nc"]


def kernel(**inputs) -> np.ndarray:
    nc = _get_nc()
    shared = _prep_shared(inputs)
    cores = [(b, s) for b in range(B) for s in range(4)]
    in_maps = [_prep_core(inputs, shared, b, s) for b, s in cores]
    from concourse import bass_utils
    import os
    res = bass_utils.run_bass_kernel_spmd(
        nc, in_maps, core_ids=list(range(NCORES)),
        trace=bool(os.environ.get("MAC_TRACE")))
    _CACHE["last_results"] = res
    out = np.empty((B, N, DIM), np.float32)
    for i, (b, s) in enumerate(cores):
        out[b, s * SEG:(s + 1) * SEG, :] = res.results[i]["yT"].T
    return out
